# revision 1
# baseline (speedup 1.0000x reference)
"""AttnBlock3d (GroupNorm + single-head self-attention + proj + residual)
on 8 Trainium2 NeuronCores.

Sharding: 8 shards = (batch sample b in 0..3) x (query-half h in 0..1).
Every core runs the SAME program (SPMD): the host permutes each sample's
N=4096 spatial columns so that the core's 2048 query positions come
first. GroupNorm / K / V are permutation-invariant in the column order,
and attention output for a query column does not depend on the ordering
of key columns, so the math is unchanged.

Key algebra (all exact; lets every big GEMM start without waiting for
the GroupNorm statistics):
  xn = A*x + Bvec per channel, A = inv_std*gamma, Bvec = beta - mean*A.
  The gamma factor is folded into the weights on the HOST
  (W' = W diag(gamma)), so q = Wq@xn + bq = inv_std*(Wq'@x) + bq2 with
  bq2 = u1 - mean*inv_std*u2 built from host vectors u1 = bq + Wq@beta,
  u2 = Wq@gamma. The scores expand as
     score[k,q] = inv^2 * s_raw[k,q] + inv*(bq2.Wk'x)[k]
                  + (terms constant along k)
  and softmax over k is invariant to anything constant along k, so only
  the [k]-indexed bias survives. That bias equals w_bias.x[:,k] with
  w_bias = h1 - mean*inv*h2 (h1/h2 host vectors), and is produced for
  free as a 257th output column of the V^T GEMM (x stationary). It
  enters the softmax through the ACT Exp per-partition bias operand;
  inv^2*SCALE enters through the per-partition scale operand.
  The V side is v = inv*v_raw + cvv; inv rides on the reciprocal
  broadcast (outer product with an inv-filled row) and cvv (2 host
  vectors + stats) is added per channel after the division.

Device-side layout:
  - scoresT[k, q] (k on partitions) so the softmax denominator is a
    ones-vector matmul on the PE and the AV contraction consumes
    scoresT directly with V^T stationary -> no transposes anywhere.
  - big GEMMs run as float32r (full PE rate at moving-dim >= 256),
    exp(attention) in bf16 for the AV contraction.
  - 1/sqrt(var+eps) = exp(-0.5*ln(var+eps)) keeps ACT in the
    natural_log_exp table set.
"""

import numpy as np
from contextlib import ExitStack

import bass_rust
import concourse.bass as bass
import concourse.tile as tile
from concourse import mybir
from concourse.bass_utils import run_bass_kernel_spmd

F32 = mybir.dt.float32
F32R = mybir.dt.float32r
BF16 = mybir.dt.bfloat16
AX = mybir.AxisListType
OP = mybir.AluOpType
AF = mybir.ActivationFunctionType

B, C, HH, WW, DD = 4, 256, 16, 16, 16
N = HH * WW * DD          # 4096 spatial positions per sample
NQ = N // 2               # 2048 query positions per core
P = 128                   # partitions
NCT = C // P              # 2 channel tiles
NKT = N // P              # 32 key tiles
QCH = 512                 # q-chunk width (one PSUM bank of fp32)
NQC = NQ // QCH           # 4 q chunks
XCH = 2                   # x DMA/stats chunks per channel tile
EPS = 1e-6
SCALE = float(C) ** -0.5  # 0.0625
TAIL_DEFER = 8            # kt steps into the next q-chunk before prev tail
USE_FP8 = True            # fp8e4m3 + DoubleRow for the AV/denominator MMs
FP8 = mybir.dt.float8e4


def _split_excess_waits(nc, cap=1):
    """walrus in this env rejects >1 sync wait per instruction; peel
    extras onto no-ops inserted before the offender on the same engine."""
    n = 0
    for f in nc.m.functions:
        for blk in f.blocks:
            insts = blk.instructions
            new_insts = []
            for inst in insts:
                si = inst.sync_info
                if si is not None and si.on_wait is not None and len(si.on_wait) > cap:
                    waits = list(si.on_wait)
                    extra, keep = waits[:-cap], waits[-cap:]
                    for j in range(0, len(extra), cap):
                        nop = mybir.InstNoOp(
                            name=f"{inst.name}_ws{j}", ins=[], outs=[]
                        )
                        nop.engine = inst.engine
                        nop.sync_info = bass_rust.SyncInfo(
                            on_wait=extra[j : j + cap], on_update=[]
                        )
                        new_insts.append(nop)
                    inst.sync_info = bass_rust.SyncInfo(
                        on_wait=keep, on_update=list(si.on_update)
                    )
                    n += 1
                new_insts.append(inst)
            if len(new_insts) != len(insts):
                insts[:] = new_insts
    return n


def _r(ap):
    return ap.bitcast(F32R)


def _f(ap):
    return ap.bitcast(F32)


def build_program(p3_repeat=1):
    nc = bass.Bass("TRN2", target_bir_lowering=False, debug=False)

    x_d = nc.dram_tensor("x", [C, N], F32R, kind="ExternalInput")
    wqt_d = nc.dram_tensor("wqt", [C, C], F32R, kind="ExternalInput")
    wkt_d = nc.dram_tensor("wkt", [C, C], F32R, kind="ExternalInput")
    wvt_d = nc.dram_tensor("wvt", [C, C], F32R, kind="ExternalInput")
    wpt_d = nc.dram_tensor("wpt", [C, C], F32R, kind="ExternalInput")
    h1_d = nc.dram_tensor("h1", [C, 1], F32R, kind="ExternalInput")
    h2_d = nc.dram_tensor("h2", [C, 1], F32R, kind="ExternalInput")
    v1_d = nc.dram_tensor("v1", [C, 1], F32, kind="ExternalInput")
    v2_d = nc.dram_tensor("v2", [C, 1], F32, kind="ExternalInput")
    cb_d = nc.dram_tensor("cb", [C, 1], F32, kind="ExternalInput")
    out_d = nc.dram_tensor("out", [C, NQ], F32, kind="ExternalOutput")

    with tile.TileContext(nc) as tc, ExitStack() as ctx:
        # ---------- persistent pools ----------
        consts = ctx.enter_context(tc.tile_pool(name="consts", bufs=1))
        qk_pool = ctx.enter_context(tc.tile_pool(name="qk", bufs=1))
        vt_pool = ctx.enter_context(tc.tile_pool(name="vt", bufs=NKT // 2))
        xb_pool = ctx.enter_context(tc.tile_pool(name="xb", bufs=1))
        x_pool = ctx.enter_context(tc.tile_pool(name="xio", bufs=1))
        exp_pool = ctx.enter_context(tc.tile_pool(name="expt", bufs=NKT // 2))

        def cload(dram_ap, shape, tag, dt=F32):
            t = consts.tile(shape, dt, tag=tag, name=tag)
            nc.sync.dma_start(t[:], dram_ap)
            return t

        def cload2(d, shape, tag, dt=F32):
            return [
                cload(d.ap()[ci * P : (ci + 1) * P, :], shape, f"{tag}{ci}", dt)
                for ci in range(NCT)
            ]

        # x first: it gates everything. Split into per-half tiles so the
        # first half (query columns) releases early.
        xA = [x_pool.tile([P, NQ], F32R, tag=f"xA{ci}", name=f"xA{ci}")
              for ci in range(NCT)]
        xB = [x_pool.tile([P, NQ], F32R, tag=f"xB{ci}", name=f"xB{ci}")
              for ci in range(NCT)]

        def xs(ci, lo, w):
            """f32r view of x columns [lo, lo+w) (must not straddle NQ)."""
            if lo < NQ:
                assert lo + w <= NQ
                return xA[ci][:, lo : lo + w]
            return xB[ci][:, lo - NQ : lo - NQ + w]

        # wv_ext: cols 0:256 gamma-scaled WvT, col 256 = h1, col 257 = h2
        # (host vectors) -> the V^T GEMM needs no statistics at all; its
        # two extra output columns are h1.x and h2.x, combined into the
        # per-key exp bias later on the DVE. Loaded FIRST: it gates the
        # first GEMM (V^T on the query-half x columns).
        wv_ext = []
        for ci in range(NCT):
            t = consts.tile([P, C + 2], F32R, tag=f"wv{ci}", name=f"wv{ci}")
            nc.sync.dma_start(t[:, 0:C], wvt_d.ap()[ci * P : (ci + 1) * P, :])
            nc.sync.dma_start(t[:, C : C + 1],
                              h1_d.ap()[ci * P : (ci + 1) * P, :])
            nc.sync.dma_start(t[:, C + 1 : C + 2],
                              h2_d.ap()[ci * P : (ci + 1) * P, :])
            wv_ext.append(t)
        for ci in range(NCT):
            nc.sync.dma_start(xA[ci][:], x_d.ap()[ci * P : (ci + 1) * P, 0:NQ])
        for ci in range(NCT):
            nc.sync.dma_start(xB[ci][:], x_d.ap()[ci * P : (ci + 1) * P, NQ:N])
        wqA = cload2(wqt_d, [P, C], "wqA", F32R)
        wkA = cload2(wkt_d, [P, C], "wkA", F32R)
        wpT = cload2(wpt_d, [P, C], "wpT", F32R)
        v1t = cload2(v1_d, [P, 1], "v1")
        v2t = cload2(v2_d, [P, 1], "v2")
        cbt = cload2(cb_d, [P, 1], "cb")

        ones_col = consts.tile([P, 1], F32, tag="ones_col", name="ones_col")
        nc.vector.memset(ones_col[:], 1.0)
        ones_fp8 = consts.tile([P, 32], FP8, tag="ones_fp8", name="ones_fp8")
        nc.vector.memset(ones_fp8[:], 1.0)
        ones_row = consts.tile([1, P], F32, tag="ones_row", name="ones_row")
        nc.vector.memset(ones_row[:], 1.0)

        q2 = qk_pool.tile([P, NCT * NQ], FP8, tag="q2", name="q2")
        k2 = qk_pool.tile([P, NCT * N], FP8, tag="k2", name="k2")
        vT = [vt_pool.tile([P, 2 * C], FP8, tag="vt", name="vt")
              for _ in range(NKT // 2)]
        xb = [xb_pool.tile([P, NQ], F32, tag=f"xb{ci}", name=f"xb{ci}")
              for ci in range(NCT)]
        bias_k = qk_pool.tile([P, NKT], F32, tag="bias_k", name="bias_k")
        # stats-derived broadcast columns:
        # [inv, -mean*inv, S*inv, S*inv^2, -S*mean*inv^2]
        mi_bc = consts.tile([P, 5], F32, tag="mi_bc", name="mi_bc")
        inv_row = consts.tile([1, P], F32, tag="inv_row", name="inv_row")
        cvv = [consts.tile([P, 1], F32, tag=f"cvv{ci}", name=f"cvv{ci}")
               for ci in range(NCT)]

        p12 = ExitStack()
        st_pool = p12.enter_context(tc.tile_pool(name="stats", bufs=1))
        ps_qkp = p12.enter_context(
            tc.tile_pool(name="ps_qkp", bufs=2, space="PSUM"))
        ps_vp = p12.enter_context(
            tc.tile_pool(name="ps_vp", bufs=2, space="PSUM"))

        # ---------- GroupNorm stats (DVE/ACT, off the PE path) ----------
        with tc.tile_pool(name="ps_tiny", bufs=1, space="PSUM") as ps_tiny:
            nch = NCT * 2
            s4 = st_pool.tile([P, nch], F32, tag="s4", name="s4")
            q4 = st_pool.tile([P, nch], F32, tag="q4", name="q4")
            scr = st_pool.tile([P, NQ], F32, tag="scr", name="scr")
            for ci in range(NCT):
                for xc, xt_ in ((0, xA[ci]), (1, xB[ci])):
                    j = ci * 2 + xc
                    chunk = _f(xt_[:])
                    nc.vector.reduce_sum(s4[:, j : j + 1], chunk, axis=AX.X)
                    nc.scalar.activation(scr[:], chunk, AF.Square,
                                         accum_out=q4[:, j : j + 1])

            ps_stat = ps_tiny.tile([1, 2 * nch], F32, tag="ps_stat",
                                   name="ps_stat")
            nc.tensor.matmul(ps_stat[:, 0:nch], ones_col[:], s4[:])
            nc.tensor.matmul(ps_stat[:, nch : 2 * nch], ones_col[:], q4[:])
            mom_sb = st_pool.tile([1, 2], F32, tag="mom", name="mom")
            nc.vector.tensor_reduce(mom_sb[:, 0:1], ps_stat[:, 0:nch],
                                    axis=AX.X, op=OP.add)
            nc.vector.tensor_reduce(mom_sb[:, 1:2], ps_stat[:, nch : 2 * nch],
                                    axis=AX.X, op=OP.add)

            inv_cn = 1.0 / float(C * N)
            mom2_sb = st_pool.tile([1, 2], F32, tag="mom2", name="mom2")
            nc.scalar.activation(mom2_sb[:], mom_sb[:], AF.Copy, scale=inv_cn)
            mean_sb = mom2_sb[:, 0:1]
            msq_sb = st_pool.tile([1, 1], F32, tag="msq", name="msq")
            nc.vector.tensor_tensor(msq_sb[:], mean_sb, mean_sb, OP.mult)
            var_sb = st_pool.tile([1, 1], F32, tag="var", name="var")
            nc.vector.tensor_tensor(var_sb[:], mom2_sb[:, 1:2], msq_sb[:],
                                    OP.subtract)
            eps_t = st_pool.tile([1, 1], F32, tag="eps", name="eps")
            nc.vector.memset(eps_t[:], EPS)
            lnv_sb = st_pool.tile([1, 1], F32, tag="lnv", name="lnv")
            nc.scalar.activation(lnv_sb[:], var_sb[:], AF.Ln, bias=eps_t[:])
            mi_sb = st_pool.tile([1, 5], F32, tag="mi", name="mi")
            inv_c = mi_sb[:, 0:1]
            nc.scalar.activation(inv_c, lnv_sb[:], AF.Exp, scale=-0.5)
            nc.vector.tensor_tensor(mi_sb[:, 1:2], mean_sb, inv_c, OP.mult)
            nc.vector.tensor_scalar(mi_sb[:, 1:2], mi_sb[:, 1:2], -1.0, None,
                                    op0=OP.mult)
            nc.vector.tensor_scalar(mi_sb[:, 2:3], inv_c, SCALE, None,
                                    op0=OP.mult)
            nc.vector.tensor_tensor(mi_sb[:, 3:4], mi_sb[:, 2:3], inv_c,
                                    OP.mult)
            nc.vector.tensor_tensor(mi_sb[:, 4:5], mi_sb[:, 1:2],
                                    mi_sb[:, 2:3], OP.mult)

            ps_bc5 = ps_tiny.tile([P, 5], F32, tag="ps_bc5", name="ps_bc5")
            nc.tensor.matmul(ps_bc5[:], ones_row[:], mi_sb[:])
            nc.vector.tensor_copy(mi_bc[:], ps_bc5[:])
            nc.vector.tensor_scalar(_r(inv_row[:]), ones_row[:],
                                    mi_sb[:, 0:1], None, op0=OP.mult)
        minv_neg = mi_bc[:, 1:2]
        si_bc = mi_bc[:, 2:3]
        si2_bc = mi_bc[:, 3:4]
        m2n_bc = mi_bc[:, 4:5]
        for ci in range(NCT):
            nc.vector.tensor_scalar(cvv[ci][:], v2t[ci][:], minv_neg,
                                    v1t[ci][:], op0=OP.mult, op1=OP.add)

        # ---------- GEMM emission helpers ----------
        def emit_vt(kt):
            lo = kt * P
            pv = ps_vp.tile([P, C + 2], F32, tag="ps_v", name="ps_v")
            for ci in range(NCT):
                nc.tensor.matmul(pv[:], xs(ci, lo, P), wv_ext[ci][:],
                                 start=(ci == 0), stop=(ci == NCT - 1))
            nc.vector.tensor_copy(
                vT[kt // 2][:, (kt % 2) * C : (kt % 2 + 1) * C], pv[:, 0:C])
            # bias_k[kt] = S*inv*(h1.x) - S*mean*inv^2*(h2.x)
            t1 = st_pool.tile([P, 1], F32, tag="bkt1", name="bkt1", bufs=2)
            nc.vector.tensor_scalar(t1[:], pv[:, C : C + 1], si_bc, None,
                                    op0=OP.mult)
            nc.vector.scalar_tensor_tensor(
                bias_k[:, kt : kt + 1], pv[:, C + 1 : C + 2], m2n_bc, t1[:],
                op0=OP.mult, op1=OP.add)

        def emit_qk(which, oc, col):
            w = wqA if which == "q" else wkA
            dst, width = (q2, NQ) if which == "q" else (k2, N)
            pk = ps_qkp.tile([P, QCH], F32, tag="ps_qk", name="ps_qk")
            for ci in range(NCT):
                nc.tensor.matmul(pk[:],
                                 w[ci][:, oc * P : (oc + 1) * P],
                                 xs(ci, col, QCH),
                                 start=(ci == 0), stop=(ci == NCT - 1))
            nc.vector.tensor_copy(
                dst[:, oc * width + col : oc * width + col + QCH], pk[:])

        # ---------- pre-sweep: V^T 0..15, Q, K chunk 0 ----------
        for kt in range(16):
            emit_vt(kt)
        for qc in range(NQC):
            for oc in range(NCT):
                emit_qk("q", oc, qc * QCH)
        for oc in range(NCT):
            emit_qk("k", oc, 0)
        # ---------- scores+exp sweep, leftover GEMMs injected ----------
        # 1024-wide exps (per key tile x query half) keep ACT saturated;
        # remaining K chunks and V^T tiles ride in the PE slack.
        exp_tiles = [None] * (NKT // 2)
        with tc.tile_pool(name="ps_s", bufs=2, space="PSUM") as ps_s:
            for kt in range(NKT):
                if kt < N // QCH - 1:          # K chunks 1..7 at kt 0..6
                    for oc in range(NCT):
                        emit_qk("k", oc, (kt + 1) * QCH)
                if 7 <= kt < 23:               # V^T 16..31 at kt 7..22
                    emit_vt(kt + 9)
                k3 = k2[:].rearrange("p (j n) -> p j n",
                                     j=2)[:, :, kt * P : (kt + 1) * P]
                q3 = q2[:].rearrange("p (j n) -> p j n", j=2)
                for half in range(2):
                    ps = ps_s.tile([P, 2 * QCH], F32, tag="s", name="s")
                    for qh in range(2):
                        qcol = half * 2 * QCH + qh * QCH
                        nc.tensor.matmul(
                            ps[:, qh * QCH : (qh + 1) * QCH],
                            k3, q3[:, :, qcol : qcol + QCH],
                            skip_group_check=True,
                            perf_mode=mybir.MatmulPerfMode.DoubleRow)
                    if kt % 2 == 0 and half == 0:
                        exp_tiles[kt // 2] = exp_pool.tile(
                            [P, 2 * NQ], FP8, tag="expt", name="expt")
                    lo = (kt % 2) * NQ + half * 2 * QCH
                    nc.scalar.activation(
                        exp_tiles[kt // 2][:, lo : lo + 2 * QCH],
                        ps[:], AF.Exp, scale=si2_bc,
                        bias=bias_k[:, kt : kt + 1])
        # xb residual (first needed by the qc0 tail, ~70us in; emitted
        # late so it stays off the DVE path that gates the exp sweep)
        for oc in range(NCT):
            nc.vector.tensor_scalar(xb[oc][:], _f(xA[oc][:]), cbt[oc][:],
                                    None, op0=OP.add)
        p12.close()

        # ---------- AV (fp8 DoubleRow) + tails ----------
        with tc.tile_pool(name="att_sb", bufs=2) as att_pool, \
             tc.tile_pool(name="out_sb", bufs=4) as out_pool, \
             tc.tile_pool(name="ps_h", bufs=1, space="PSUM") as ps_h, \
             tc.tile_pool(name="ps_d", bufs=2, space="PSUM") as ps_d, \
             tc.tile_pool(name="ps_tail", bufs=2, space="PSUM") as ps_tail:

            def make_tail(qc, ph, pd):
                qsl = slice(qc * QCH, (qc + 1) * QCH)

                def tail():
                    rec = att_pool.tile([1, QCH], F32, tag="rec", name="rec")
                    with nc.allow_low_precision(reason="f32r fp32-width"):
                        nc.vector.reciprocal(_r(rec[:]), pd[:])
                    pbc = ps_tail.tile([P, QCH], F32, tag="tail", name="bc")
                    nc.tensor.matmul(pbc[:], _r(inv_row[:]), _r(rec[:]),
                                     skip_group_check=True)
                    rec_bc = att_pool.tile([P, QCH], F32, tag="rec_bc",
                                           name="rec_bc")
                    nc.vector.tensor_copy(rec_bc[:], pbc[:])

                    h_sb = []
                    for ct in range(NCT):
                        h = att_pool.tile([P, QCH], F32, tag=f"hsb{ct}",
                                          name=f"hsb{ct}")
                        nc.vector.tensor_tensor(_r(h[:]), ph[ct][:],
                                                rec_bc[:], OP.mult)
                        nc.vector.tensor_scalar(_r(h[:]), h[:], cvv[ct][:],
                                                None, op0=OP.add)
                        h_sb.append(h)

                    for oc in range(NCT):
                        po = ps_tail.tile([P, QCH], F32, tag="tail", name="o")
                        for ci in range(NCT):
                            nc.tensor.matmul(
                                po[:],
                                wpT[ci][:, oc * P : (oc + 1) * P],
                                _r(h_sb[ci][:]),
                                start=(ci == 0), stop=(ci == NCT - 1),
                                skip_group_check=True)
                        ot = out_pool.tile([P, QCH], F32, tag="ot", name="ot")
                        nc.vector.tensor_tensor(ot[:], po[:], xb[oc][:, qsl],
                                                OP.add)
                        nc.sync.dma_start(
                            out_d.ap()[oc * P : (oc + 1) * P, qsl], ot[:])
                return tail

            NP2 = NKT // 2
            ones3 = ones_fp8[:].rearrange("p (j o) -> p j o", j=2)[:, :, 0:1]
            pending_tail = None
            for qc_rep in range(NQC * p3_repeat):
                qc = qc_rep % NQC
                ph = [ps_h.tile([P, QCH], F32, tag=f"h{ct}", name=f"h{ct}")
                      for ct in range(NCT)]
                pd = ps_d.tile([1, QCH], F32, tag="d", name="d")
                for p in range(NP2):
                    first, last = p == 0, p == NP2 - 1
                    et3 = exp_tiles[p].rearrange(
                        "p (j q) -> p j q",
                        j=2)[:, :, qc * QCH : (qc + 1) * QCH]
                    vt3 = vT[p].rearrange("p (j c) -> p j c", j=2)
                    for ct in range(NCT):
                        nc.tensor.matmul(
                            ph[ct][:],
                            vt3[:, :, ct * P : (ct + 1) * P],
                            et3[:],
                            start=first, stop=last, skip_group_check=True,
                            perf_mode=mybir.MatmulPerfMode.DoubleRow)
                    nc.tensor.matmul(
                        pd[:], ones3, et3[:],
                        start=first, stop=last, skip_group_check=True,
                        perf_mode=mybir.MatmulPerfMode.DoubleRow)
                    if p == 2 and pending_tail is not None:
                        pending_tail()
                        pending_tail = None
                pending_tail = make_tail(qc, ph, pd)
            pending_tail()

    _split_excess_waits(nc)
    return nc


def make_in_maps(x, norm_gamma, norm_beta, qkv_w, qkv_b, proj_w, proj_b):
    f = np.float32
    d = np.float64
    qkv_w = np.asarray(qkv_w, dtype=d)
    qkv_b = np.asarray(qkv_b, dtype=d)
    proj_w = np.asarray(proj_w, dtype=d)
    proj_b = np.asarray(proj_b, dtype=d)
    g = np.asarray(norm_gamma, dtype=d)
    beta = np.asarray(norm_beta, dtype=d)
    Wq, Wk, Wv = qkv_w[0:C], qkv_w[C : 2 * C], qkv_w[2 * C : 3 * C]
    bq, bk, bv = qkv_b[0:C], qkv_b[C : 2 * C], qkv_b[2 * C : 3 * C]

    wqT = (Wq.T * g[:, None])          # [c_in, c_out], rows scaled by gamma
    wkT = (Wk.T * g[:, None])
    wvT = (Wv.T * g[:, None])
    u1 = bq + Wq @ beta
    u2 = Wq @ g
    h1 = wkT @ u1
    h2 = wkT @ u2
    v1 = Wv @ beta
    v2 = Wv @ g
    cb = proj_w @ bv + proj_b

    def col(a):
        return np.ascontiguousarray(a.reshape(C, 1), dtype=f)

    shared = {
        "wqt": np.ascontiguousarray(wqT, dtype=f),
        "wkt": np.ascontiguousarray(wkT, dtype=f),
        "wvt": np.ascontiguousarray(wvT, dtype=f),
        "wpt": np.ascontiguousarray(proj_w.T, dtype=f),
        "h1": col(h1), "h2": col(h2), "v1": col(v1), "v2": col(v2),
        "cb": col(cb),
    }
    in_maps = []
    xf = np.asarray(x, dtype=f).reshape(B, C, N)
    for core in range(8):
        b, h = divmod(core, 2)
        xs = xf[b]
        if h == 1:
            xs = np.concatenate([xs[:, NQ:], xs[:, :NQ]], axis=1)
        in_maps.append({"x": np.ascontiguousarray(xs), **shared})
    return in_maps


def assemble_output(results):
    out = np.empty((B, C, N), dtype=np.float32)
    for core in range(8):
        b, h = divmod(core, 2)
        out[b][:, h * NQ : (h + 1) * NQ] = results[core]["out"]
    return out.reshape(B, C, HH, WW, DD)


_PROGRAM = None
_N_CALLS = 0
_RUNNER = None


def get_program():
    global _PROGRAM
    if _PROGRAM is None:
        _PROGRAM = build_program()
    return _PROGRAM


def _build_cached_runner(nc):
    """Persistent jitted executor (same execution path that
    run_bass_kernel_spmd takes under axon, via bass2jax/PJRT) so repeat
    kernel() calls skip the multi-minute neuronx-cc recompile."""
    import jax
    from jax.experimental.shard_map import shard_map
    from jax.sharding import Mesh, PartitionSpec
    from concourse import bass2jax

    bass2jax.install_neuronx_cc_hook()
    n_cores = 8
    partition_name = (nc.partition_id_tensor.name
                      if nc.partition_id_tensor else None)
    in_names, out_names, out_avals, zero_outs = [], [], [], []
    for alloc in nc.m.functions[0].allocations:
        if not isinstance(alloc, mybir.MemoryLocationSet):
            continue
        name = alloc.memorylocations[0].name
        if alloc.kind == "ExternalInput":
            if name != partition_name:
                in_names.append(name)
        elif alloc.kind == "ExternalOutput":
            out_names.append(name)
            shape = tuple(alloc.tensor_shape)
            dtype = mybir.dt.np(alloc.dtype)
            out_avals.append(jax.core.ShapedArray(shape, dtype))
            zero_outs.append(np.zeros(shape, dtype))
    n_params = len(in_names)
    all_in_names = list(in_names) + list(out_names)
    if partition_name is not None:
        all_in_names.append(partition_name)

    def _body(*args):
        operands = list(args)
        if partition_name is not None:
            operands.append(bass2jax.partition_id_tensor())
        outs = bass2jax._bass_exec_p.bind(
            *operands,
            out_avals=tuple(out_avals),
            in_names=tuple(all_in_names),
            out_names=tuple(out_names),
            lowering_input_output_aliases=(),
            sim_require_finite=True,
            sim_require_nnan=True,
            nc=nc,
        )
        return tuple(outs)

    devices = jax.devices()[:n_cores]
    mesh = Mesh(np.asarray(devices), ("core",))
    n_outs = len(out_names)
    fn = jax.jit(
        shard_map(_body, mesh=mesh,
                  in_specs=(PartitionSpec("core"),) * (n_params + n_outs),
                  out_specs=(PartitionSpec("core"),) * n_outs,
                  check_rep=False),
        keep_unused=True,
    )

    def run(in_maps):
        per_core = [[np.asarray(m[name]) for name in in_names]
                    for m in in_maps]
        concat_in = [
            np.concatenate([per_core[c][i] for c in range(n_cores)], axis=0)
            for i in range(n_params)
        ]
        concat_zeros = [
            np.zeros((n_cores * z.shape[0], *z.shape[1:]), z.dtype)
            for z in zero_outs
        ]
        out_arrs = fn(*concat_in, *concat_zeros)
        return [
            {name: np.asarray(out_arrs[i]).reshape(
                n_cores, *out_avals[i].shape)[c]
             for i, name in enumerate(out_names)}
            for c in range(n_cores)
        ]

    return run


def kernel(x, norm_gamma, norm_beta, qkv_w, qkv_b, proj_w, proj_b):
    global _N_CALLS, _RUNNER
    nc = get_program()
    in_maps = make_in_maps(x, norm_gamma, norm_beta, qkv_w, qkv_b,
                           proj_w, proj_b)
    _N_CALLS += 1
    if _N_CALLS == 1:
        res = run_bass_kernel_spmd(nc, in_maps, core_ids=list(range(8)))
        return assemble_output(res.results)
    if _RUNNER is None:
        _RUNNER = _build_cached_runner(nc)
    return assemble_output(_RUNNER(in_maps))



# revision 5
# speedup vs baseline: 1.0637x; 1.0637x over previous
"""AttnBlock3d (GroupNorm + single-head self-attention + proj + residual)
on 8 Trainium2 NeuronCores.

Sharding: 8 shards = (batch sample b in 0..3) x (query-half h in 0..1).
Every core runs the SAME program (SPMD): the host permutes each sample's
N=4096 spatial columns so that the core's 2048 query positions come
first. GroupNorm / K / V are permutation-invariant in the column order,
and attention output for a query column does not depend on the ordering
of key columns, so the math is unchanged.

Key algebra (all exact; lets every big GEMM start without waiting for
the GroupNorm statistics):
  xn = A*x + Bvec per channel, A = inv_std*gamma, Bvec = beta - mean*A.
  The gamma factor is folded into the weights on the HOST
  (W' = W diag(gamma)), so q = Wq@xn + bq = inv_std*(Wq'@x) + bq2.
  Softmax over k is invariant to anything constant along k, so only the
  [k]-indexed part of the score bias survives; it comes out of two
  extra output columns of the V^T GEMM (h1.x, h2.x) combined with the
  stats. inv^2*SCALE enters through the ACT Exp per-partition scale.
  The V-side affine (v = inv*v_raw + cvv) is folded THROUGH the proj:
  out = WpT@(ph * inv/denom) + [Wp@cvv + Wp@bv + bp + x], with the
  channel constant dcc = dc1 + (-mean*inv)*pc2 built from host vectors
  dc1 = Wp@(bv + Wv@beta) + bp and pc2 = Wp@Wv@gamma, pre-added into
  the residual tiles.

Schedule (q-major two-phase sweep):
  - prologue: 2 packed weight DMAs + 8 x-piece DMAs; GroupNorm moments
    via PE column-sum matmuls + ACT Squares (both idle then); Q(H0)/K0
    GEMMs and fp8 copies so the first exp fires as soon as the stats
    chain resolves.
  - phase A: for kt in 0..31: scoresT(kt, query-half H0) -> 1024-wide
    Exp. Leftover K chunks / V^T tiles / Q(H1) GEMMs ride the PE+DVE
    slack early in A; AV chain for qc0 runs kt>=16 (PSUM frees then);
  - phase B: same over H1; AV for qc1 (burst) + qc2 (paced) and the
    divide/proj/residual tails for qc0..2 all inside the sweep.
  - epilogue: only qc3's AV + tail.
  PSUM: scores 2x[128,1024] (4 banks) + AV chain ph0,ph1,pd (3) +
  proj po (1) = 8, with prologue pools (stats, qkv) scoped to close
  before the chain/proj pools open.
"""

import numpy as np
from contextlib import ExitStack

import bass_rust
import concourse.bass as bass
import concourse.tile as tile
from concourse import mybir
from concourse.bass_utils import run_bass_kernel_spmd

F32 = mybir.dt.float32
F32R = mybir.dt.float32r
BF16 = mybir.dt.bfloat16
FP8 = mybir.dt.float8e4
AX = mybir.AxisListType
OP = mybir.AluOpType
AF = mybir.ActivationFunctionType

B, C, HH, WW, DD = 4, 256, 16, 16, 16
N = HH * WW * DD          # 4096 spatial positions per sample
NQ = N // 2               # 2048 query positions per core
P = 128                   # partitions
NCT = C // P              # 2 channel tiles
NKT = N // P              # 32 key tiles
QCH = 512                 # q-chunk width (one PSUM bank of fp32)
NQC = NQ // QCH           # 4 q chunks
EPS = 1e-6
SCALE = float(C) ** -0.5  # 0.0625
PACKW = 258 + 3 * C + 2   # wv_ext | wq | wk | wp | dc1 | pc2


def _split_excess_waits(nc, cap=1):
    """walrus in this env rejects >1 sync wait per instruction; peel
    extras onto no-ops inserted before the offender on the same engine."""
    n = 0
    for f in nc.m.functions:
        for blk in f.blocks:
            insts = blk.instructions
            new_insts = []
            for inst in insts:
                si = inst.sync_info
                if si is not None and si.on_wait is not None and len(si.on_wait) > cap:
                    waits = list(si.on_wait)
                    extra, keep = waits[:-cap], waits[-cap:]
                    for j in range(0, len(extra), cap):
                        nop = mybir.InstNoOp(
                            name=f"{inst.name}_ws{j}", ins=[], outs=[]
                        )
                        nop.engine = inst.engine
                        nop.sync_info = bass_rust.SyncInfo(
                            on_wait=extra[j : j + cap], on_update=[]
                        )
                        new_insts.append(nop)
                    inst.sync_info = bass_rust.SyncInfo(
                        on_wait=keep, on_update=list(si.on_update)
                    )
                    n += 1
                new_insts.append(inst)
            if len(new_insts) != len(insts):
                insts[:] = new_insts
    return n


def _r(ap):
    return ap.bitcast(F32R)


def _f(ap):
    return ap.bitcast(F32)


def build_program():
    nc = bass.Bass("TRN2", target_bir_lowering=False, debug=False)

    x_d = nc.dram_tensor("x", [C, N], F32R, kind="ExternalInput")
    w_d = nc.dram_tensor("wpack", [C, PACKW], F32R, kind="ExternalInput")
    out_d = nc.dram_tensor("out", [C, NQ], F32, kind="ExternalOutput")

    with tile.TileContext(nc) as tc, ExitStack() as ctx:
        # ---------- persistent pools ----------
        consts = ctx.enter_context(tc.tile_pool(name="consts", bufs=1))
        qk_pool = ctx.enter_context(tc.tile_pool(name="qk", bufs=1))
        vt_pool = ctx.enter_context(tc.tile_pool(name="vt", bufs=NKT // 2))
        xb_pool = ctx.enter_context(tc.tile_pool(name="xb", bufs=1))
        x_pool = ctx.enter_context(tc.tile_pool(name="xio", bufs=1))
        exp_pool = ctx.enter_context(tc.tile_pool(name="expt", bufs=NKT // 2))

        # packed weights: 2 DMAs total (one per channel tile)
        wpk = [consts.tile([P, PACKW], F32R, tag=f"wpk{ci}", name=f"wpk{ci}")
               for ci in range(NCT)]
        for ci in range(NCT):
            nc.sync.dma_start(wpk[ci][:], w_d.ap()[ci * P : (ci + 1) * P, :])
        wv_ext = [wpk[ci][:, 0:258] for ci in range(NCT)]
        wqA = [wpk[ci][:, 258 : 258 + C] for ci in range(NCT)]
        wkA = [wpk[ci][:, 258 + C : 258 + 2 * C] for ci in range(NCT)]
        wpT = [wpk[ci][:, 258 + 2 * C : 258 + 3 * C] for ci in range(NCT)]
        dc1 = [_f(wpk[ci][:, 258 + 3 * C : 259 + 3 * C]) for ci in range(NCT)]
        pc2 = [_f(wpk[ci][:, 259 + 3 * C : 260 + 3 * C]) for ci in range(NCT)]

        # x in 8 pieces of [128, 1024]: both channel tiles of the query
        # half first so Q/V^T GEMMs and stats start at ~1/4 of the load.
        xA = [x_pool.tile([P, NQ], F32R, tag=f"xA{ci}", name=f"xA{ci}")
              for ci in range(NCT)]
        xB = [x_pool.tile([P, NQ], F32R, tag=f"xB{ci}", name=f"xB{ci}")
              for ci in range(NCT)]
        x_pieces = []  # (ci, tile, col_lo) in DMA order
        for half, xt in ((0, xA), (1, xB)):
            for sub in range(2):
                for ci in range(NCT):
                    lo = sub * (NQ // 2)
                    nc.sync.dma_start(
                        xt[ci][:, lo : lo + NQ // 2],
                        x_d.ap()[ci * P : (ci + 1) * P,
                                 half * NQ + lo : half * NQ + lo + NQ // 2])
                    x_pieces.append((ci, xt[ci], lo))

        def xs(ci, lo, w):
            """f32r view of x columns [lo, lo+w) (must not straddle NQ)."""
            if lo < NQ:
                assert lo + w <= NQ
                return xA[ci][:, lo : lo + w]
            return xB[ci][:, lo - NQ : lo - NQ + w]

        ones_col = consts.tile([P, 1], F32, tag="ones_col", name="ones_col")
        nc.vector.memset(ones_col[:], 1.0)
        ones_fp8 = consts.tile([P, 32], FP8, tag="ones_fp8", name="ones_fp8")
        nc.vector.memset(ones_fp8[:], 1.0)
        ones_row = consts.tile([1, P], F32, tag="ones_row", name="ones_row")
        nc.vector.memset(ones_row[:], 1.0)

        q2 = qk_pool.tile([P, NCT * NQ], FP8, tag="q2", name="q2")
        k2 = qk_pool.tile([P, NCT * N], FP8, tag="k2", name="k2")
        vT = [vt_pool.tile([P, 2 * C], FP8, tag="vt", name="vt")
              for _ in range(NKT // 2)]
        xbd = [xb_pool.tile([P, NQ], F32, tag=f"xb{ci}", name=f"xb{ci}")
               for ci in range(NCT)]
        hx = qk_pool.tile([P, 2 * NKT], F32, tag="hx", name="hx")
        hx3 = hx[:].rearrange("p (c k) -> p c k", c=2)
        bias_k = qk_pool.tile([P, NKT], F32, tag="bias_k", name="bias_k")
        # stats-derived broadcast columns:
        # [inv, -mean*inv, S*inv, S*inv^2, -S*mean*inv^2]
        mi_bc = consts.tile([P, 5], F32, tag="mi_bc", name="mi_bc")
        inv_row = consts.tile([1, P], F32, tag="inv_row", name="inv_row")
        dcc = [consts.tile([P, 1], F32, tag=f"dcc{ci}", name=f"dcc{ci}")
               for ci in range(NCT)]

        p_pre = ExitStack()
        st_pool = p_pre.enter_context(tc.tile_pool(name="stats", bufs=1))
        ps_st = p_pre.enter_context(
            tc.tile_pool(name="ps_st", bufs=1, space="PSUM"))
        ps_qkp = p_pre.enter_context(
            tc.tile_pool(name="ps_qkp", bufs=2, space="PSUM"))

        # ---------- GroupNorm moments ----------
        # column sums on the PE (idle in the prologue): 16 accumulating
        # [1,512] matmuls in x-piece DMA order; sums of squares on the
        # ACT (also idle) with the free-dim accumulator. Stats for piece
        # i are interleaved with the prologue GEMMs so at most two
        # not-yet-satisfied stat matmuls sit in the PE wait queue.
        colsum = ps_st.tile([1, QCH], F32, tag="colsum", name="colsum")
        q4 = st_pool.tile([P, 8], F32, tag="q4", name="q4")
        scr = st_pool.tile([P, NQ // 2], F32, tag="scr", name="scr")

        def emit_stats(i):
            ci, xt, lo = x_pieces[i]
            for s2 in range(2):
                nc.tensor.matmul(
                    colsum[:], _r(ones_col[:]),
                    xt[:, lo + s2 * QCH : lo + (s2 + 1) * QCH],
                    start=(i == 0 and s2 == 0), stop=(i == 7 and s2 == 1),
                    skip_group_check=True)
            nc.scalar.activation(scr[:], _f(xt[:, lo : lo + NQ // 2]),
                                 AF.Square, accum_out=q4[:, i : i + 1])

        # ---------- prologue GEMMs (emission order = arrival order) ----
        def emit_vt(kt):
            lo = kt * P
            pv = ps_qkp.tile([P, C + 2], F32, tag="ps_v", name="ps_v")
            for ci in range(NCT):
                nc.tensor.matmul(pv[:], xs(ci, lo, P), wv_ext[ci],
                                 start=(ci == 0), stop=(ci == NCT - 1))
            # stash the two bias columns so pv can be released without
            # waiting for the stats; bias_k is batch-built later.
            nc.vector.tensor_copy(hx3[:, :, kt : kt + 1], pv[:, C : C + 2])
            nc.vector.tensor_copy(
                vT[kt // 2][:, (kt % 2) * C : (kt % 2 + 1) * C], pv[:, 0:C])

        def emit_qk(which, oc, col):
            w = wqA if which == "q" else wkA
            dst, width = (q2, NQ) if which == "q" else (k2, N)
            pk = ps_qkp.tile([P, QCH], F32, tag="ps_qk", name="ps_qk")
            for ci in range(NCT):
                nc.tensor.matmul(pk[:],
                                 w[ci][:, oc * P : (oc + 1) * P],
                                 xs(ci, col, QCH),
                                 start=(ci == 0), stop=(ci == NCT - 1))
            nc.vector.tensor_copy(
                dst[:, oc * width + col : oc * width + col + QCH], pk[:])

        # stats p0,p1 | V^T 0..3 + Q(qc0) | stats p2,p3 | V^T 4..7 +
        # Q(qc1) + K0 | stats p4..7 (all GEMMs need only x pieces 0,1)
        emit_stats(0)
        emit_stats(1)
        for kt in range(4):
            emit_vt(kt)
        for oc in range(NCT):
            emit_qk("q", oc, 0)
        emit_stats(2)
        emit_stats(3)
        for kt in range(4, 8):
            emit_vt(kt)
        for oc in range(NCT):
            emit_qk("q", oc, QCH)
        for oc in range(NCT):
            emit_qk("k", oc, 0)
        for i in range(4, 8):
            emit_stats(i)

        # ---------- stats chain (small serial ops, mostly DVE) --------
        s_tot = st_pool.tile([1, 2], F32, tag="stot", name="stot")
        nc.vector.tensor_reduce(s_tot[:, 0:1], colsum[:], axis=AX.X,
                                op=OP.add)
        psq = ps_st.tile([1, 8], F32, tag="psq", name="psq")
        nc.tensor.matmul(psq[:], ones_col[:], q4[:])
        nc.vector.tensor_reduce(s_tot[:, 1:2], psq[:], axis=AX.X, op=OP.add)

        inv_cn = 1.0 / float(C * N)
        mom = st_pool.tile([1, 2], F32, tag="mom", name="mom")
        nc.vector.tensor_scalar(mom[:], s_tot[:], inv_cn, None, op0=OP.mult)
        mean_sb = mom[:, 0:1]
        msq = st_pool.tile([1, 1], F32, tag="msq", name="msq")
        nc.vector.tensor_tensor(msq[:], mean_sb, mean_sb, OP.mult)
        var_sb = st_pool.tile([1, 1], F32, tag="var", name="var")
        nc.vector.tensor_tensor(var_sb[:], mom[:, 1:2], msq[:], OP.subtract)
        eps_t = st_pool.tile([1, 1], F32, tag="eps", name="eps")
        nc.vector.memset(eps_t[:], EPS)
        lnv = st_pool.tile([1, 1], F32, tag="lnv", name="lnv")
        nc.scalar.activation(lnv[:], var_sb[:], AF.Ln, bias=eps_t[:])
        mi_sb = st_pool.tile([1, 5], F32, tag="mi", name="mi")
        inv_c = mi_sb[:, 0:1]
        nc.scalar.activation(inv_c, lnv[:], AF.Exp, scale=-0.5)
        nc.vector.tensor_tensor(mi_sb[:, 1:2], mean_sb, inv_c, OP.mult)
        nc.vector.tensor_scalar(mi_sb[:, 1:2], mi_sb[:, 1:2], -1.0, None,
                                op0=OP.mult)
        nc.vector.tensor_scalar(mi_sb[:, 2:3], inv_c, SCALE, None,
                                op0=OP.mult)
        nc.vector.tensor_tensor(mi_sb[:, 3:4], mi_sb[:, 2:3], inv_c, OP.mult)
        nc.vector.tensor_tensor(mi_sb[:, 4:5], mi_sb[:, 1:2], mi_sb[:, 2:3],
                                OP.mult)
        ps_bc5 = ps_st.tile([P, 5], F32, tag="ps_bc5", name="ps_bc5")
        nc.tensor.matmul(ps_bc5[:], ones_row[:], mi_sb[:])
        nc.vector.tensor_copy(mi_bc[:], ps_bc5[:])
        nc.vector.tensor_scalar(_r(inv_row[:]), ones_row[:], mi_sb[:, 0:1],
                                None, op0=OP.mult)
        minv_neg = mi_bc[:, 1:2]
        si_bc = mi_bc[:, 2:3]
        si2_bc = mi_bc[:, 3:4]
        m2n_bc = mi_bc[:, 4:5]
        for ci in range(NCT):
            nc.vector.scalar_tensor_tensor(dcc[ci][:], pc2[ci], minv_neg,
                                           dc1[ci], op0=OP.mult, op1=OP.add)
        # bias_k[kt] for kt 0..15 = S*inv*(h1.x) - S*mean*inv^2*(h2.x)
        bt = qk_pool.tile([P, NKT], F32, tag="bt", name="bt")
        nc.vector.tensor_scalar(bt[:, 0:16], hx3[:, 0, 0:16], si_bc, None,
                                op0=OP.mult)
        nc.vector.scalar_tensor_tensor(bias_k[:, 0:16], hx3[:, 1, 0:16],
                                       m2n_bc, bt[:, 0:16],
                                       op0=OP.mult, op1=OP.add)
        # residual tiles on the (idle) GPSIMD engine: xbd = x + dcc
        for ci in range(NCT):
            nc.gpsimd.tensor_scalar(xbd[ci][:], _f(xA[ci][:]), dcc[ci][:],
                                    None, op0=OP.add)

        p_pre.close()

        # ---------- sweep pools ----------
        p_sw = ExitStack()
        ps_s = p_sw.enter_context(
            tc.tile_pool(name="ps_s", bufs=2, space="PSUM"))
        p_qkv2 = ExitStack()
        ps_qk2 = p_qkv2.enter_context(
            tc.tile_pool(name="ps_qk2", bufs=2, space="PSUM"))

        def emit_vt2(kt):
            lo = kt * P
            pv = ps_qk2.tile([P, C + 2], F32, tag="ps_v2", name="ps_v2")
            for ci in range(NCT):
                nc.tensor.matmul(pv[:], xs(ci, lo, P), wv_ext[ci],
                                 start=(ci == 0), stop=(ci == NCT - 1))
            nc.vector.tensor_copy(hx3[:, :, kt : kt + 1], pv[:, C : C + 2])
            nc.vector.tensor_copy(
                vT[kt // 2][:, (kt % 2) * C : (kt % 2 + 1) * C], pv[:, 0:C])

        def emit_qk2(which, oc, col):
            w = wqA if which == "q" else wkA
            dst, width = (q2, NQ) if which == "q" else (k2, N)
            pk = ps_qk2.tile([P, QCH], F32, tag="ps_qk2t", name="ps_qk2t")
            for ci in range(NCT):
                nc.tensor.matmul(pk[:],
                                 w[ci][:, oc * P : (oc + 1) * P],
                                 xs(ci, col, QCH),
                                 start=(ci == 0), stop=(ci == NCT - 1))
            nc.vector.tensor_copy(
                dst[:, oc * width + col : oc * width + col + QCH], pk[:])

        k3all = k2[:].rearrange("p (j n) -> p j n", j=2)
        q3 = q2[:].rearrange("p (j n) -> p j n", j=2)

        def emit_scores_exp(kt, half):
            """scoresT + 1024-wide exp for (key tile kt, query half)."""
            ps = ps_s.tile([P, 2 * QCH], F32, tag="s", name="s")
            k3 = k3all[:, :, kt * P : (kt + 1) * P]
            for qh in range(2):
                qcol = half * 2 * QCH + qh * QCH
                nc.tensor.matmul(
                    ps[:, qh * QCH : (qh + 1) * QCH],
                    k3, q3[:, :, qcol : qcol + QCH],
                    skip_group_check=True,
                    perf_mode=mybir.MatmulPerfMode.DoubleRow)
            if kt % 2 == 0 and half == 0:
                exp_tiles[kt // 2] = exp_pool.tile(
                    [P, 2 * NQ], FP8, tag="expt", name="expt")
            lo = (kt % 2) * NQ + half * 2 * QCH
            nc.scalar.activation(
                exp_tiles[kt // 2][:, lo : lo + 2 * QCH],
                ps[:], AF.Exp, scale=si2_bc, bias=bias_k[:, kt : kt + 1])

        exp_tiles = [None] * (NKT // 2)
        ones3 = ones_fp8[:].rearrange("p (j o) -> p j o", j=2)[:, :, 0:1]

        # AV chain state (one chain at a time; 3 PSUM banks)
        p_ch = ExitStack()
        ch_h = None  # opened lazily at phase-A kt16

        def av_step(qc, p, ph, pd, first, last):
            et3 = exp_tiles[p].rearrange(
                "p (j q) -> p j q", j=2)[:, :, qc * QCH : (qc + 1) * QCH]
            vt3 = vT[p].rearrange("p (j c) -> p j c", j=2)
            for ct in range(NCT):
                nc.tensor.matmul(
                    ph[ct][:], vt3[:, :, ct * P : (ct + 1) * P], et3[:],
                    start=first, stop=last, skip_group_check=True,
                    perf_mode=mybir.MatmulPerfMode.DoubleRow)
            nc.tensor.matmul(
                pd[:], ones3, et3[:],
                start=first, stop=last, skip_group_check=True,
                perf_mode=mybir.MatmulPerfMode.DoubleRow)

        p_tail = ExitStack()

        with tc.tile_pool(name="att_sb", bufs=2) as att_pool, \
             tc.tile_pool(name="out_sb", bufs=4) as out_pool:

            tail_state = {}

            def tail_stage1(qc, ph, pd):
                """recip + broadcast (DVE + GPSIMD)."""
                rec = att_pool.tile([1, QCH], F32, tag="rec", name="rec")
                with nc.allow_low_precision(reason="f32r fp32-width"):
                    nc.vector.reciprocal(_r(rec[:]), pd[:])
                rec_bc = att_pool.tile([P, QCH], F32, tag="rec_bc",
                                       name="rec_bc")
                nc.gpsimd.partition_broadcast(rec_bc[:], rec[:])
                tail_state[qc] = (ph, rec_bc)

            def tail_stage2(qc):
                """h = ph * (inv/denom) into SBUF; releases the chain."""
                ph, rec_bc = tail_state[qc]
                h_sb = []
                for ct in range(NCT):
                    h = att_pool.tile([P, QCH], F32, tag=f"hsb{ct}",
                                      name=f"hsb{ct}")
                    nc.vector.tensor_tensor(_r(h[:]), ph[ct][:], rec_bc[:],
                                            OP.mult)
                    h_sb.append(h)
                tail_state[qc] = h_sb

            def tail_stage3(qc, oc):
                """proj GEMM + residual add + store for one oc."""
                h_sb = tail_state[qc]
                qsl = slice(qc * QCH, (qc + 1) * QCH)
                po = ps_po.tile([P, QCH], F32, tag="po", name="po")
                for ci in range(NCT):
                    nc.tensor.matmul(
                        po[:], wpT[ci][:, oc * P : (oc + 1) * P],
                        _r(h_sb[ci][:]),
                        start=(ci == 0), stop=(ci == NCT - 1),
                        skip_group_check=True)
                ot = out_pool.tile([P, QCH], F32, tag="ot", name="ot")
                nc.vector.tensor_tensor(ot[:], po[:], xbd[oc][:, qsl],
                                        OP.add)
                nc.sync.dma_start(out_d.ap()[oc * P : (oc + 1) * P, qsl],
                                  ot[:])

            # ================= phase A (query half 0) =================
            NP2 = NKT // 2
            ph_cur = pd_cur = None
            av_done = 0  # p index consumed for current chain
            for kt in range(NKT):
                # leftover GEMM injections: V^T 8..23 at kt 0..7 (2/kt),
                # 24..31 at kt 8..15; K chunks 1..7 at kt 0..6; Q half-1
                # at kt 2..5.
                if kt < 8:
                    emit_vt2(8 + 2 * kt)
                    emit_vt2(9 + 2 * kt)
                elif kt < 16:
                    emit_vt2(16 + kt)
                if kt < 7:
                    for oc in range(NCT):
                        emit_qk2("k", oc, (kt + 1) * QCH)
                if 2 <= kt < 6:
                    j = kt - 2
                    emit_qk2("q", j % 2, 2 * QCH + (j // 2) * QCH)
                if kt == 14:
                    # batch bias for kt 16..31 (hx complete by now)
                    nc.vector.tensor_scalar(bt[:, 16:32], hx3[:, 0, 16:32],
                                            si_bc, None, op0=OP.mult)
                    nc.vector.scalar_tensor_tensor(
                        bias_k[:, 16:32], hx3[:, 1, 16:32], m2n_bc,
                        bt[:, 16:32], op0=OP.mult, op1=OP.add)
                if kt == 16:
                    p_qkv2.close()

                emit_scores_exp(kt, 0)

                if kt >= 17:
                    if kt == 17:
                        ch_h = p_ch.enter_context(
                            tc.tile_pool(name="ps_ch", bufs=1, space="PSUM"))
                        ph_cur = [ch_h.tile([P, QCH], F32, tag=f"h{ct}",
                                            name=f"h{ct}")
                                  for ct in range(NCT)]
                        pd_cur = ch_h.tile([1, QCH], F32, tag="d", name="d")
                    # consume p with exp done (2p+1 <= kt), max 2/step
                    target = min((kt - 1) // 2 + 1, NP2)
                    budget = 2
                    while av_done < target and budget > 0:
                        av_step(0, av_done, ph_cur, pd_cur,
                                av_done == 0, av_done == NP2 - 1)
                        av_done += 1
                        budget -= 1
            # finish qc0 chain (p15 needs the last A exp)
            while av_done < NP2:
                av_step(0, av_done, ph_cur, pd_cur,
                        av_done == 0, av_done == NP2 - 1)
                av_done += 1

            ps_po = p_tail.enter_context(
                tc.tile_pool(name="ps_po", bufs=1, space="PSUM"))

            # ================= phase B (query half 1) =================
            tail_stage1(0, ph_cur, pd_cur)
            qc_av = 1        # chain currently running
            av_done = 0
            for kt in range(NKT):
                emit_scores_exp(kt, 1)
                if kt == 0:
                    tail_stage2(0)   # frees the qc0 chain PSUM
                if kt == 1:
                    tail_stage3(0, 0)
                if kt == 2:
                    tail_stage3(0, 1)
                # AV for qc1 (burst; all H0..no, all its exps exist) then
                # qc2 (paced behind the B exp sweep)
                if kt >= 1 and qc_av <= 2:
                    if qc_av == 1:
                        target = NP2
                        budget = 3
                    else:
                        target = min((kt - 1) // 2 + 1, NP2)
                        budget = 4
                    while av_done < target and budget > 0:
                        av_step(qc_av, av_done, ph_cur, pd_cur,
                                av_done == 0, av_done == NP2 - 1)
                        av_done += 1
                        budget -= 1
                    if av_done == NP2:
                        tail_stage1(qc_av, ph_cur, pd_cur)
                        tail_stage2(qc_av)
                        if qc_av == 1:
                            qc_av = 2
                            av_done = 0
                        else:
                            qc_av = 3
                if kt == 10:
                    tail_stage3(1, 0)
                if kt == 11:
                    tail_stage3(1, 1)
            # ================= epilogue: qc2 tail + qc3 ===============
            if qc_av == 2:
                while av_done < NP2:
                    av_step(2, av_done, ph_cur, pd_cur,
                            av_done == 0, av_done == NP2 - 1)
                    av_done += 1
                tail_stage1(2, ph_cur, pd_cur)
                tail_stage2(2)
            tail_stage3(2, 0)
            av_done = 0
            while av_done < NP2:
                av_step(3, av_done, ph_cur, pd_cur,
                        av_done == 0, av_done == NP2 - 1)
                av_done += 1
            tail_stage3(2, 1)
            tail_stage1(3, ph_cur, pd_cur)
            tail_stage2(3)
            tail_stage3(3, 0)
            tail_stage3(3, 1)
            p_tail.close()
            p_ch.close()
        p_sw.close()

    _split_excess_waits(nc)
    return nc


def make_in_maps(x, norm_gamma, norm_beta, qkv_w, qkv_b, proj_w, proj_b):
    f = np.float32
    d = np.float64
    qkv_w = np.asarray(qkv_w, dtype=d)
    qkv_b = np.asarray(qkv_b, dtype=d)
    proj_w = np.asarray(proj_w, dtype=d)
    proj_b = np.asarray(proj_b, dtype=d)
    g = np.asarray(norm_gamma, dtype=d)
    beta = np.asarray(norm_beta, dtype=d)
    Wq, Wk, Wv = qkv_w[0:C], qkv_w[C : 2 * C], qkv_w[2 * C : 3 * C]
    bq, bk, bv = qkv_b[0:C], qkv_b[C : 2 * C], qkv_b[2 * C : 3 * C]

    wqT = (Wq.T * g[:, None])          # [c_in, c_out], rows scaled by gamma
    wkT = (Wk.T * g[:, None])
    wvT = (Wv.T * g[:, None])
    u1 = bq + Wq @ beta
    u2 = Wq @ g
    h1 = wkT @ u1
    h2 = wkT @ u2
    dc1 = proj_w @ (bv + Wv @ beta) + proj_b
    pc2 = proj_w @ (Wv @ g)

    wpack = np.zeros((C, PACKW), dtype=f)
    wpack[:, 0:C] = wvT
    wpack[:, C] = h1
    wpack[:, C + 1] = h2
    wpack[:, 258 : 258 + C] = wqT
    wpack[:, 258 + C : 258 + 2 * C] = wkT
    wpack[:, 258 + 2 * C : 258 + 3 * C] = proj_w.T
    wpack[:, 258 + 3 * C] = dc1
    wpack[:, 259 + 3 * C] = pc2
    wpack = np.ascontiguousarray(wpack)

    in_maps = []
    xf = np.asarray(x, dtype=f).reshape(B, C, N)
    for core in range(8):
        b, h = divmod(core, 2)
        xs = xf[b]
        if h == 1:
            xs = np.concatenate([xs[:, NQ:], xs[:, :NQ]], axis=1)
        in_maps.append({"x": np.ascontiguousarray(xs), "wpack": wpack})
    return in_maps


def assemble_output(results):
    out = np.empty((B, C, N), dtype=np.float32)
    for core in range(8):
        b, h = divmod(core, 2)
        out[b][:, h * NQ : (h + 1) * NQ] = results[core]["out"]
    return out.reshape(B, C, HH, WW, DD)


_PROGRAM = None
_N_CALLS = 0
_RUNNER = None


def get_program():
    global _PROGRAM
    if _PROGRAM is None:
        _PROGRAM = build_program()
    return _PROGRAM


def _build_cached_runner(nc):
    """Persistent jitted executor (same execution path that
    run_bass_kernel_spmd takes under axon, via bass2jax/PJRT) so repeat
    kernel() calls skip the multi-minute neuronx-cc recompile."""
    import jax
    from jax.experimental.shard_map import shard_map
    from jax.sharding import Mesh, PartitionSpec
    from concourse import bass2jax

    bass2jax.install_neuronx_cc_hook()
    n_cores = 8
    partition_name = (nc.partition_id_tensor.name
                      if nc.partition_id_tensor else None)
    in_names, out_names, out_avals, zero_outs = [], [], [], []
    for alloc in nc.m.functions[0].allocations:
        if not isinstance(alloc, mybir.MemoryLocationSet):
            continue
        name = alloc.memorylocations[0].name
        if alloc.kind == "ExternalInput":
            if name != partition_name:
                in_names.append(name)
        elif alloc.kind == "ExternalOutput":
            out_names.append(name)
            shape = tuple(alloc.tensor_shape)
            dtype = mybir.dt.np(alloc.dtype)
            out_avals.append(jax.core.ShapedArray(shape, dtype))
            zero_outs.append(np.zeros(shape, dtype))
    n_params = len(in_names)
    all_in_names = list(in_names) + list(out_names)
    if partition_name is not None:
        all_in_names.append(partition_name)

    def _body(*args):
        operands = list(args)
        if partition_name is not None:
            operands.append(bass2jax.partition_id_tensor())
        outs = bass2jax._bass_exec_p.bind(
            *operands,
            out_avals=tuple(out_avals),
            in_names=tuple(all_in_names),
            out_names=tuple(out_names),
            lowering_input_output_aliases=(),
            sim_require_finite=True,
            sim_require_nnan=True,
            nc=nc,
        )
        return tuple(outs)

    devices = jax.devices()[:n_cores]
    mesh = Mesh(np.asarray(devices), ("core",))
    n_outs = len(out_names)
    fn = jax.jit(
        shard_map(_body, mesh=mesh,
                  in_specs=(PartitionSpec("core"),) * (n_params + n_outs),
                  out_specs=(PartitionSpec("core"),) * n_outs,
                  check_rep=False),
        keep_unused=True,
    )

    def run(in_maps):
        per_core = [[np.asarray(m[name]) for name in in_names]
                    for m in in_maps]
        concat_in = [
            np.concatenate([per_core[c][i] for c in range(n_cores)], axis=0)
            for i in range(n_params)
        ]
        concat_zeros = [
            np.zeros((n_cores * z.shape[0], *z.shape[1:]), z.dtype)
            for z in zero_outs
        ]
        out_arrs = fn(*concat_in, *concat_zeros)
        return [
            {name: np.asarray(out_arrs[i]).reshape(
                n_cores, *out_avals[i].shape)[c]
             for i, name in enumerate(out_names)}
            for c in range(n_cores)
        ]

    return run


def kernel(x, norm_gamma, norm_beta, qkv_w, qkv_b, proj_w, proj_b):
    global _N_CALLS, _RUNNER
    nc = get_program()
    in_maps = make_in_maps(x, norm_gamma, norm_beta, qkv_w, qkv_b,
                           proj_w, proj_b)
    _N_CALLS += 1
    if _N_CALLS == 1:
        res = run_bass_kernel_spmd(nc, in_maps, core_ids=list(range(8)))
        return assemble_output(res.results)
    if _RUNNER is None:
        _RUNNER = _build_cached_runner(nc)
    return assemble_output(_RUNNER(in_maps))


# revision 7
# speedup vs baseline: 1.0729x; 1.0086x over previous
"""AttnBlock3d (GroupNorm + single-head self-attention + proj + residual)
on 8 Trainium2 NeuronCores.

Sharding: 8 shards = (batch sample b in 0..3) x (query-half h in 0..1).
Every core runs the SAME program (SPMD): the host permutes each sample's
N=4096 spatial columns so that the core's 2048 query positions come
first. GroupNorm / K / V are permutation-invariant in the column order,
and attention output for a query column does not depend on the ordering
of key columns, so the math is unchanged.

Key algebra (all exact; lets every big GEMM start without waiting for
the GroupNorm statistics):
  xn = A*x + Bvec per channel, A = inv_std*gamma, Bvec = beta - mean*A.
  The gamma factor is folded into the weights on the HOST
  (W' = W diag(gamma)), so q = Wq@xn + bq = inv_std*(Wq'@x) + bq2.
  Softmax over k is invariant to anything constant along k, so only the
  [k]-indexed part of the score bias survives; it comes out of two
  extra output columns of the V^T GEMM (h1.x, h2.x) combined with the
  stats. inv^2*SCALE enters through the ACT Exp per-partition scale.
  The V-side affine (v = inv*v_raw + cvv) is folded THROUGH the proj:
  out = WpT@(ph * inv/denom) + [Wp@cvv + Wp@bv + bp + x], with the
  channel constant dcc = dc1 + (-mean*inv)*pc2 built from host vectors
  dc1 = Wp@(bv + Wv@beta) + bp and pc2 = Wp@Wv@gamma, pre-added into
  the residual tiles.

Schedule (q-major two-phase sweep):
  - prologue: 2 packed weight DMAs + 8 x-piece DMAs; GroupNorm moments
    via PE column-sum matmuls + ACT Squares (both idle then); Q(H0)/K0
    GEMMs and fp8 copies so the first exp fires as soon as the stats
    chain resolves.
  - phase A: for kt in 0..31: scoresT(kt, query-half H0) -> 1024-wide
    Exp. Leftover K chunks / V^T tiles / Q(H1) GEMMs ride the PE+DVE
    slack early in A; AV chain for qc0 runs kt>=16 (PSUM frees then);
  - phase B: same over H1; AV for qc1 (burst) + qc2 (paced) and the
    divide/proj/residual tails for qc0..2 all inside the sweep.
  - epilogue: only qc3's AV + tail.
  PSUM: scores 2x[128,1024] (4 banks) + AV chain ph0,ph1,pd (3) +
  proj po (1) = 8, with prologue pools (stats, qkv) scoped to close
  before the chain/proj pools open.
"""

import numpy as np
from contextlib import ExitStack

import bass_rust
import concourse.bass as bass
import concourse.tile as tile
from concourse import mybir
from concourse.bass_utils import run_bass_kernel_spmd

F32 = mybir.dt.float32
F32R = mybir.dt.float32r
BF16 = mybir.dt.bfloat16
FP8 = mybir.dt.float8e4
AX = mybir.AxisListType
OP = mybir.AluOpType
AF = mybir.ActivationFunctionType

B, C, HH, WW, DD = 4, 256, 16, 16, 16
N = HH * WW * DD          # 4096 spatial positions per sample
NQ = N // 2               # 2048 query positions per core
P = 128                   # partitions
NCT = C // P              # 2 channel tiles
NKT = N // P              # 32 key tiles
QCH = 512                 # q-chunk width (one PSUM bank of fp32)
NQC = NQ // QCH           # 4 q chunks
EPS = 1e-6
SCALE = float(C) ** -0.5  # 0.0625
PACKW = 258 + 3 * C + 2   # wv_ext | wq | wk | wp | dc1 | pc2


def _split_excess_waits(nc, cap=1):
    """walrus in this env rejects >1 sync wait per instruction; peel
    extras onto no-ops inserted before the offender on the same engine."""
    n = 0
    for f in nc.m.functions:
        for blk in f.blocks:
            insts = blk.instructions
            new_insts = []
            for inst in insts:
                si = inst.sync_info
                if si is not None and si.on_wait is not None and len(si.on_wait) > cap:
                    waits = list(si.on_wait)
                    extra, keep = waits[:-cap], waits[-cap:]
                    for j in range(0, len(extra), cap):
                        nop = mybir.InstNoOp(
                            name=f"{inst.name}_ws{j}", ins=[], outs=[]
                        )
                        nop.engine = inst.engine
                        nop.sync_info = bass_rust.SyncInfo(
                            on_wait=extra[j : j + cap], on_update=[]
                        )
                        new_insts.append(nop)
                    inst.sync_info = bass_rust.SyncInfo(
                        on_wait=keep, on_update=list(si.on_update)
                    )
                    n += 1
                new_insts.append(inst)
            if len(new_insts) != len(insts):
                insts[:] = new_insts
    return n


def _r(ap):
    return ap.bitcast(F32R)


def _f(ap):
    return ap.bitcast(F32)


def build_program():
    nc = bass.Bass("TRN2", target_bir_lowering=False, debug=False)

    x_d = nc.dram_tensor("x", [C, N], F32R, kind="ExternalInput")
    w_d = nc.dram_tensor("wpack", [C, PACKW], F32R, kind="ExternalInput")
    out_d = nc.dram_tensor("out", [C, NQ], F32, kind="ExternalOutput")

    with tile.TileContext(nc) as tc, ExitStack() as ctx:
        # ---------- persistent pools ----------
        consts = ctx.enter_context(tc.tile_pool(name="consts", bufs=1))
        qk_pool = ctx.enter_context(tc.tile_pool(name="qk", bufs=1))
        vt_pool = ctx.enter_context(tc.tile_pool(name="vt", bufs=NKT // 2))
        xb_pool = ctx.enter_context(tc.tile_pool(name="xb", bufs=1))
        x_pool = ctx.enter_context(tc.tile_pool(name="xio", bufs=1))
        exp_pool = ctx.enter_context(tc.tile_pool(name="expt", bufs=NKT // 2))

        # packed weights: 2 DMAs total (one per channel tile)
        wpk = [consts.tile([P, PACKW], F32R, tag=f"wpk{ci}", name=f"wpk{ci}")
               for ci in range(NCT)]
        for ci in range(NCT):
            nc.sync.dma_start(wpk[ci][:], w_d.ap()[ci * P : (ci + 1) * P, :])
        wv_ext = [wpk[ci][:, 0:258] for ci in range(NCT)]
        wqA = [wpk[ci][:, 258 : 258 + C] for ci in range(NCT)]
        wkA = [wpk[ci][:, 258 + C : 258 + 2 * C] for ci in range(NCT)]
        wpT = [wpk[ci][:, 258 + 2 * C : 258 + 3 * C] for ci in range(NCT)]
        dc1 = [_f(wpk[ci][:, 258 + 3 * C : 259 + 3 * C]) for ci in range(NCT)]
        pc2 = [_f(wpk[ci][:, 259 + 3 * C : 260 + 3 * C]) for ci in range(NCT)]

        # x in 8 pieces of [128, 1024]: both channel tiles of the query
        # half first so Q/V^T GEMMs and stats start at ~1/4 of the load.
        xA = [x_pool.tile([P, NQ], F32R, tag=f"xA{ci}", name=f"xA{ci}")
              for ci in range(NCT)]
        xB = [x_pool.tile([P, NQ], F32R, tag=f"xB{ci}", name=f"xB{ci}")
              for ci in range(NCT)]
        x_pieces = []  # (ci, tile, col_lo) in DMA order
        for half, xt in ((0, xA), (1, xB)):
            for sub in range(2):
                for ci in range(NCT):
                    lo = sub * (NQ // 2)
                    nc.sync.dma_start(
                        xt[ci][:, lo : lo + NQ // 2],
                        x_d.ap()[ci * P : (ci + 1) * P,
                                 half * NQ + lo : half * NQ + lo + NQ // 2])
                    x_pieces.append((ci, xt[ci], lo))

        def xs(ci, lo, w):
            """f32r view of x columns [lo, lo+w) (must not straddle NQ)."""
            if lo < NQ:
                assert lo + w <= NQ
                return xA[ci][:, lo : lo + w]
            return xB[ci][:, lo - NQ : lo - NQ + w]

        ones_col = consts.tile([P, 1], F32, tag="ones_col", name="ones_col")
        nc.vector.memset(ones_col[:], 1.0)
        ones_fp8 = consts.tile([P, 32], FP8, tag="ones_fp8", name="ones_fp8")
        nc.vector.memset(ones_fp8[:], 1.0)
        ones_row = consts.tile([1, P], F32, tag="ones_row", name="ones_row")
        nc.vector.memset(ones_row[:], 1.0)

        q2 = qk_pool.tile([P, NCT * NQ], FP8, tag="q2", name="q2")
        k2 = qk_pool.tile([P, NCT * N], FP8, tag="k2", name="k2")
        vT = [vt_pool.tile([P, 2 * C], FP8, tag="vt", name="vt")
              for _ in range(NKT // 2)]
        xbd = [xb_pool.tile([P, NQ], F32, tag=f"xb{ci}", name=f"xb{ci}")
               for ci in range(NCT)]
        hx = qk_pool.tile([P, 2 * NKT], F32, tag="hx", name="hx")
        hx3 = hx[:].rearrange("p (c k) -> p c k", c=2)
        bias_k = qk_pool.tile([P, NKT], F32, tag="bias_k", name="bias_k")
        # stats-derived broadcast columns:
        # [inv, -mean*inv, S*inv, S*inv^2, -S*mean*inv^2]
        mi_bc = consts.tile([P, 5], F32, tag="mi_bc", name="mi_bc")
        inv_row = consts.tile([1, P], F32, tag="inv_row", name="inv_row")
        dcc = [consts.tile([P, 1], F32, tag=f"dcc{ci}", name=f"dcc{ci}")
               for ci in range(NCT)]

        p_pre = ExitStack()
        st_pool = p_pre.enter_context(tc.tile_pool(name="stats", bufs=1))
        ps_st = p_pre.enter_context(
            tc.tile_pool(name="ps_st", bufs=1, space="PSUM"))
        ps_qkp = p_pre.enter_context(
            tc.tile_pool(name="ps_qkp", bufs=2, space="PSUM"))

        # ---------- GroupNorm moments ----------
        # column sums on the PE (idle in the prologue): 16 accumulating
        # [1,512] matmuls in x-piece DMA order; sums of squares on the
        # ACT (also idle) with the free-dim accumulator. Stats for piece
        # i are interleaved with the prologue GEMMs so at most two
        # not-yet-satisfied stat matmuls sit in the PE wait queue.
        colsum = ps_st.tile([1, QCH], F32, tag="colsum", name="colsum")
        q4 = st_pool.tile([P, 8], F32, tag="q4", name="q4")
        scr = st_pool.tile([P, NQ // 2], F32, tag="scr", name="scr")

        def emit_stats(i):
            ci, xt, lo = x_pieces[i]
            for s2 in range(2):
                nc.tensor.matmul(
                    colsum[:], _r(ones_col[:]),
                    xt[:, lo + s2 * QCH : lo + (s2 + 1) * QCH],
                    start=(i == 0 and s2 == 0), stop=(i == 7 and s2 == 1),
                    skip_group_check=True)
            nc.scalar.activation(scr[:], _f(xt[:, lo : lo + NQ // 2]),
                                 AF.Square, accum_out=q4[:, i : i + 1])

        # ---------- prologue GEMMs (emission order = arrival order) ----
        def emit_vt(kt):
            lo = kt * P
            pv = ps_qkp.tile([P, C + 2], F32, tag="ps_v", name="ps_v")
            for ci in range(NCT):
                nc.tensor.matmul(pv[:], xs(ci, lo, P), wv_ext[ci],
                                 start=(ci == 0), stop=(ci == NCT - 1))
            # stash the two bias columns so pv can be released without
            # waiting for the stats; bias_k is batch-built later.
            nc.vector.tensor_copy(hx3[:, :, kt : kt + 1], pv[:, C : C + 2])
            nc.vector.tensor_copy(
                vT[kt // 2][:, (kt % 2) * C : (kt % 2 + 1) * C], pv[:, 0:C])

        def emit_qk(which, oc, col):
            w = wqA if which == "q" else wkA
            dst, width = (q2, NQ) if which == "q" else (k2, N)
            pk = ps_qkp.tile([P, QCH], F32, tag="ps_qk", name="ps_qk")
            for ci in range(NCT):
                nc.tensor.matmul(pk[:],
                                 w[ci][:, oc * P : (oc + 1) * P],
                                 xs(ci, col, QCH),
                                 start=(ci == 0), stop=(ci == NCT - 1))
            nc.vector.tensor_copy(
                dst[:, oc * width + col : oc * width + col + QCH], pk[:])

        # stats p0,p1 | V^T 0..3 + Q(qc0) | stats p2,p3 | V^T 4..7 +
        # Q(qc1) + K0 | stats p4..7 (all GEMMs need only x pieces 0,1)
        emit_stats(0)
        emit_stats(1)
        for kt in range(4):
            emit_vt(kt)
        for oc in range(NCT):
            emit_qk("q", oc, 0)
        emit_stats(2)
        emit_stats(3)
        for kt in range(4, 8):
            emit_vt(kt)
        for oc in range(NCT):
            emit_qk("q", oc, QCH)
        for oc in range(NCT):
            emit_qk("k", oc, 0)
        for i in range(4, 8):
            emit_stats(i)

        # ---------- stats chain ----------
        # Entirely on the ACT engine (idle in the prologue, and immune
        # to the DVE copy congestion): activation computes
        # func(in*scale + bias) with per-partition AP scale, which gives
        # scalar-scalar multiply via scale=AP. The two tensor-tensor
        # combines that ACT cannot do (dcc, bias_k) go to GPSIMD.
        with tc.high_priority():
            s_tot = st_pool.tile([1, 2], F32, tag="stot", name="stot")
            scr8 = st_pool.tile([1, 8], F32, tag="scr8", name="scr8")
            scr512 = st_pool.tile([1, QCH], F32, tag="scr512", name="scr512")
            nc.scalar.activation(scr512[:], colsum[:], AF.Copy,
                                 accum_out=s_tot[:, 0:1])
            psq = ps_st.tile([1, 8], F32, tag="psq", name="psq")
            nc.tensor.matmul(psq[:], ones_col[:], q4[:])
            nc.scalar.activation(scr8[:], psq[:], AF.Copy,
                                 accum_out=s_tot[:, 1:2])

            inv_cn = 1.0 / float(C * N)
            mean_sb = st_pool.tile([1, 1], F32, tag="mean", name="mean")
            nc.scalar.activation(mean_sb[:], s_tot[:, 0:1], AF.Copy,
                                 scale=inv_cn)
            msq = st_pool.tile([1, 1], F32, tag="msq", name="msq")
            nc.scalar.activation(msq[:], mean_sb[:], AF.Square)
            epsm = st_pool.tile([1, 1], F32, tag="epsm", name="epsm")
            nc.scalar.activation(epsm[:], msq[:], AF.Copy, scale=-1.0,
                                 bias=EPS)
            lnv = st_pool.tile([1, 1], F32, tag="lnv", name="lnv")
            nc.scalar.activation(lnv[:], s_tot[:, 1:2], AF.Ln, scale=inv_cn,
                                 bias=epsm[:])
            mi_sb = st_pool.tile([1, 5], F32, tag="mi", name="mi")
            inv_c = mi_sb[:, 0:1]
            nc.scalar.activation(inv_c, lnv[:], AF.Exp, scale=-0.5)
            ninv = st_pool.tile([1, 1], F32, tag="ninv", name="ninv")
            nc.scalar.activation(ninv[:], inv_c, AF.Copy, scale=-1.0)
            nc.scalar.activation(mi_sb[:, 1:2], mean_sb[:], AF.Copy,
                                 scale=ninv[:])                  # -mean*inv
            nc.scalar.activation(mi_sb[:, 2:3], inv_c, AF.Copy,
                                 scale=SCALE)                    # S*inv
            nc.scalar.activation(mi_sb[:, 3:4], inv_c, AF.Copy,
                                 scale=mi_sb[:, 2:3])            # S*inv^2
            nc.scalar.activation(mi_sb[:, 4:5], mi_sb[:, 1:2], AF.Copy,
                                 scale=mi_sb[:, 2:3])            # -S*m*inv^2
            ps_bc5 = ps_st.tile([P, 5], F32, tag="ps_bc5", name="ps_bc5")
            nc.tensor.matmul(ps_bc5[:], ones_row[:], mi_sb[:])
            nc.scalar.activation(mi_bc[:], ps_bc5[:], AF.Copy)
            nc.scalar.activation(inv_row[:], ones_row[:], AF.Copy,
                                 scale=mi_sb[:, 0:1])
            minv_neg = mi_bc[:, 1:2]
            si_bc = mi_bc[:, 2:3]
            si2_bc = mi_bc[:, 3:4]
            m2n_bc = mi_bc[:, 4:5]
            for ci in range(NCT):
                nc.gpsimd.scalar_tensor_tensor(dcc[ci][:], pc2[ci], minv_neg,
                                               dc1[ci], op0=OP.mult,
                                               op1=OP.add)
            # bias_k = S*inv*(h1.x) - S*mean*inv^2*(h2.x), batch 0..15
            bt = qk_pool.tile([P, NKT], F32, tag="bt", name="bt")

            def emit_bias(lo, hi):
                nc.gpsimd.tensor_scalar(bt[:, lo:hi], hx3[:, 0, lo:hi],
                                        si_bc, None, op0=OP.mult)
                nc.gpsimd.scalar_tensor_tensor(bias_k[:, lo:hi],
                                               hx3[:, 1, lo:hi], m2n_bc,
                                               bt[:, lo:hi],
                                               op0=OP.mult, op1=OP.add)

            emit_bias(0, 16)
        # residual tiles on the (idle) GPSIMD engine: xbd = x + dcc
        for ci in range(NCT):
            nc.gpsimd.tensor_scalar(xbd[ci][:], _f(xA[ci][:]), dcc[ci][:],
                                    None, op0=OP.add)

        p_pre.close()

        # ---------- sweep pools ----------
        p_sw = ExitStack()
        ps_s = p_sw.enter_context(
            tc.tile_pool(name="ps_s", bufs=2, space="PSUM"))
        p_qkv2 = ExitStack()
        ps_qk2 = p_qkv2.enter_context(
            tc.tile_pool(name="ps_qk2", bufs=2, space="PSUM"))

        def emit_vt2(kt):
            lo = kt * P
            pv = ps_qk2.tile([P, C + 2], F32, tag="ps_v2", name="ps_v2")
            for ci in range(NCT):
                nc.tensor.matmul(pv[:], xs(ci, lo, P), wv_ext[ci],
                                 start=(ci == 0), stop=(ci == NCT - 1))
            nc.vector.tensor_copy(hx3[:, :, kt : kt + 1], pv[:, C : C + 2])
            nc.vector.tensor_copy(
                vT[kt // 2][:, (kt % 2) * C : (kt % 2 + 1) * C], pv[:, 0:C])

        def emit_qk2(which, oc, col):
            w = wqA if which == "q" else wkA
            dst, width = (q2, NQ) if which == "q" else (k2, N)
            pk = ps_qk2.tile([P, QCH], F32, tag="ps_qk2t", name="ps_qk2t")
            for ci in range(NCT):
                nc.tensor.matmul(pk[:],
                                 w[ci][:, oc * P : (oc + 1) * P],
                                 xs(ci, col, QCH),
                                 start=(ci == 0), stop=(ci == NCT - 1))
            nc.vector.tensor_copy(
                dst[:, oc * width + col : oc * width + col + QCH], pk[:])

        k3all = k2[:].rearrange("p (j n) -> p j n", j=2)
        q3 = q2[:].rearrange("p (j n) -> p j n", j=2)

        def emit_scores_exp(kt, half):
            """scoresT + 1024-wide exp for (key tile kt, query half)."""
            ps = ps_s.tile([P, 2 * QCH], F32, tag="s", name="s")
            k3 = k3all[:, :, kt * P : (kt + 1) * P]
            for qh in range(2):
                qcol = half * 2 * QCH + qh * QCH
                nc.tensor.matmul(
                    ps[:, qh * QCH : (qh + 1) * QCH],
                    k3, q3[:, :, qcol : qcol + QCH],
                    skip_group_check=True,
                    perf_mode=mybir.MatmulPerfMode.DoubleRow)
            if kt % 2 == 0 and half == 0:
                exp_tiles[kt // 2] = exp_pool.tile(
                    [P, 2 * NQ], FP8, tag="expt", name="expt")
            lo = (kt % 2) * NQ + half * 2 * QCH
            nc.scalar.activation(
                exp_tiles[kt // 2][:, lo : lo + 2 * QCH],
                ps[:], AF.Exp, scale=si2_bc, bias=bias_k[:, kt : kt + 1])

        exp_tiles = [None] * (NKT // 2)
        ones3 = ones_fp8[:].rearrange("p (j o) -> p j o", j=2)[:, :, 0:1]

        # AV chain state (one chain at a time; 3 PSUM banks)
        p_ch = ExitStack()
        ch_h = None  # opened lazily at phase-A kt16

        def av_step(qc, p, ph, pd, first, last):
            et3 = exp_tiles[p].rearrange(
                "p (j q) -> p j q", j=2)[:, :, qc * QCH : (qc + 1) * QCH]
            vt3 = vT[p].rearrange("p (j c) -> p j c", j=2)
            for ct in range(NCT):
                nc.tensor.matmul(
                    ph[ct][:], vt3[:, :, ct * P : (ct + 1) * P], et3[:],
                    start=first, stop=last, skip_group_check=True,
                    perf_mode=mybir.MatmulPerfMode.DoubleRow)
            nc.tensor.matmul(
                pd[:], ones3, et3[:],
                start=first, stop=last, skip_group_check=True,
                perf_mode=mybir.MatmulPerfMode.DoubleRow)

        p_tail = ExitStack()

        with tc.tile_pool(name="att_sb", bufs=2) as att_pool, \
             tc.tile_pool(name="out_sb", bufs=4) as out_pool:

            tail_state = {}

            def tail_stage1(qc, ph, pd):
                """recip + broadcast (DVE + GPSIMD)."""
                rec = att_pool.tile([1, QCH], F32, tag="rec", name="rec")
                with nc.allow_low_precision(reason="f32r fp32-width"):
                    nc.vector.reciprocal(_r(rec[:]), pd[:])
                rec_bc = att_pool.tile([P, QCH], F32, tag="rec_bc",
                                       name="rec_bc")
                nc.gpsimd.partition_broadcast(rec_bc[:], rec[:])
                tail_state[qc] = (ph, rec_bc)

            def tail_stage2(qc):
                """h = ph * (inv/denom) into SBUF; releases the chain."""
                ph, rec_bc = tail_state[qc]
                h_sb = []
                for ct in range(NCT):
                    h = att_pool.tile([P, QCH], F32, tag=f"hsb{ct}",
                                      name=f"hsb{ct}")
                    nc.vector.tensor_tensor(_r(h[:]), ph[ct][:], rec_bc[:],
                                            OP.mult)
                    h_sb.append(h)
                tail_state[qc] = h_sb

            def tail_stage3(qc, oc):
                """proj GEMM + residual add + store for one oc."""
                h_sb = tail_state[qc]
                qsl = slice(qc * QCH, (qc + 1) * QCH)
                po = ps_po.tile([P, QCH], F32, tag="po", name="po")
                for ci in range(NCT):
                    nc.tensor.matmul(
                        po[:], wpT[ci][:, oc * P : (oc + 1) * P],
                        _r(h_sb[ci][:]),
                        start=(ci == 0), stop=(ci == NCT - 1),
                        skip_group_check=True)
                ot = out_pool.tile([P, QCH], F32, tag="ot", name="ot")
                nc.vector.tensor_tensor(ot[:], po[:], xbd[oc][:, qsl],
                                        OP.add)
                nc.sync.dma_start(out_d.ap()[oc * P : (oc + 1) * P, qsl],
                                  ot[:])

            # ================= phase A (query half 0) =================
            NP2 = NKT // 2
            ph_cur = pd_cur = None
            av_done = 0  # p index consumed for current chain
            for kt in range(NKT):
                # leftover GEMM injections: V^T 8..31 at kt 0..11 (2/kt),
                # K chunks 1..7 at kt 0..6; Q half-1 at kt 2..5; bias
                # batches (GPSIMD) once their hx columns have landed.
                if kt < 12:
                    emit_vt2(8 + 2 * kt)
                    emit_vt2(9 + 2 * kt)
                if kt < 7:
                    for oc in range(NCT):
                        emit_qk2("k", oc, (kt + 1) * QCH)
                if 2 <= kt < 6:
                    j = kt - 2
                    emit_qk2("q", j % 2, 2 * QCH + (j // 2) * QCH)
                if kt == 11:
                    emit_bias(16, 24)
                if kt == 14:
                    emit_bias(24, 32)
                if kt == 16:
                    p_qkv2.close()

                emit_scores_exp(kt, 0)

                if kt >= 17:
                    if kt == 17:
                        ch_h = p_ch.enter_context(
                            tc.tile_pool(name="ps_ch", bufs=1, space="PSUM"))
                        ph_cur = [ch_h.tile([P, QCH], F32, tag=f"h{ct}",
                                            name=f"h{ct}")
                                  for ct in range(NCT)]
                        pd_cur = ch_h.tile([1, QCH], F32, tag="d", name="d")
                    # consume p with exp done (2p+1 <= kt), max 2/step
                    target = min((kt - 1) // 2 + 1, NP2)
                    budget = 2
                    while av_done < target and budget > 0:
                        av_step(0, av_done, ph_cur, pd_cur,
                                av_done == 0, av_done == NP2 - 1)
                        av_done += 1
                        budget -= 1
            # finish qc0 chain (p15 needs the last A exp)
            while av_done < NP2:
                av_step(0, av_done, ph_cur, pd_cur,
                        av_done == 0, av_done == NP2 - 1)
                av_done += 1

            ps_po = p_tail.enter_context(
                tc.tile_pool(name="ps_po", bufs=1, space="PSUM"))

            # ================= phase B (query half 1) =================
            tail_stage1(0, ph_cur, pd_cur)
            qc_av = 1        # chain currently running
            av_done = 0
            for kt in range(NKT):
                emit_scores_exp(kt, 1)
                if kt == 0:
                    tail_stage2(0)   # frees the qc0 chain PSUM
                if kt == 1:
                    tail_stage3(0, 0)
                if kt == 2:
                    tail_stage3(0, 1)
                # AV for qc1 (burst; all H0..no, all its exps exist) then
                # qc2 (paced behind the B exp sweep)
                if kt >= 1 and qc_av <= 2:
                    if qc_av == 1:
                        target = NP2
                        budget = 3
                    else:
                        target = min((kt - 1) // 2 + 1, NP2)
                        budget = 4
                    while av_done < target and budget > 0:
                        av_step(qc_av, av_done, ph_cur, pd_cur,
                                av_done == 0, av_done == NP2 - 1)
                        av_done += 1
                        budget -= 1
                    if av_done == NP2:
                        tail_stage1(qc_av, ph_cur, pd_cur)
                        tail_stage2(qc_av)
                        if qc_av == 1:
                            qc_av = 2
                            av_done = 0
                        else:
                            qc_av = 3
                if kt == 10:
                    tail_stage3(1, 0)
                if kt == 11:
                    tail_stage3(1, 1)
            # ================= epilogue: qc2 tail + qc3 ===============
            if qc_av == 2:
                while av_done < NP2:
                    av_step(2, av_done, ph_cur, pd_cur,
                            av_done == 0, av_done == NP2 - 1)
                    av_done += 1
                tail_stage1(2, ph_cur, pd_cur)
                tail_stage2(2)
            tail_stage3(2, 0)
            av_done = 0
            while av_done < NP2:
                av_step(3, av_done, ph_cur, pd_cur,
                        av_done == 0, av_done == NP2 - 1)
                av_done += 1
            tail_stage3(2, 1)
            tail_stage1(3, ph_cur, pd_cur)
            tail_stage2(3)
            tail_stage3(3, 0)
            tail_stage3(3, 1)
            p_tail.close()
            p_ch.close()
        p_sw.close()

    _split_excess_waits(nc)
    return nc


def make_in_maps(x, norm_gamma, norm_beta, qkv_w, qkv_b, proj_w, proj_b):
    f = np.float32
    d = np.float64
    qkv_w = np.asarray(qkv_w, dtype=d)
    qkv_b = np.asarray(qkv_b, dtype=d)
    proj_w = np.asarray(proj_w, dtype=d)
    proj_b = np.asarray(proj_b, dtype=d)
    g = np.asarray(norm_gamma, dtype=d)
    beta = np.asarray(norm_beta, dtype=d)
    Wq, Wk, Wv = qkv_w[0:C], qkv_w[C : 2 * C], qkv_w[2 * C : 3 * C]
    bq, bk, bv = qkv_b[0:C], qkv_b[C : 2 * C], qkv_b[2 * C : 3 * C]

    wqT = (Wq.T * g[:, None])          # [c_in, c_out], rows scaled by gamma
    wkT = (Wk.T * g[:, None])
    wvT = (Wv.T * g[:, None])
    u1 = bq + Wq @ beta
    u2 = Wq @ g
    h1 = wkT @ u1
    h2 = wkT @ u2
    dc1 = proj_w @ (bv + Wv @ beta) + proj_b
    pc2 = proj_w @ (Wv @ g)

    wpack = np.zeros((C, PACKW), dtype=f)
    wpack[:, 0:C] = wvT
    wpack[:, C] = h1
    wpack[:, C + 1] = h2
    wpack[:, 258 : 258 + C] = wqT
    wpack[:, 258 + C : 258 + 2 * C] = wkT
    wpack[:, 258 + 2 * C : 258 + 3 * C] = proj_w.T
    wpack[:, 258 + 3 * C] = dc1
    wpack[:, 259 + 3 * C] = pc2
    wpack = np.ascontiguousarray(wpack)

    in_maps = []
    xf = np.asarray(x, dtype=f).reshape(B, C, N)
    for core in range(8):
        b, h = divmod(core, 2)
        xs = xf[b]
        if h == 1:
            xs = np.concatenate([xs[:, NQ:], xs[:, :NQ]], axis=1)
        in_maps.append({"x": np.ascontiguousarray(xs), "wpack": wpack})
    return in_maps


def assemble_output(results):
    out = np.empty((B, C, N), dtype=np.float32)
    for core in range(8):
        b, h = divmod(core, 2)
        out[b][:, h * NQ : (h + 1) * NQ] = results[core]["out"]
    return out.reshape(B, C, HH, WW, DD)


_PROGRAM = None
_N_CALLS = 0
_RUNNER = None


def get_program():
    global _PROGRAM
    if _PROGRAM is None:
        _PROGRAM = build_program()
    return _PROGRAM


def _build_cached_runner(nc):
    """Persistent jitted executor (same execution path that
    run_bass_kernel_spmd takes under axon, via bass2jax/PJRT) so repeat
    kernel() calls skip the multi-minute neuronx-cc recompile."""
    import jax
    from jax.experimental.shard_map import shard_map
    from jax.sharding import Mesh, PartitionSpec
    from concourse import bass2jax

    bass2jax.install_neuronx_cc_hook()
    n_cores = 8
    partition_name = (nc.partition_id_tensor.name
                      if nc.partition_id_tensor else None)
    in_names, out_names, out_avals, zero_outs = [], [], [], []
    for alloc in nc.m.functions[0].allocations:
        if not isinstance(alloc, mybir.MemoryLocationSet):
            continue
        name = alloc.memorylocations[0].name
        if alloc.kind == "ExternalInput":
            if name != partition_name:
                in_names.append(name)
        elif alloc.kind == "ExternalOutput":
            out_names.append(name)
            shape = tuple(alloc.tensor_shape)
            dtype = mybir.dt.np(alloc.dtype)
            out_avals.append(jax.core.ShapedArray(shape, dtype))
            zero_outs.append(np.zeros(shape, dtype))
    n_params = len(in_names)
    all_in_names = list(in_names) + list(out_names)
    if partition_name is not None:
        all_in_names.append(partition_name)

    def _body(*args):
        operands = list(args)
        if partition_name is not None:
            operands.append(bass2jax.partition_id_tensor())
        outs = bass2jax._bass_exec_p.bind(
            *operands,
            out_avals=tuple(out_avals),
            in_names=tuple(all_in_names),
            out_names=tuple(out_names),
            lowering_input_output_aliases=(),
            sim_require_finite=True,
            sim_require_nnan=True,
            nc=nc,
        )
        return tuple(outs)

    devices = jax.devices()[:n_cores]
    mesh = Mesh(np.asarray(devices), ("core",))
    n_outs = len(out_names)
    fn = jax.jit(
        shard_map(_body, mesh=mesh,
                  in_specs=(PartitionSpec("core"),) * (n_params + n_outs),
                  out_specs=(PartitionSpec("core"),) * n_outs,
                  check_rep=False),
        keep_unused=True,
    )

    def run(in_maps):
        per_core = [[np.asarray(m[name]) for name in in_names]
                    for m in in_maps]
        concat_in = [
            np.concatenate([per_core[c][i] for c in range(n_cores)], axis=0)
            for i in range(n_params)
        ]
        concat_zeros = [
            np.zeros((n_cores * z.shape[0], *z.shape[1:]), z.dtype)
            for z in zero_outs
        ]
        out_arrs = fn(*concat_in, *concat_zeros)
        return [
            {name: np.asarray(out_arrs[i]).reshape(
                n_cores, *out_avals[i].shape)[c]
             for i, name in enumerate(out_names)}
            for c in range(n_cores)
        ]

    return run


def kernel(x, norm_gamma, norm_beta, qkv_w, qkv_b, proj_w, proj_b):
    global _N_CALLS, _RUNNER
    nc = get_program()
    in_maps = make_in_maps(x, norm_gamma, norm_beta, qkv_w, qkv_b,
                           proj_w, proj_b)
    _N_CALLS += 1
    if _N_CALLS == 1:
        res = run_bass_kernel_spmd(nc, in_maps, core_ids=list(range(8)))
        return assemble_output(res.results)
    if _RUNNER is None:
        _RUNNER = _build_cached_runner(nc)
    return assemble_output(_RUNNER(in_maps))


# revision 10
# speedup vs baseline: 1.0749x; 1.0019x over previous
"""AttnBlock3d (GroupNorm + single-head self-attention + proj + residual)
on 8 Trainium2 NeuronCores.

Sharding: 8 shards = (batch sample b in 0..3) x (query-half h in 0..1).
Every core runs the SAME program (SPMD): the host permutes each sample's
N=4096 spatial columns so that the core's 2048 query positions come
first. GroupNorm / K / V are permutation-invariant in the column order,
and attention output for a query column does not depend on the ordering
of key columns, so the math is unchanged.

Key algebra (all exact; lets every big GEMM start without waiting for
the GroupNorm statistics):
  xn = A*x + Bvec per channel, A = inv_std*gamma, Bvec = beta - mean*A.
  The gamma factor is folded into the weights on the HOST
  (W' = W diag(gamma)), so q = Wq@xn + bq = inv_std*(Wq'@x) + bq2.
  Softmax over k is invariant to anything constant along k, so only the
  [k]-indexed part of the score bias survives; it comes out of two
  extra output columns of the V^T GEMM (h1.x, h2.x) combined with the
  stats. inv^2*SCALE enters through the ACT Exp per-partition scale.
  The V-side affine (v = inv*v_raw + cvv) is folded THROUGH the proj:
  out = WpT@(ph * inv/denom) + [Wp@cvv + Wp@bv + bp + x], with the
  channel constant dcc = dc1 + (-mean*inv)*pc2 built from host vectors
  dc1 = Wp@(bv + Wv@beta) + bp and pc2 = Wp@Wv@gamma, pre-added into
  the residual tiles.

Schedule (q-major two-phase sweep):
  - prologue: 2 packed weight DMAs + 8 x-piece DMAs; GroupNorm moments
    via PE column-sum matmuls + ACT Squares (both idle then); Q(H0)/K0
    GEMMs and fp8 copies so the first exp fires as soon as the stats
    chain resolves.
  - phase A: for kt in 0..31: scoresT(kt, query-half H0) -> 1024-wide
    Exp. Leftover K chunks / V^T tiles / Q(H1) GEMMs ride the PE+DVE
    slack early in A; AV chain for qc0 runs kt>=16 (PSUM frees then);
  - phase B: same over H1; AV for qc1 (burst) + qc2 (paced) and the
    divide/proj/residual tails for qc0..2 all inside the sweep.
  - epilogue: only qc3's AV + tail.
  PSUM: scores 2x[128,1024] (4 banks) + AV chain ph0,ph1,pd (3) +
  proj po (1) = 8, with prologue pools (stats, qkv) scoped to close
  before the chain/proj pools open.
"""

import numpy as np
from contextlib import ExitStack

import bass_rust
import concourse.bass as bass
import concourse.tile as tile
from concourse import mybir
from concourse.bass_utils import run_bass_kernel_spmd

F32 = mybir.dt.float32
F32R = mybir.dt.float32r
BF16 = mybir.dt.bfloat16
FP8 = mybir.dt.float8e4
AX = mybir.AxisListType
OP = mybir.AluOpType
AF = mybir.ActivationFunctionType

B, C, HH, WW, DD = 4, 256, 16, 16, 16
N = HH * WW * DD          # 4096 spatial positions per sample
NQ = N // 2               # 2048 query positions per core
P = 128                   # partitions
NCT = C // P              # 2 channel tiles
NKT = N // P              # 32 key tiles
QCH = 512                 # q-chunk width (one PSUM bank of fp32)
NQC = NQ // QCH           # 4 q chunks
EPS = 1e-6
SCALE = float(C) ** -0.5  # 0.0625
PACKW = 258 + 3 * C + 2   # wv_ext | wq | wk | wp | dc1 | pc2


def _split_excess_waits(nc, cap=1):
    """walrus in this env rejects >1 sync wait per instruction; peel
    extras onto no-ops inserted before the offender on the same engine."""
    n = 0
    for f in nc.m.functions:
        for blk in f.blocks:
            insts = blk.instructions
            new_insts = []
            for inst in insts:
                si = inst.sync_info
                if si is not None and si.on_wait is not None and len(si.on_wait) > cap:
                    waits = list(si.on_wait)
                    extra, keep = waits[:-cap], waits[-cap:]
                    for j in range(0, len(extra), cap):
                        nop = mybir.InstNoOp(
                            name=f"{inst.name}_ws{j}", ins=[], outs=[]
                        )
                        nop.engine = inst.engine
                        nop.sync_info = bass_rust.SyncInfo(
                            on_wait=extra[j : j + cap], on_update=[]
                        )
                        new_insts.append(nop)
                    inst.sync_info = bass_rust.SyncInfo(
                        on_wait=keep, on_update=list(si.on_update)
                    )
                    n += 1
                new_insts.append(inst)
            if len(new_insts) != len(insts):
                insts[:] = new_insts
    return n


def _r(ap):
    return ap.bitcast(F32R)


def _f(ap):
    return ap.bitcast(F32)


def build_program():
    nc = bass.Bass("TRN2", target_bir_lowering=False, debug=False)

    x_d = nc.dram_tensor("x", [C, N], F32R, kind="ExternalInput")
    w_d = nc.dram_tensor("wpack", [C, PACKW], F32R, kind="ExternalInput")
    out_d = nc.dram_tensor("out", [C, NQ], F32, kind="ExternalOutput")

    with tile.TileContext(nc) as tc, ExitStack() as ctx:
        # ---------- persistent pools ----------
        consts = ctx.enter_context(tc.tile_pool(name="consts", bufs=1))
        qk_pool = ctx.enter_context(tc.tile_pool(name="qk", bufs=1))
        vt_pool = ctx.enter_context(tc.tile_pool(name="vt", bufs=NKT // 2))
        xb_pool = ctx.enter_context(tc.tile_pool(name="xb", bufs=1))
        x_pool = ctx.enter_context(tc.tile_pool(name="xio", bufs=1))
        exp_pool = ctx.enter_context(tc.tile_pool(name="expt", bufs=NKT // 2))

        # packed weights: 2 DMAs total (one per channel tile)
        wpk = [consts.tile([P, PACKW], F32R, tag=f"wpk{ci}", name=f"wpk{ci}")
               for ci in range(NCT)]
        for ci in range(NCT):
            nc.sync.dma_start(wpk[ci][:], w_d.ap()[ci * P : (ci + 1) * P, :])
        wv_ext = [wpk[ci][:, 0:258] for ci in range(NCT)]
        wqA = [wpk[ci][:, 258 : 258 + C] for ci in range(NCT)]
        wkA = [wpk[ci][:, 258 + C : 258 + 2 * C] for ci in range(NCT)]
        wpT = [wpk[ci][:, 258 + 2 * C : 258 + 3 * C] for ci in range(NCT)]
        dc1 = [_f(wpk[ci][:, 258 + 3 * C : 259 + 3 * C]) for ci in range(NCT)]
        pc2 = [_f(wpk[ci][:, 259 + 3 * C : 260 + 3 * C]) for ci in range(NCT)]

        # x in 8 pieces of [128, 1024]: both channel tiles of the query
        # half first so Q/V^T GEMMs and stats start at ~1/4 of the load.
        xA = [x_pool.tile([P, NQ], F32R, tag=f"xA{ci}", name=f"xA{ci}")
              for ci in range(NCT)]
        xB = [x_pool.tile([P, NQ], F32R, tag=f"xB{ci}", name=f"xB{ci}")
              for ci in range(NCT)]
        x_pieces = []  # (ci, tile, col_lo) in DMA order
        for half, xt in ((0, xA), (1, xB)):
            for sub in range(2):
                for ci in range(NCT):
                    lo = sub * (NQ // 2)
                    nc.sync.dma_start(
                        xt[ci][:, lo : lo + NQ // 2],
                        x_d.ap()[ci * P : (ci + 1) * P,
                                 half * NQ + lo : half * NQ + lo + NQ // 2])
                    x_pieces.append((ci, xt[ci], lo))

        def xs(ci, lo, w):
            """f32r view of x columns [lo, lo+w) (must not straddle NQ)."""
            if lo < NQ:
                assert lo + w <= NQ
                return xA[ci][:, lo : lo + w]
            return xB[ci][:, lo - NQ : lo - NQ + w]

        ones_col = consts.tile([P, 1], F32, tag="ones_col", name="ones_col")
        nc.vector.memset(ones_col[:], 1.0)
        ones_fp8 = consts.tile([P, 32], FP8, tag="ones_fp8", name="ones_fp8")
        nc.vector.memset(ones_fp8[:], 1.0)
        ones_row = consts.tile([1, P], F32, tag="ones_row", name="ones_row")
        nc.vector.memset(ones_row[:], 1.0)

        q2 = qk_pool.tile([P, NCT * NQ], FP8, tag="q2", name="q2")
        k2 = qk_pool.tile([P, NCT * N], FP8, tag="k2", name="k2")
        vT = [vt_pool.tile([P, 2 * C], FP8, tag="vt", name="vt")
              for _ in range(NKT // 2)]
        xbd = [xb_pool.tile([P, NQ], F32, tag=f"xb{ci}", name=f"xb{ci}")
               for ci in range(NCT)]
        hx = qk_pool.tile([P, 2 * NKT], F32, tag="hx", name="hx")
        hx3 = hx[:].rearrange("p (c k) -> p c k", c=2)
        bias_k = qk_pool.tile([P, NKT], F32, tag="bias_k", name="bias_k")
        # stats-derived broadcast columns:
        # [inv, -mean*inv, S*inv, S*inv^2, -S*mean*inv^2]
        mi_bc = consts.tile([P, 5], F32, tag="mi_bc", name="mi_bc")
        inv_row = consts.tile([1, P], F32, tag="inv_row", name="inv_row")
        dcc = [consts.tile([P, 1], F32, tag=f"dcc{ci}", name=f"dcc{ci}")
               for ci in range(NCT)]

        p_pre = ExitStack()
        st_pool = p_pre.enter_context(tc.tile_pool(name="stats", bufs=1))
        ps_st = p_pre.enter_context(
            tc.tile_pool(name="ps_st", bufs=1, space="PSUM"))
        ps_qkp = p_pre.enter_context(
            tc.tile_pool(name="ps_qkp", bufs=2, space="PSUM"))

        # ---------- GroupNorm moments ----------
        # column sums on the PE (idle in the prologue): 16 accumulating
        # [1,512] matmuls in x-piece DMA order; sums of squares on the
        # ACT (also idle) with the free-dim accumulator. Stats for piece
        # i are interleaved with the prologue GEMMs so at most two
        # not-yet-satisfied stat matmuls sit in the PE wait queue.
        colsum = ps_st.tile([1, QCH], F32, tag="colsum", name="colsum")
        q4 = st_pool.tile([P, 8], F32, tag="q4", name="q4")
        scr = st_pool.tile([P, NQ // 2], F32, tag="scr", name="scr")

        def emit_stats(i):
            ci, xt, lo = x_pieces[i]
            for s2 in range(2):
                nc.tensor.matmul(
                    colsum[:], _r(ones_col[:]),
                    xt[:, lo + s2 * QCH : lo + (s2 + 1) * QCH],
                    start=(i == 0 and s2 == 0), stop=(i == 7 and s2 == 1),
                    skip_group_check=True)
            nc.scalar.activation(scr[:], _f(xt[:, lo : lo + NQ // 2]),
                                 AF.Square, accum_out=q4[:, i : i + 1])

        # ---------- prologue GEMMs (emission order = arrival order) ----
        def emit_vt(kt):
            lo = kt * P
            pv = ps_qkp.tile([P, C + 2], F32, tag="ps_v", name="ps_v")
            for ci in range(NCT):
                nc.tensor.matmul(pv[:], xs(ci, lo, P), wv_ext[ci],
                                 start=(ci == 0), stop=(ci == NCT - 1))
            # stash the two bias columns so pv can be released without
            # waiting for the stats; bias_k is batch-built later.
            nc.vector.tensor_copy(hx3[:, :, kt : kt + 1], pv[:, C : C + 2])
            nc.vector.tensor_copy(
                vT[kt // 2][:, (kt % 2) * C : (kt % 2 + 1) * C], pv[:, 0:C])

        def emit_qk(which, oc, col):
            w = wqA if which == "q" else wkA
            dst, width = (q2, NQ) if which == "q" else (k2, N)
            pk = ps_qkp.tile([P, QCH], F32, tag="ps_qk", name="ps_qk")
            for ci in range(NCT):
                nc.tensor.matmul(pk[:],
                                 w[ci][:, oc * P : (oc + 1) * P],
                                 xs(ci, col, QCH),
                                 start=(ci == 0), stop=(ci == NCT - 1))
            nc.vector.tensor_copy(
                dst[:, oc * width + col : oc * width + col + QCH], pk[:])

        # stats p0,p1 | V^T 0..3 + Q(qc0) | stats p2,p3 | V^T 4..7 +
        # Q(qc1) + K0 | stats p4..7 (all GEMMs need only x pieces 0,1)
        emit_stats(0)
        emit_stats(1)
        for kt in range(4):
            emit_vt(kt)
        for oc in range(NCT):
            emit_qk("q", oc, 0)
        emit_stats(2)
        emit_stats(3)
        for kt in range(4, 8):
            emit_vt(kt)
        for oc in range(NCT):
            emit_qk("q", oc, QCH)
        for oc in range(NCT):
            emit_qk("k", oc, 0)
        for i in range(4, 8):
            emit_stats(i)

        # ---------- stats chain ----------
        # Entirely on the ACT engine (idle in the prologue, and immune
        # to the DVE copy congestion): activation computes
        # func(in*scale + bias) with per-partition AP scale, which gives
        # scalar-scalar multiply via scale=AP. The two tensor-tensor
        # combines that ACT cannot do (dcc, bias_k) go to GPSIMD.
        with tc.high_priority():
            s_tot = st_pool.tile([1, 2], F32, tag="stot", name="stot")
            scr8 = st_pool.tile([1, 8], F32, tag="scr8", name="scr8")
            scr512 = st_pool.tile([1, QCH], F32, tag="scr512", name="scr512")
            nc.scalar.activation(scr512[:], colsum[:], AF.Copy,
                                 accum_out=s_tot[:, 0:1])
            psq = ps_st.tile([1, 8], F32, tag="psq", name="psq")
            nc.tensor.matmul(psq[:], ones_col[:], q4[:])
            nc.scalar.activation(scr8[:], psq[:], AF.Copy,
                                 accum_out=s_tot[:, 1:2])

            inv_cn = 1.0 / float(C * N)
            mean_sb = st_pool.tile([1, 1], F32, tag="mean", name="mean")
            nc.scalar.activation(mean_sb[:], s_tot[:, 0:1], AF.Copy,
                                 scale=inv_cn)
            msq = st_pool.tile([1, 1], F32, tag="msq", name="msq")
            nc.scalar.activation(msq[:], mean_sb[:], AF.Square)
            epsm = st_pool.tile([1, 1], F32, tag="epsm", name="epsm")
            nc.scalar.activation(epsm[:], msq[:], AF.Copy, scale=-1.0,
                                 bias=EPS)
            lnv = st_pool.tile([1, 1], F32, tag="lnv", name="lnv")
            nc.scalar.activation(lnv[:], s_tot[:, 1:2], AF.Ln, scale=inv_cn,
                                 bias=epsm[:])
            mi_sb = st_pool.tile([1, 5], F32, tag="mi", name="mi")
            inv_c = mi_sb[:, 0:1]
            nc.scalar.activation(inv_c, lnv[:], AF.Exp, scale=-0.5)
            ninv = st_pool.tile([1, 1], F32, tag="ninv", name="ninv")
            nc.scalar.activation(ninv[:], inv_c, AF.Copy, scale=-1.0)
            nc.scalar.activation(mi_sb[:, 1:2], mean_sb[:], AF.Copy,
                                 scale=ninv[:])                  # -mean*inv
            nc.scalar.activation(mi_sb[:, 2:3], inv_c, AF.Copy,
                                 scale=SCALE)                    # S*inv
            nc.scalar.activation(mi_sb[:, 3:4], inv_c, AF.Copy,
                                 scale=mi_sb[:, 2:3])            # S*inv^2
            nc.scalar.activation(mi_sb[:, 4:5], mi_sb[:, 1:2], AF.Copy,
                                 scale=mi_sb[:, 2:3])            # -S*m*inv^2
            ps_bc5 = ps_st.tile([P, 5], F32, tag="ps_bc5", name="ps_bc5")
            nc.tensor.matmul(ps_bc5[:], ones_row[:], mi_sb[:])
            nc.scalar.activation(mi_bc[:], ps_bc5[:], AF.Copy)
            nc.scalar.activation(inv_row[:], ones_row[:], AF.Copy,
                                 scale=mi_sb[:, 0:1])
            minv_neg = mi_bc[:, 1:2]
            si_bc = mi_bc[:, 2:3]
            si2_bc = mi_bc[:, 3:4]
            m2n_bc = mi_bc[:, 4:5]
            for ci in range(NCT):
                nc.gpsimd.scalar_tensor_tensor(dcc[ci][:], pc2[ci], minv_neg,
                                               dc1[ci], op0=OP.mult,
                                               op1=OP.add)
            # bias_k = S*inv*(h1.x) - S*mean*inv^2*(h2.x), batch 0..15
            bt = qk_pool.tile([P, NKT], F32, tag="bt", name="bt")

            def emit_bias(lo, hi):
                nc.gpsimd.tensor_scalar(bt[:, lo:hi], hx3[:, 0, lo:hi],
                                        si_bc, None, op0=OP.mult)
                nc.gpsimd.scalar_tensor_tensor(bias_k[:, lo:hi],
                                               hx3[:, 1, lo:hi], m2n_bc,
                                               bt[:, lo:hi],
                                               op0=OP.mult, op1=OP.add)

            emit_bias(0, 16)

        p_pre.close()

        # ---------- sweep pools ----------
        p_sw = ExitStack()
        ps_s = p_sw.enter_context(
            tc.tile_pool(name="ps_s", bufs=2, space="PSUM"))
        p_qkv2 = ExitStack()
        ps_qk2 = p_qkv2.enter_context(
            tc.tile_pool(name="ps_qk2", bufs=2, space="PSUM"))

        def emit_vt2(kt):
            lo = kt * P
            pv = ps_qk2.tile([P, C + 2], F32, tag="ps_v2", name="ps_v2")
            for ci in range(NCT):
                nc.tensor.matmul(pv[:], xs(ci, lo, P), wv_ext[ci],
                                 start=(ci == 0), stop=(ci == NCT - 1))
            nc.vector.tensor_copy(hx3[:, :, kt : kt + 1], pv[:, C : C + 2])
            nc.vector.tensor_copy(
                vT[kt // 2][:, (kt % 2) * C : (kt % 2 + 1) * C], pv[:, 0:C])

        def emit_qk2(which, oc, col):
            w = wqA if which == "q" else wkA
            dst, width = (q2, NQ) if which == "q" else (k2, N)
            pk = ps_qk2.tile([P, QCH], F32, tag="ps_qk2t", name="ps_qk2t")
            for ci in range(NCT):
                nc.tensor.matmul(pk[:],
                                 w[ci][:, oc * P : (oc + 1) * P],
                                 xs(ci, col, QCH),
                                 start=(ci == 0), stop=(ci == NCT - 1))
            nc.vector.tensor_copy(
                dst[:, oc * width + col : oc * width + col + QCH], pk[:])

        k3all = k2[:].rearrange("p (j n) -> p j n", j=2)
        q3 = q2[:].rearrange("p (j n) -> p j n", j=2)

        def emit_scores_exp(kt, half):
            """scoresT + 1024-wide exp for (key tile kt, query half)."""
            ps = ps_s.tile([P, 2 * QCH], F32, tag="s", name="s")
            k3 = k3all[:, :, kt * P : (kt + 1) * P]
            for qh in range(2):
                qcol = half * 2 * QCH + qh * QCH
                nc.tensor.matmul(
                    ps[:, qh * QCH : (qh + 1) * QCH],
                    k3, q3[:, :, qcol : qcol + QCH],
                    skip_group_check=True,
                    perf_mode=mybir.MatmulPerfMode.DoubleRow)
            if kt % 2 == 0 and half == 0:
                exp_tiles[kt // 2] = exp_pool.tile(
                    [P, 2 * NQ], FP8, tag="expt", name="expt")
            lo = (kt % 2) * NQ + half * 2 * QCH
            nc.scalar.activation(
                exp_tiles[kt // 2][:, lo : lo + 2 * QCH],
                ps[:], AF.Exp, scale=si2_bc, bias=bias_k[:, kt : kt + 1])

        exp_tiles = [None] * (NKT // 2)
        ones3 = ones_fp8[:].rearrange("p (j o) -> p j o", j=2)[:, :, 0:1]

        # AV chain state (one chain at a time; 3 PSUM banks)
        p_ch = ExitStack()
        ch_h = None  # opened lazily at phase-A kt16

        def av_step(qc, p, ph, pd, first, last):
            et3 = exp_tiles[p].rearrange(
                "p (j q) -> p j q", j=2)[:, :, qc * QCH : (qc + 1) * QCH]
            vt3 = vT[p].rearrange("p (j c) -> p j c", j=2)
            for ct in range(NCT):
                nc.tensor.matmul(
                    ph[ct][:], vt3[:, :, ct * P : (ct + 1) * P], et3[:],
                    start=first, stop=last, skip_group_check=True,
                    perf_mode=mybir.MatmulPerfMode.DoubleRow)
            nc.tensor.matmul(
                pd[:], ones3, et3[:],
                start=first, stop=last, skip_group_check=True,
                perf_mode=mybir.MatmulPerfMode.DoubleRow)

        p_tail = ExitStack()

        with tc.tile_pool(name="att_sb", bufs=2) as att_pool, \
             tc.tile_pool(name="out_sb", bufs=4) as out_pool:

            tail_state = {}

            def tail_stage1(qc, ph, pd):
                """recip + inv-scaled broadcast (DVE + PE)."""
                rec = att_pool.tile([1, QCH], F32, tag="rec", name="rec")
                with nc.allow_low_precision(reason="f32r fp32-width"):
                    nc.vector.reciprocal(_r(rec[:]), pd[:])
                pbc = ps_po.tile([P, QCH], F32, tag="po", name="pbc")
                nc.tensor.matmul(pbc[:], _r(inv_row[:]), _r(rec[:]),
                                 skip_group_check=True)
                rec_bc = att_pool.tile([P, QCH], F32, tag="rec_bc",
                                       name="rec_bc")
                nc.vector.tensor_copy(rec_bc[:], pbc[:])
                tail_state[qc] = (ph, rec_bc)

            def tail_stage2(qc):
                """h = ph * (inv/denom) into SBUF; releases the chain."""
                ph, rec_bc = tail_state[qc]
                h_sb = []
                for ct in range(NCT):
                    h = att_pool.tile([P, QCH], F32, tag=f"hsb{ct}",
                                      name=f"hsb{ct}")
                    nc.vector.tensor_tensor(_r(h[:]), ph[ct][:], rec_bc[:],
                                            OP.mult)
                    h_sb.append(h)
                tail_state[qc] = h_sb

            def tail_stage3(qc, oc):
                """proj GEMM + residual add + store for one oc."""
                h_sb = tail_state[qc]
                qsl = slice(qc * QCH, (qc + 1) * QCH)
                po = ps_po.tile([P, QCH], F32, tag="po", name="po")
                for ci in range(NCT):
                    nc.tensor.matmul(
                        po[:], wpT[ci][:, oc * P : (oc + 1) * P],
                        _r(h_sb[ci][:]),
                        start=(ci == 0), stop=(ci == NCT - 1),
                        skip_group_check=True)
                ot = out_pool.tile([P, QCH], F32, tag="ot", name="ot")
                nc.vector.tensor_tensor(ot[:], po[:], xbd[oc][:, qsl],
                                        OP.add)
                nc.sync.dma_start(out_d.ap()[oc * P : (oc + 1) * P, qsl],
                                  ot[:])

            # ================= phase A (query half 0) =================
            NP2 = NKT // 2
            ph_cur = pd_cur = None
            av_done = 0  # p index consumed for current chain
            for kt in range(NKT):
                # leftover GEMM injections: V^T 8..31 at kt 0..11 (2/kt),
                # K chunks 1..7 at kt 0..6; Q half-1 at kt 2..5; bias
                # batches (GPSIMD) once their hx columns have landed.
                if kt < 12:
                    emit_vt2(8 + 2 * kt)
                    emit_vt2(9 + 2 * kt)
                if kt < 7:
                    for oc in range(NCT):
                        emit_qk2("k", oc, (kt + 1) * QCH)
                if 2 <= kt < 6:
                    j = kt - 2
                    emit_qk2("q", j % 2, 2 * QCH + (j // 2) * QCH)
                if kt == 11:
                    emit_bias(16, 24)
                if kt == 14:
                    emit_bias(24, 32)
                if kt in (2, 4):
                    # residual tiles on the (idle) GPSIMD engine, emitted
                    # here so the scheduler keeps them off the bias path
                    ci = kt // 2 - 1
                    nc.gpsimd.tensor_scalar(xbd[ci][:], _f(xA[ci][:]),
                                            dcc[ci][:], None, op0=OP.add)
                if kt == 16:
                    p_qkv2.close()

                emit_scores_exp(kt, 0)

                if kt >= 17:
                    if kt == 17:
                        ch_h = p_ch.enter_context(
                            tc.tile_pool(name="ps_ch", bufs=1, space="PSUM"))
                        ph_cur = [ch_h.tile([P, QCH], F32, tag=f"h{ct}",
                                            name=f"h{ct}")
                                  for ct in range(NCT)]
                        pd_cur = ch_h.tile([1, QCH], F32, tag="d", name="d")
                    # consume p with exp done (2p+1 <= kt), max 2/step
                    target = min((kt - 1) // 2 + 1, NP2)
                    budget = 2
                    while av_done < target and budget > 0:
                        av_step(0, av_done, ph_cur, pd_cur,
                                av_done == 0, av_done == NP2 - 1)
                        av_done += 1
                        budget -= 1
            # finish qc0 chain (p15 needs the last A exp)
            while av_done < NP2:
                av_step(0, av_done, ph_cur, pd_cur,
                        av_done == 0, av_done == NP2 - 1)
                av_done += 1

            ps_po = p_tail.enter_context(
                tc.tile_pool(name="ps_po", bufs=1, space="PSUM"))

            # ================= phase B (query half 1) =================
            tail_stage1(0, ph_cur, pd_cur)
            qc_av = 1        # chain currently running
            av_done = 0
            for kt in range(NKT):
                emit_scores_exp(kt, 1)
                if kt == 0:
                    tail_stage2(0)   # frees the qc0 chain PSUM
                if kt == 1:
                    tail_stage3(0, 0)
                if kt == 2:
                    tail_stage3(0, 1)
                # AV for qc1 (burst; all H0..no, all its exps exist) then
                # qc2 (paced behind the B exp sweep)
                if kt >= 1 and qc_av <= 2:
                    if qc_av == 1:
                        target = NP2
                        budget = 3
                    else:
                        target = min((kt - 1) // 2 + 1, NP2)
                        budget = 4
                    while av_done < target and budget > 0:
                        av_step(qc_av, av_done, ph_cur, pd_cur,
                                av_done == 0, av_done == NP2 - 1)
                        av_done += 1
                        budget -= 1
                    if av_done == NP2:
                        tail_stage1(qc_av, ph_cur, pd_cur)
                        tail_stage2(qc_av)
                        if qc_av == 1:
                            qc_av = 2
                            av_done = 0
                        else:
                            qc_av = 3
                if kt == 10:
                    tail_stage3(1, 0)
                if kt == 11:
                    tail_stage3(1, 1)
            # ================= epilogue: qc2 tail + qc3 ===============
            if qc_av == 2:
                while av_done < NP2:
                    av_step(2, av_done, ph_cur, pd_cur,
                            av_done == 0, av_done == NP2 - 1)
                    av_done += 1
                tail_stage1(2, ph_cur, pd_cur)
                tail_stage2(2)
            tail_stage3(2, 0)
            av_done = 0
            while av_done < NP2:
                av_step(3, av_done, ph_cur, pd_cur,
                        av_done == 0, av_done == NP2 - 1)
                av_done += 1
            tail_stage3(2, 1)
            tail_stage1(3, ph_cur, pd_cur)
            tail_stage2(3)
            tail_stage3(3, 0)
            tail_stage3(3, 1)
            p_tail.close()
            p_ch.close()
        p_sw.close()

    _split_excess_waits(nc)
    return nc


def make_in_maps(x, norm_gamma, norm_beta, qkv_w, qkv_b, proj_w, proj_b):
    f = np.float32
    d = np.float64
    qkv_w = np.asarray(qkv_w, dtype=d)
    qkv_b = np.asarray(qkv_b, dtype=d)
    proj_w = np.asarray(proj_w, dtype=d)
    proj_b = np.asarray(proj_b, dtype=d)
    g = np.asarray(norm_gamma, dtype=d)
    beta = np.asarray(norm_beta, dtype=d)
    Wq, Wk, Wv = qkv_w[0:C], qkv_w[C : 2 * C], qkv_w[2 * C : 3 * C]
    bq, bk, bv = qkv_b[0:C], qkv_b[C : 2 * C], qkv_b[2 * C : 3 * C]

    wqT = (Wq.T * g[:, None])          # [c_in, c_out], rows scaled by gamma
    wkT = (Wk.T * g[:, None])
    wvT = (Wv.T * g[:, None])
    u1 = bq + Wq @ beta
    u2 = Wq @ g
    h1 = wkT @ u1
    h2 = wkT @ u2
    dc1 = proj_w @ (bv + Wv @ beta) + proj_b
    pc2 = proj_w @ (Wv @ g)

    wpack = np.zeros((C, PACKW), dtype=f)
    wpack[:, 0:C] = wvT
    wpack[:, C] = h1
    wpack[:, C + 1] = h2
    wpack[:, 258 : 258 + C] = wqT
    wpack[:, 258 + C : 258 + 2 * C] = wkT
    wpack[:, 258 + 2 * C : 258 + 3 * C] = proj_w.T
    wpack[:, 258 + 3 * C] = dc1
    wpack[:, 259 + 3 * C] = pc2
    wpack = np.ascontiguousarray(wpack)

    in_maps = []
    xf = np.asarray(x, dtype=f).reshape(B, C, N)
    for core in range(8):
        b, h = divmod(core, 2)
        xs = xf[b]
        if h == 1:
            xs = np.concatenate([xs[:, NQ:], xs[:, :NQ]], axis=1)
        in_maps.append({"x": np.ascontiguousarray(xs), "wpack": wpack})
    return in_maps


def assemble_output(results):
    out = np.empty((B, C, N), dtype=np.float32)
    for core in range(8):
        b, h = divmod(core, 2)
        out[b][:, h * NQ : (h + 1) * NQ] = results[core]["out"]
    return out.reshape(B, C, HH, WW, DD)


_PROGRAM = None
_N_CALLS = 0
_RUNNER = None


def get_program():
    global _PROGRAM
    if _PROGRAM is None:
        _PROGRAM = build_program()
    return _PROGRAM


def _build_cached_runner(nc):
    """Persistent jitted executor (same execution path that
    run_bass_kernel_spmd takes under axon, via bass2jax/PJRT) so repeat
    kernel() calls skip the multi-minute neuronx-cc recompile."""
    import jax
    from jax.experimental.shard_map import shard_map
    from jax.sharding import Mesh, PartitionSpec
    from concourse import bass2jax

    bass2jax.install_neuronx_cc_hook()
    n_cores = 8
    partition_name = (nc.partition_id_tensor.name
                      if nc.partition_id_tensor else None)
    in_names, out_names, out_avals, zero_outs = [], [], [], []
    for alloc in nc.m.functions[0].allocations:
        if not isinstance(alloc, mybir.MemoryLocationSet):
            continue
        name = alloc.memorylocations[0].name
        if alloc.kind == "ExternalInput":
            if name != partition_name:
                in_names.append(name)
        elif alloc.kind == "ExternalOutput":
            out_names.append(name)
            shape = tuple(alloc.tensor_shape)
            dtype = mybir.dt.np(alloc.dtype)
            out_avals.append(jax.core.ShapedArray(shape, dtype))
            zero_outs.append(np.zeros(shape, dtype))
    n_params = len(in_names)
    all_in_names = list(in_names) + list(out_names)
    if partition_name is not None:
        all_in_names.append(partition_name)

    def _body(*args):
        operands = list(args)
        if partition_name is not None:
            operands.append(bass2jax.partition_id_tensor())
        outs = bass2jax._bass_exec_p.bind(
            *operands,
            out_avals=tuple(out_avals),
            in_names=tuple(all_in_names),
            out_names=tuple(out_names),
            lowering_input_output_aliases=(),
            sim_require_finite=True,
            sim_require_nnan=True,
            nc=nc,
        )
        return tuple(outs)

    devices = jax.devices()[:n_cores]
    mesh = Mesh(np.asarray(devices), ("core",))
    n_outs = len(out_names)
    fn = jax.jit(
        shard_map(_body, mesh=mesh,
                  in_specs=(PartitionSpec("core"),) * (n_params + n_outs),
                  out_specs=(PartitionSpec("core"),) * n_outs,
                  check_rep=False),
        keep_unused=True,
    )

    def run(in_maps):
        per_core = [[np.asarray(m[name]) for name in in_names]
                    for m in in_maps]
        concat_in = [
            np.concatenate([per_core[c][i] for c in range(n_cores)], axis=0)
            for i in range(n_params)
        ]
        concat_zeros = [
            np.zeros((n_cores * z.shape[0], *z.shape[1:]), z.dtype)
            for z in zero_outs
        ]
        out_arrs = fn(*concat_in, *concat_zeros)
        return [
            {name: np.asarray(out_arrs[i]).reshape(
                n_cores, *out_avals[i].shape)[c]
             for i, name in enumerate(out_names)}
            for c in range(n_cores)
        ]

    return run


def kernel(x, norm_gamma, norm_beta, qkv_w, qkv_b, proj_w, proj_b):
    global _N_CALLS, _RUNNER
    nc = get_program()
    in_maps = make_in_maps(x, norm_gamma, norm_beta, qkv_w, qkv_b,
                           proj_w, proj_b)
    _N_CALLS += 1
    if _N_CALLS == 1:
        res = run_bass_kernel_spmd(nc, in_maps, core_ids=list(range(8)))
        return assemble_output(res.results)
    if _RUNNER is None:
        _RUNNER = _build_cached_runner(nc)
    return assemble_output(_RUNNER(in_maps))


# revision 14
# speedup vs baseline: 1.1129x; 1.0353x over previous
"""AttnBlock3d (GroupNorm + single-head self-attention + proj + residual)
on 8 Trainium2 NeuronCores.

Sharding: 8 shards = (batch sample b in 0..3) x (query-half h in 0..1).
Every core runs the SAME program (SPMD): the host permutes each sample's
N=4096 spatial columns so that the core's 2048 query positions come
first. GroupNorm / K / V are permutation-invariant in the column order,
and attention output for a query column does not depend on the ordering
of key columns, so the math is unchanged.

Key algebra (all exact; lets every big GEMM start without waiting for
the GroupNorm statistics):
  xn = A*x + Bvec per channel, A = inv_std*gamma, Bvec = beta - mean*A.
  The gamma factor is folded into the weights on the HOST
  (W' = W diag(gamma)), so q = Wq@xn + bq = inv_std*(Wq'@x) + bq2.
  Softmax over k is invariant to anything constant along k, so only the
  [k]-indexed part of the score bias survives; it comes out of two
  extra output columns of the V^T GEMM (h1.x, h2.x) combined with the
  stats. inv^2*SCALE enters through the ACT Exp per-partition scale.
  The V-side affine (v = inv*v_raw + cvv) is folded THROUGH the proj:
  out = WpT@(ph * inv/denom) + [Wp@cvv + Wp@bv + bp + x], with the
  channel constant dcc = dc1 + (-mean*inv)*pc2 built from host vectors
  dc1 = Wp@(bv + Wv@beta) + bp and pc2 = Wp@Wv@gamma, pre-added into
  the residual tiles.

Schedule (q-major two-phase sweep):
  - prologue: 2 packed weight DMAs + 8 x-piece DMAs; GroupNorm moments
    via PE column-sum matmuls + ACT Squares (both idle then); Q(H0)/K0
    GEMMs and fp8 copies so the first exp fires as soon as the stats
    chain resolves.
  - phase A: for kt in 0..31: scoresT(kt, query-half H0) -> 1024-wide
    Exp. Leftover K chunks / V^T tiles / Q(H1) GEMMs ride the PE+DVE
    slack early in A; AV chain for qc0 runs kt>=16 (PSUM frees then);
  - phase B: same over H1; AV for qc1 (burst) + qc2 (paced) and the
    divide/proj/residual tails for qc0..2 all inside the sweep.
  - epilogue: only qc3's AV + tail.
  PSUM: scores 2x[128,1024] (4 banks) + AV chain ph0,ph1,pd (3) +
  proj po (1) = 8, with prologue pools (stats, qkv) scoped to close
  before the chain/proj pools open.
"""

import numpy as np
from contextlib import ExitStack

import bass_rust
import concourse.bass as bass
import concourse.tile as tile
from concourse import mybir
from concourse.bass_utils import run_bass_kernel_spmd

F32 = mybir.dt.float32
F32R = mybir.dt.float32r
BF16 = mybir.dt.bfloat16
FP8 = mybir.dt.float8e4
AX = mybir.AxisListType
OP = mybir.AluOpType
AF = mybir.ActivationFunctionType

B, C, HH, WW, DD = 4, 256, 16, 16, 16
N = HH * WW * DD          # 4096 spatial positions per sample
NQ = N // 2               # 2048 query positions per core
P = 128                   # partitions
NCT = C // P              # 2 channel tiles
NKT = N // P              # 32 key tiles
QCH = 512                 # q-chunk width (one PSUM bank of fp32)
NQC = NQ // QCH           # 4 q chunks
EPS = 1e-6
SCALE = float(C) ** -0.5  # 0.0625
PACKW = 258 + 3 * C + 2   # wv_ext | wq | wk | wp | dc1 | pc2


def _split_excess_waits(nc, cap=1):
    """walrus in this env rejects >1 sync wait per instruction; peel
    extras onto no-ops inserted before the offender on the same engine."""
    n = 0
    for f in nc.m.functions:
        for blk in f.blocks:
            insts = blk.instructions
            new_insts = []
            for inst in insts:
                si = inst.sync_info
                if si is not None and si.on_wait is not None and len(si.on_wait) > cap:
                    waits = list(si.on_wait)
                    extra, keep = waits[:-cap], waits[-cap:]
                    for j in range(0, len(extra), cap):
                        nop = mybir.InstNoOp(
                            name=f"{inst.name}_ws{j}", ins=[], outs=[]
                        )
                        nop.engine = inst.engine
                        nop.sync_info = bass_rust.SyncInfo(
                            on_wait=extra[j : j + cap], on_update=[]
                        )
                        new_insts.append(nop)
                    inst.sync_info = bass_rust.SyncInfo(
                        on_wait=keep, on_update=list(si.on_update)
                    )
                    n += 1
                new_insts.append(inst)
            if len(new_insts) != len(insts):
                insts[:] = new_insts
    return n


def _r(ap):
    return ap.bitcast(F32R)


def _f(ap):
    return ap.bitcast(F32)


def build_program():
    nc = bass.Bass("TRN2", target_bir_lowering=False, debug=False)

    x_d = nc.dram_tensor("x", [C, N], F32R, kind="ExternalInput")
    w_d = nc.dram_tensor("wpack", [C, PACKW], F32R, kind="ExternalInput")
    out_d = nc.dram_tensor("out", [C, NQ], F32, kind="ExternalOutput")

    with tile.TileContext(nc) as tc, ExitStack() as ctx:
        # ---------- persistent pools ----------
        consts = ctx.enter_context(tc.tile_pool(name="consts", bufs=1))
        qk_pool = ctx.enter_context(tc.tile_pool(name="qk", bufs=1))
        vt_pool = ctx.enter_context(tc.tile_pool(name="vt", bufs=NKT // 2))
        xb_pool = ctx.enter_context(tc.tile_pool(name="xb", bufs=1))
        x_pool = ctx.enter_context(tc.tile_pool(name="xio", bufs=1))
        exp_pool = ctx.enter_context(tc.tile_pool(name="expt", bufs=NKT // 2))

        # packed weights: 2 DMAs total (one per channel tile)
        wpk = [consts.tile([P, PACKW], F32R, tag=f"wpk{ci}", name=f"wpk{ci}")
               for ci in range(NCT)]
        for ci in range(NCT):
            nc.sync.dma_start(wpk[ci][:], w_d.ap()[ci * P : (ci + 1) * P, :])
        wv_ext = [wpk[ci][:, 0:258] for ci in range(NCT)]
        wqA = [wpk[ci][:, 258 : 258 + C] for ci in range(NCT)]
        wkA = [wpk[ci][:, 258 + C : 258 + 2 * C] for ci in range(NCT)]
        wpT = [wpk[ci][:, 258 + 2 * C : 258 + 3 * C] for ci in range(NCT)]
        dc1 = [_f(wpk[ci][:, 258 + 3 * C : 259 + 3 * C]) for ci in range(NCT)]
        pc2 = [_f(wpk[ci][:, 259 + 3 * C : 260 + 3 * C]) for ci in range(NCT)]

        # x in 8 pieces of [128, 1024]: both channel tiles of the query
        # half first so Q/V^T GEMMs and stats start at ~1/4 of the load.
        xA = [x_pool.tile([P, NQ], F32R, tag=f"xA{ci}", name=f"xA{ci}")
              for ci in range(NCT)]
        xB = [x_pool.tile([P, NQ], F32R, tag=f"xB{ci}", name=f"xB{ci}")
              for ci in range(NCT)]
        x_pieces = []  # (ci, tile, col_lo) in DMA order
        for half, xt in ((0, xA), (1, xB)):
            for sub in range(2):
                for ci in range(NCT):
                    lo = sub * (NQ // 2)
                    nc.sync.dma_start(
                        xt[ci][:, lo : lo + NQ // 2],
                        x_d.ap()[ci * P : (ci + 1) * P,
                                 half * NQ + lo : half * NQ + lo + NQ // 2])
                    x_pieces.append((ci, xt[ci], lo))

        def xs(ci, lo, w):
            """f32r view of x columns [lo, lo+w) (must not straddle NQ)."""
            if lo < NQ:
                assert lo + w <= NQ
                return xA[ci][:, lo : lo + w]
            return xB[ci][:, lo - NQ : lo - NQ + w]

        ones_col = consts.tile([P, 1], F32, tag="ones_col", name="ones_col")
        nc.vector.memset(ones_col[:], 1.0)
        ones_fp8 = consts.tile([P, 32], FP8, tag="ones_fp8", name="ones_fp8")
        nc.vector.memset(ones_fp8[:], 1.0)
        ones_row = consts.tile([1, P], F32, tag="ones_row", name="ones_row")
        nc.vector.memset(ones_row[:], 1.0)

        q2 = qk_pool.tile([P, NCT * NQ], FP8, tag="q2", name="q2")
        k2 = qk_pool.tile([P, NCT * N], FP8, tag="k2", name="k2")
        vT = [vt_pool.tile([P, 2 * C], FP8, tag="vt", name="vt")
              for _ in range(NKT // 2)]
        xbd = [xb_pool.tile([P, NQ], F32, tag=f"xb{ci}", name=f"xb{ci}")
               for ci in range(NCT)]
        hx = qk_pool.tile([P, 2 * NKT], F32, tag="hx", name="hx")
        hx3 = hx[:].rearrange("p (c k) -> p c k", c=2)
        bias_k = qk_pool.tile([P, NKT], F32, tag="bias_k", name="bias_k")
        # stats-derived broadcast columns:
        # [inv, -mean*inv, S*inv, S*inv^2, -S*mean*inv^2]
        mi_bc = consts.tile([P, 5], F32, tag="mi_bc", name="mi_bc")
        inv_row = consts.tile([1, P], F32, tag="inv_row", name="inv_row")
        dcc = [consts.tile([P, 1], F32, tag=f"dcc{ci}", name=f"dcc{ci}")
               for ci in range(NCT)]

        p_pre = ExitStack()
        st_pool = p_pre.enter_context(tc.tile_pool(name="stats", bufs=1))
        ps_st = p_pre.enter_context(
            tc.tile_pool(name="ps_st", bufs=1, space="PSUM"))
        ps_qkp = p_pre.enter_context(
            tc.tile_pool(name="ps_qkp", bufs=2, space="PSUM"))

        # ---------- GroupNorm moments ----------
        # column sums on the PE (idle in the prologue): 16 accumulating
        # [1,512] matmuls in x-piece DMA order; sums of squares on the
        # ACT (also idle) with the free-dim accumulator. Stats for piece
        # i are interleaved with the prologue GEMMs so at most two
        # not-yet-satisfied stat matmuls sit in the PE wait queue.
        colsum = ps_st.tile([1, QCH], F32, tag="colsum", name="colsum")
        q4 = st_pool.tile([P, 8], F32, tag="q4", name="q4")
        scr = st_pool.tile([P, NQ // 2], F32, tag="scr", name="scr")

        def emit_stats(i):
            ci, xt, lo = x_pieces[i]
            for s2 in range(2):
                nc.tensor.matmul(
                    colsum[:], _r(ones_col[:]),
                    xt[:, lo + s2 * QCH : lo + (s2 + 1) * QCH],
                    start=(i == 0 and s2 == 0), stop=(i == 7 and s2 == 1),
                    skip_group_check=True)
            nc.scalar.activation(scr[:], _f(xt[:, lo : lo + NQ // 2]),
                                 AF.Square, accum_out=q4[:, i : i + 1])

        # ---------- prologue GEMMs (emission order = arrival order) ----
        def emit_vt(kt):
            lo = kt * P
            pv = ps_qkp.tile([P, C + 2], F32, tag="ps_v", name="ps_v")
            for ci in range(NCT):
                nc.tensor.matmul(pv[:], xs(ci, lo, P), wv_ext[ci],
                                 start=(ci == 0), stop=(ci == NCT - 1))
            # stash the two bias columns so pv can be released without
            # waiting for the stats; bias_k is batch-built later.
            nc.vector.tensor_copy(hx3[:, :, kt : kt + 1], pv[:, C : C + 2])
            nc.vector.tensor_copy(
                vT[kt // 2][:, (kt % 2) * C : (kt % 2 + 1) * C], pv[:, 0:C])

        def emit_qk(which, oc, col):
            w = wqA if which == "q" else wkA
            dst, width = (q2, NQ) if which == "q" else (k2, N)
            pk = ps_qkp.tile([P, QCH], F32, tag="ps_qk", name="ps_qk")
            for ci in range(NCT):
                nc.tensor.matmul(pk[:],
                                 w[ci][:, oc * P : (oc + 1) * P],
                                 xs(ci, col, QCH),
                                 start=(ci == 0), stop=(ci == NCT - 1))
            nc.vector.tensor_copy(
                dst[:, oc * width + col : oc * width + col + QCH], pk[:])

        # stats p0,p1 | V^T 0..3 + Q(qc0) | stats p2,p3 | V^T 4..7 +
        # Q(qc1) + K0 | stats p4..7 (all GEMMs need only x pieces 0,1)
        emit_stats(0)
        emit_stats(1)
        for kt in range(4):
            emit_vt(kt)
        for oc in range(NCT):
            emit_qk("q", oc, 0)
        emit_stats(2)
        emit_stats(3)
        for kt in range(4, 8):
            emit_vt(kt)
        for oc in range(NCT):
            emit_qk("q", oc, QCH)
        for oc in range(NCT):
            emit_qk("k", oc, 0)
        for i in range(4, 8):
            emit_stats(i)

        # ---------- stats chain ----------
        # Entirely on the ACT engine (idle in the prologue, and immune
        # to the DVE copy congestion): activation computes
        # func(in*scale + bias) with per-partition AP scale, which gives
        # scalar-scalar multiply via scale=AP. The two tensor-tensor
        # combines that ACT cannot do (dcc, bias_k) go to GPSIMD.
        with tc.high_priority():
            s_tot = st_pool.tile([1, 2], F32, tag="stot", name="stot")
            scr8 = st_pool.tile([1, 8], F32, tag="scr8", name="scr8")
            scr512 = st_pool.tile([1, QCH], F32, tag="scr512", name="scr512")
            nc.scalar.activation(scr512[:], colsum[:], AF.Copy,
                                 accum_out=s_tot[:, 0:1])
            psq = ps_st.tile([1, 8], F32, tag="psq", name="psq")
            nc.tensor.matmul(psq[:], ones_col[:], q4[:])
            nc.scalar.activation(scr8[:], psq[:], AF.Copy,
                                 accum_out=s_tot[:, 1:2])

            inv_cn = 1.0 / float(C * N)
            mean_sb = st_pool.tile([1, 1], F32, tag="mean", name="mean")
            nc.scalar.activation(mean_sb[:], s_tot[:, 0:1], AF.Copy,
                                 scale=inv_cn)
            msq = st_pool.tile([1, 1], F32, tag="msq", name="msq")
            nc.scalar.activation(msq[:], mean_sb[:], AF.Square)
            epsm = st_pool.tile([1, 1], F32, tag="epsm", name="epsm")
            nc.scalar.activation(epsm[:], msq[:], AF.Copy, scale=-1.0,
                                 bias=EPS)
            lnv = st_pool.tile([1, 1], F32, tag="lnv", name="lnv")
            nc.scalar.activation(lnv[:], s_tot[:, 1:2], AF.Ln, scale=inv_cn,
                                 bias=epsm[:])
            mi_sb = st_pool.tile([1, 5], F32, tag="mi", name="mi")
            inv_c = mi_sb[:, 0:1]
            nc.scalar.activation(inv_c, lnv[:], AF.Exp, scale=-0.5)
            ninv = st_pool.tile([1, 1], F32, tag="ninv", name="ninv")
            nc.scalar.activation(ninv[:], inv_c, AF.Copy, scale=-1.0)
            nc.scalar.activation(mi_sb[:, 1:2], mean_sb[:], AF.Copy,
                                 scale=ninv[:])                  # -mean*inv
            nc.scalar.activation(mi_sb[:, 2:3], inv_c, AF.Copy,
                                 scale=SCALE)                    # S*inv
            nc.scalar.activation(mi_sb[:, 3:4], inv_c, AF.Copy,
                                 scale=mi_sb[:, 2:3])            # S*inv^2
            nc.scalar.activation(mi_sb[:, 4:5], mi_sb[:, 1:2], AF.Copy,
                                 scale=mi_sb[:, 2:3])            # -S*m*inv^2
            ps_bc5 = ps_st.tile([P, 5], F32, tag="ps_bc5", name="ps_bc5")
            nc.tensor.matmul(ps_bc5[:], ones_row[:], mi_sb[:])
            nc.scalar.activation(mi_bc[:], ps_bc5[:], AF.Copy)
            nc.scalar.activation(inv_row[:], ones_row[:], AF.Copy,
                                 scale=mi_sb[:, 0:1])
            minv_neg = mi_bc[:, 1:2]
            si_bc = mi_bc[:, 2:3]
            si2_bc = mi_bc[:, 3:4]
            m2n_bc = mi_bc[:, 4:5]
            for ci in range(NCT):
                nc.gpsimd.tensor_scalar(dcc[ci][:], pc2[ci], minv_neg,
                                        dc1[ci], op0=OP.mult, op1=OP.add)
            # bias_k = S*inv*(h1.x) - S*mean*inv^2*(h2.x), batch 0..15
            bt = qk_pool.tile([P, 2 * NKT], F32, tag="bt", name="bt")

            def emit_bias(lo, hi):
                nc.gpsimd.tensor_scalar(bt[:, lo:hi], hx3[:, 0, lo:hi],
                                        si_bc, None, op0=OP.mult)
                nc.gpsimd.tensor_scalar(bt[:, 32 + lo : 32 + hi],
                                        hx3[:, 1, lo:hi], m2n_bc, None,
                                        op0=OP.mult)
                nc.gpsimd.tensor_tensor(bias_k[:, lo:hi], bt[:, lo:hi],
                                        bt[:, 32 + lo : 32 + hi], OP.add)

            emit_bias(0, 16)

        p_pre.close()

        # ---------- sweep pools ----------
        p_sw = ExitStack()
        ps_s = p_sw.enter_context(
            tc.tile_pool(name="ps_s", bufs=2, space="PSUM"))
        p_qkv2 = ExitStack()
        ps_qk2 = p_qkv2.enter_context(
            tc.tile_pool(name="ps_qk2", bufs=2, space="PSUM"))

        def emit_vt2(kt):
            lo = kt * P
            pv = ps_qk2.tile([P, C + 2], F32, tag="ps_v2", name="ps_v2")
            for ci in range(NCT):
                nc.tensor.matmul(pv[:], xs(ci, lo, P), wv_ext[ci],
                                 start=(ci == 0), stop=(ci == NCT - 1))
            nc.vector.tensor_copy(hx3[:, :, kt : kt + 1], pv[:, C : C + 2])
            nc.vector.tensor_copy(
                vT[kt // 2][:, (kt % 2) * C : (kt % 2 + 1) * C], pv[:, 0:C])

        def emit_qk2(which, oc, col):
            w = wqA if which == "q" else wkA
            dst, width = (q2, NQ) if which == "q" else (k2, N)
            pk = ps_qk2.tile([P, QCH], F32, tag="ps_qk2t", name="ps_qk2t")
            for ci in range(NCT):
                nc.tensor.matmul(pk[:],
                                 w[ci][:, oc * P : (oc + 1) * P],
                                 xs(ci, col, QCH),
                                 start=(ci == 0), stop=(ci == NCT - 1))
            nc.vector.tensor_copy(
                dst[:, oc * width + col : oc * width + col + QCH], pk[:])

        k3all = k2[:].rearrange("p (j n) -> p j n", j=2)
        q3 = q2[:].rearrange("p (j n) -> p j n", j=2)

        def emit_scores_exp(kt, half):
            """scoresT + 1024-wide exp for (key tile kt, query half).
            High priority: the exp stream is the whole-kernel critical
            path, so its scores matmuls must win PE arbitration over AV
            bursts whenever both are ready."""
            if kt % 2 == 0 and half == 0:
                exp_tiles[kt // 2] = exp_pool.tile(
                    [P, 2 * NQ], FP8, tag="expt", name="expt")
            with tc.high_priority():
                ps = ps_s.tile([P, 2 * QCH], F32, tag="s", name="s")
                k3 = k3all[:, :, kt * P : (kt + 1) * P]
                for qh in range(2):
                    qcol = half * 2 * QCH + qh * QCH
                    nc.tensor.matmul(
                        ps[:, qh * QCH : (qh + 1) * QCH],
                        k3, q3[:, :, qcol : qcol + QCH],
                        skip_group_check=True,
                        perf_mode=mybir.MatmulPerfMode.DoubleRow)
                lo = (kt % 2) * NQ + half * 2 * QCH
                nc.scalar.activation(
                    exp_tiles[kt // 2][:, lo : lo + 2 * QCH],
                    ps[:], AF.Exp, scale=si2_bc, bias=bias_k[:, kt : kt + 1])

        exp_tiles = [None] * (NKT // 2)
        ones3 = ones_fp8[:].rearrange("p (j o) -> p j o", j=2)[:, :, 0:1]

        # AV chain state (one chain at a time; 3 PSUM banks)
        p_ch = ExitStack()
        ch_h = None  # opened lazily at phase-A kt16

        def av_step(qc, p, ph, pd, first, last):
            et3 = exp_tiles[p].rearrange(
                "p (j q) -> p j q", j=2)[:, :, qc * QCH : (qc + 1) * QCH]
            vt3 = vT[p].rearrange("p (j c) -> p j c", j=2)
            for ct in range(NCT):
                nc.tensor.matmul(
                    ph[ct][:], vt3[:, :, ct * P : (ct + 1) * P], et3[:],
                    start=first, stop=last, skip_group_check=True,
                    perf_mode=mybir.MatmulPerfMode.DoubleRow)
            nc.tensor.matmul(
                pd[0:1, :], ones3, et3[:],
                start=first, stop=last, skip_group_check=True,
                perf_mode=mybir.MatmulPerfMode.DoubleRow)

        p_tail = ExitStack()

        with tc.tile_pool(name="att_sb", bufs=2) as att_pool, \
             tc.tile_pool(name="out_sb", bufs=4) as out_pool:

            tail_state = {}

            def tail_stage1(qc, ph, pd):
                """recip + inv-scaled broadcast (DVE + PE). The
                broadcast lands back in the (now-free) denominator bank,
                so no extra PSUM bank is needed."""
                rec = att_pool.tile([1, QCH], F32, tag="rec", name="rec")
                with nc.allow_low_precision(reason="f32r fp32-width"):
                    nc.vector.reciprocal(_r(rec[:]), pd[0:1, :])
                nc.tensor.matmul(pd[:], _r(inv_row[:]), _r(rec[:]),
                                 skip_group_check=True)
                rec_bc = att_pool.tile([P, QCH], F32, tag="rec_bc",
                                       name="rec_bc")
                nc.vector.tensor_copy(rec_bc[:], pd[:])
                tail_state[qc] = (ph, rec_bc)

            def tail_stage2(qc):
                """h = ph * (inv/denom) into SBUF; releases the chain."""
                ph, rec_bc = tail_state[qc]
                h_sb = []
                for ct in range(NCT):
                    h = att_pool.tile([P, QCH], F32, tag=f"hsb{ct}",
                                      name=f"hsb{ct}")
                    nc.vector.tensor_tensor(_r(h[:]), ph[ct][:], rec_bc[:],
                                            OP.mult)
                    h_sb.append(h)
                tail_state[qc] = h_sb

            def tail_stage3(qc, oc):
                """proj GEMM + residual add + store for one oc."""
                h_sb = tail_state[qc]
                qsl = slice(qc * QCH, (qc + 1) * QCH)
                po = ch_h.tile([P, QCH], F32, tag="po", name="po")
                for ci in range(NCT):
                    nc.tensor.matmul(
                        po[:], wpT[ci][:, oc * P : (oc + 1) * P],
                        _r(h_sb[ci][:]),
                        start=(ci == 0), stop=(ci == NCT - 1),
                        skip_group_check=True)
                ot = out_pool.tile([P, QCH], F32, tag="ot", name="ot")
                nc.vector.tensor_tensor(ot[:], po[:], xbd[oc][:, qsl],
                                        OP.add)
                nc.sync.dma_start(out_d.ap()[oc * P : (oc + 1) * P, qsl],
                                  ot[:])

            # ================= phase A (query half 0) =================
            NP2 = NKT // 2
            ph_cur = pd_cur = None
            av_done = 0  # p index consumed for current chain
            for kt in range(NKT):
                # leftover GEMM injections: V^T 8..31 at kt 0..11 (2/kt),
                # K chunks 1..7 at kt 0..6; Q half-1 at kt 2..5; bias
                # batches (GPSIMD) once their hx columns have landed.
                if kt < 12:
                    emit_vt2(8 + 2 * kt)
                    emit_vt2(9 + 2 * kt)
                if kt < 7:
                    for oc in range(NCT):
                        emit_qk2("k", oc, (kt + 1) * QCH)
                if 2 <= kt < 6:
                    j = kt - 2
                    emit_qk2("q", j % 2, 2 * QCH + (j // 2) * QCH)
                if kt == 11:
                    emit_bias(16, 24)
                if kt == 14:
                    emit_bias(24, 32)
                if kt in (2, 4):
                    # residual tiles on the (idle) GPSIMD engine, emitted
                    # here so the scheduler keeps them off the bias path
                    ci = kt // 2 - 1
                    nc.gpsimd.tensor_scalar(xbd[ci][:], _f(xA[ci][:]),
                                            dcc[ci][:], None, op0=OP.add)
                if kt == 16:
                    p_qkv2.close()

                emit_scores_exp(kt, 0)

                if kt >= 17:
                    if kt == 17:
                        ch_h = p_ch.enter_context(
                            tc.tile_pool(name="ps_ch", bufs=1, space="PSUM"))
                        ph_cur = [ch_h.tile([P, QCH], F32, tag=f"h{ct}",
                                            name=f"h{ct}")
                                  for ct in range(NCT)]
                        pd_cur = ch_h.tile([P, QCH], F32, tag="d", name="d")
                    # consume p with exp done (2p+1 <= kt), max 2/step
                    target = min((kt - 1) // 2 + 1, NP2)
                    budget = 2
                    while av_done < target and budget > 0:
                        av_step(0, av_done, ph_cur, pd_cur,
                                av_done == 0, av_done == NP2 - 1)
                        av_done += 1
                        budget -= 1
            # finish qc0 chain (p15 needs the last A exp)
            while av_done < NP2:
                av_step(0, av_done, ph_cur, pd_cur,
                        av_done == 0, av_done == NP2 - 1)
                av_done += 1

            # ================= phase B (query half 1) =================
            tail_stage1(0, ph_cur, pd_cur)
            qc_av = 1        # chain currently running
            av_done = 0
            for kt in range(NKT):
                emit_scores_exp(kt, 1)
                if kt == 0:
                    tail_stage2(0)   # frees the qc0 chain PSUM
                if kt == 1:
                    tail_stage3(0, 0)
                if kt == 2:
                    tail_stage3(0, 1)
                # AV for qc1 (burst; all H0..no, all its exps exist) then
                # qc2 (paced behind the B exp sweep)
                if kt >= 1 and qc_av <= 2:
                    if qc_av == 1:
                        target = NP2
                        budget = 3
                    else:
                        target = min((kt - 1) // 2 + 1, NP2)
                        budget = 4
                    while av_done < target and budget > 0:
                        av_step(qc_av, av_done, ph_cur, pd_cur,
                                av_done == 0, av_done == NP2 - 1)
                        av_done += 1
                        budget -= 1
                    if av_done == NP2:
                        tail_stage1(qc_av, ph_cur, pd_cur)
                        tail_stage2(qc_av)
                        if qc_av == 1:
                            qc_av = 2
                            av_done = 0
                        else:
                            qc_av = 3
                if kt == 10:
                    tail_stage3(1, 0)
                if kt == 11:
                    tail_stage3(1, 1)
            # ================= epilogue: qc2 tail + qc3 ===============
            if qc_av == 2:
                while av_done < NP2:
                    av_step(2, av_done, ph_cur, pd_cur,
                            av_done == 0, av_done == NP2 - 1)
                    av_done += 1
                tail_stage1(2, ph_cur, pd_cur)
                tail_stage2(2)
            tail_stage3(2, 0)
            av_done = 0
            while av_done < NP2:
                av_step(3, av_done, ph_cur, pd_cur,
                        av_done == 0, av_done == NP2 - 1)
                av_done += 1
            tail_stage3(2, 1)
            tail_stage1(3, ph_cur, pd_cur)
            tail_stage2(3)
            tail_stage3(3, 0)
            tail_stage3(3, 1)
            p_tail.close()
            p_ch.close()
            p_sw.close()

    _split_excess_waits(nc)
    return nc


def make_in_maps(x, norm_gamma, norm_beta, qkv_w, qkv_b, proj_w, proj_b):
    f = np.float32
    d = np.float64
    qkv_w = np.asarray(qkv_w, dtype=d)
    qkv_b = np.asarray(qkv_b, dtype=d)
    proj_w = np.asarray(proj_w, dtype=d)
    proj_b = np.asarray(proj_b, dtype=d)
    g = np.asarray(norm_gamma, dtype=d)
    beta = np.asarray(norm_beta, dtype=d)
    Wq, Wk, Wv = qkv_w[0:C], qkv_w[C : 2 * C], qkv_w[2 * C : 3 * C]
    bq, bk, bv = qkv_b[0:C], qkv_b[C : 2 * C], qkv_b[2 * C : 3 * C]

    wqT = (Wq.T * g[:, None])          # [c_in, c_out], rows scaled by gamma
    wkT = (Wk.T * g[:, None])
    wvT = (Wv.T * g[:, None])
    u1 = bq + Wq @ beta
    u2 = Wq @ g
    h1 = wkT @ u1
    h2 = wkT @ u2
    dc1 = proj_w @ (bv + Wv @ beta) + proj_b
    pc2 = proj_w @ (Wv @ g)

    wpack = np.zeros((C, PACKW), dtype=f)
    wpack[:, 0:C] = wvT
    wpack[:, C] = h1
    wpack[:, C + 1] = h2
    wpack[:, 258 : 258 + C] = wqT
    wpack[:, 258 + C : 258 + 2 * C] = wkT
    wpack[:, 258 + 2 * C : 258 + 3 * C] = proj_w.T
    wpack[:, 258 + 3 * C] = dc1
    wpack[:, 259 + 3 * C] = pc2
    wpack = np.ascontiguousarray(wpack)

    in_maps = []
    xf = np.asarray(x, dtype=f).reshape(B, C, N)
    for core in range(8):
        b, h = divmod(core, 2)
        xs = xf[b]
        if h == 1:
            xs = np.concatenate([xs[:, NQ:], xs[:, :NQ]], axis=1)
        in_maps.append({"x": np.ascontiguousarray(xs), "wpack": wpack})
    return in_maps


def assemble_output(results):
    out = np.empty((B, C, N), dtype=np.float32)
    for core in range(8):
        b, h = divmod(core, 2)
        out[b][:, h * NQ : (h + 1) * NQ] = results[core]["out"]
    return out.reshape(B, C, HH, WW, DD)


_PROGRAM = None
_N_CALLS = 0
_RUNNER = None


def get_program():
    global _PROGRAM
    if _PROGRAM is None:
        _PROGRAM = build_program()
    return _PROGRAM


def _build_cached_runner(nc):
    """Persistent jitted executor (same execution path that
    run_bass_kernel_spmd takes under axon, via bass2jax/PJRT) so repeat
    kernel() calls skip the multi-minute neuronx-cc recompile."""
    import jax
    from jax.experimental.shard_map import shard_map
    from jax.sharding import Mesh, PartitionSpec
    from concourse import bass2jax

    bass2jax.install_neuronx_cc_hook()
    n_cores = 8
    partition_name = (nc.partition_id_tensor.name
                      if nc.partition_id_tensor else None)
    in_names, out_names, out_avals, zero_outs = [], [], [], []
    for alloc in nc.m.functions[0].allocations:
        if not isinstance(alloc, mybir.MemoryLocationSet):
            continue
        name = alloc.memorylocations[0].name
        if alloc.kind == "ExternalInput":
            if name != partition_name:
                in_names.append(name)
        elif alloc.kind == "ExternalOutput":
            out_names.append(name)
            shape = tuple(alloc.tensor_shape)
            dtype = mybir.dt.np(alloc.dtype)
            out_avals.append(jax.core.ShapedArray(shape, dtype))
            zero_outs.append(np.zeros(shape, dtype))
    n_params = len(in_names)
    all_in_names = list(in_names) + list(out_names)
    if partition_name is not None:
        all_in_names.append(partition_name)

    def _body(*args):
        operands = list(args)
        if partition_name is not None:
            operands.append(bass2jax.partition_id_tensor())
        outs = bass2jax._bass_exec_p.bind(
            *operands,
            out_avals=tuple(out_avals),
            in_names=tuple(all_in_names),
            out_names=tuple(out_names),
            lowering_input_output_aliases=(),
            sim_require_finite=True,
            sim_require_nnan=True,
            nc=nc,
        )
        return tuple(outs)

    devices = jax.devices()[:n_cores]
    mesh = Mesh(np.asarray(devices), ("core",))
    n_outs = len(out_names)
    fn = jax.jit(
        shard_map(_body, mesh=mesh,
                  in_specs=(PartitionSpec("core"),) * (n_params + n_outs),
                  out_specs=(PartitionSpec("core"),) * n_outs,
                  check_rep=False),
        keep_unused=True,
    )

    def run(in_maps):
        per_core = [[np.asarray(m[name]) for name in in_names]
                    for m in in_maps]
        concat_in = [
            np.concatenate([per_core[c][i] for c in range(n_cores)], axis=0)
            for i in range(n_params)
        ]
        concat_zeros = [
            np.zeros((n_cores * z.shape[0], *z.shape[1:]), z.dtype)
            for z in zero_outs
        ]
        out_arrs = fn(*concat_in, *concat_zeros)
        return [
            {name: np.asarray(out_arrs[i]).reshape(
                n_cores, *out_avals[i].shape)[c]
             for i, name in enumerate(out_names)}
            for c in range(n_cores)
        ]

    return run


def kernel(x, norm_gamma, norm_beta, qkv_w, qkv_b, proj_w, proj_b):
    global _N_CALLS, _RUNNER
    nc = get_program()
    in_maps = make_in_maps(x, norm_gamma, norm_beta, qkv_w, qkv_b,
                           proj_w, proj_b)
    _N_CALLS += 1
    if _N_CALLS == 1:
        res = run_bass_kernel_spmd(nc, in_maps, core_ids=list(range(8)))
        return assemble_output(res.results)
    if _RUNNER is None:
        _RUNNER = _build_cached_runner(nc)
    return assemble_output(_RUNNER(in_maps))


# revision 17
# speedup vs baseline: 1.1547x; 1.0375x over previous
"""AttnBlock3d (GroupNorm + single-head self-attention + proj + residual)
on 8 Trainium2 NeuronCores.

Sharding: 8 shards = (batch sample b in 0..3) x (query-half h in 0..1).
Every core runs the SAME program (SPMD): the host permutes each sample's
N=4096 spatial columns so that the core's 2048 query positions come
first. GroupNorm / K / V are permutation-invariant in the column order,
and attention output for a query column does not depend on the ordering
of key columns, so the math is unchanged.

Key algebra (all exact; lets every big GEMM start without waiting for
the GroupNorm statistics):
  xn = A*x + Bvec per channel, A = inv_std*gamma, Bvec = beta - mean*A.
  The gamma factor is folded into the weights on the HOST
  (W' = W diag(gamma)), so q = Wq@xn + bq = inv_std*(Wq'@x) + bq2.
  Softmax over k is invariant to anything constant along k, so only the
  [k]-indexed part of the score bias survives; it comes out of two
  extra output columns of the V^T GEMM (h1.x, h2.x) combined with the
  stats. inv^2*SCALE enters through the ACT Exp per-partition scale.
  The V-side affine (v = inv*v_raw + cvv) is folded THROUGH the proj:
  out = WpT@(ph * inv/denom) + [Wp@cvv + Wp@bv + bp + x], with the
  channel constant dcc = dc1 + (-mean*inv)*pc2 built from host vectors
  dc1 = Wp@(bv + Wv@beta) + bp and pc2 = Wp@Wv@gamma, pre-added into
  the residual tiles.

Schedule (q-major two-phase sweep):
  - prologue: 2 packed weight DMAs + 8 x-piece DMAs; GroupNorm moments
    via PE column-sum matmuls + ACT Squares (both idle then); Q(H0)/K0
    GEMMs and fp8 copies so the first exp fires as soon as the stats
    chain resolves.
  - phase A: for kt in 0..31: scoresT(kt, query-half H0) -> 1024-wide
    Exp. Leftover K chunks / V^T tiles / Q(H1) GEMMs ride the PE+DVE
    slack early in A; AV chain for qc0 runs kt>=16 (PSUM frees then);
  - phase B: same over H1; AV for qc1 (burst) + qc2 (paced) and the
    divide/proj/residual tails for qc0..2 all inside the sweep.
  - epilogue: only qc3's AV + tail.
  PSUM: scores 2x[128,1024] (4 banks) + AV chain ph0,ph1,pd (3) +
  proj po (1) = 8, with prologue pools (stats, qkv) scoped to close
  before the chain/proj pools open.
"""

import numpy as np
from contextlib import ExitStack

import bass_rust
import concourse.bass as bass
import concourse.tile as tile
from concourse import mybir
from concourse.bass_utils import run_bass_kernel_spmd

F32 = mybir.dt.float32
F32R = mybir.dt.float32r
BF16 = mybir.dt.bfloat16
FP8 = mybir.dt.float8e4
AX = mybir.AxisListType
OP = mybir.AluOpType
AF = mybir.ActivationFunctionType

B, C, HH, WW, DD = 4, 256, 16, 16, 16
N = HH * WW * DD          # 4096 spatial positions per sample
NQ = N // 2               # 2048 query positions per core
P = 128                   # partitions
NCT = C // P              # 2 channel tiles
NKT = N // P              # 32 key tiles
QCH = 512                 # q-chunk width (one PSUM bank of fp32)
NQC = NQ // QCH           # 4 q chunks
EPS = 1e-6
SCALE = float(C) ** -0.5  # 0.0625
PACKW = 258 + 3 * C + 2   # wv_ext | wq | wk | wp | dc1 | pc2


def _split_excess_waits(nc, cap=1):
    """walrus in this env rejects >1 sync wait per instruction; peel
    extras onto no-ops inserted before the offender on the same engine."""
    n = 0
    for f in nc.m.functions:
        for blk in f.blocks:
            insts = blk.instructions
            new_insts = []
            for inst in insts:
                si = inst.sync_info
                if si is not None and si.on_wait is not None and len(si.on_wait) > cap:
                    waits = list(si.on_wait)
                    extra, keep = waits[:-cap], waits[-cap:]
                    for j in range(0, len(extra), cap):
                        nop = mybir.InstNoOp(
                            name=f"{inst.name}_ws{j}", ins=[], outs=[]
                        )
                        nop.engine = inst.engine
                        nop.sync_info = bass_rust.SyncInfo(
                            on_wait=extra[j : j + cap], on_update=[]
                        )
                        new_insts.append(nop)
                    inst.sync_info = bass_rust.SyncInfo(
                        on_wait=keep, on_update=list(si.on_update)
                    )
                    n += 1
                new_insts.append(inst)
            if len(new_insts) != len(insts):
                insts[:] = new_insts
    return n


def _r(ap):
    return ap.bitcast(F32R)


def _f(ap):
    return ap.bitcast(F32)


def build_program():
    nc = bass.Bass("TRN2", target_bir_lowering=False, debug=False)

    x_d = nc.dram_tensor("x", [C, N], F32R, kind="ExternalInput")
    w_d = nc.dram_tensor("wpack", [C, PACKW], F32R, kind="ExternalInput")
    out_d = nc.dram_tensor("out", [C, NQ], F32, kind="ExternalOutput")

    with tile.TileContext(nc) as tc, ExitStack() as ctx:
        # ---------- persistent pools ----------
        consts = ctx.enter_context(tc.tile_pool(name="consts", bufs=1))
        qk_pool = ctx.enter_context(tc.tile_pool(name="qk", bufs=1))
        vt_pool = ctx.enter_context(tc.tile_pool(name="vt", bufs=NKT // 2))
        xb_pool = ctx.enter_context(tc.tile_pool(name="xb", bufs=1))
        x_pool = ctx.enter_context(tc.tile_pool(name="xio", bufs=1))
        exp_pool = ctx.enter_context(tc.tile_pool(name="expt", bufs=NKT // 2))

        # packed weights: 2 DMAs total (one per channel tile)
        wpk = [consts.tile([P, PACKW], F32R, tag=f"wpk{ci}", name=f"wpk{ci}")
               for ci in range(NCT)]
        for ci in range(NCT):
            nc.sync.dma_start(wpk[ci][:], w_d.ap()[ci * P : (ci + 1) * P, :])
        wv_ext = [wpk[ci][:, 0:258] for ci in range(NCT)]
        wqA = [wpk[ci][:, 258 : 258 + C] for ci in range(NCT)]
        wkA = [wpk[ci][:, 258 + C : 258 + 2 * C] for ci in range(NCT)]
        wpT = [wpk[ci][:, 258 + 2 * C : 258 + 3 * C] for ci in range(NCT)]
        dc1 = [_f(wpk[ci][:, 258 + 3 * C : 259 + 3 * C]) for ci in range(NCT)]
        pc2 = [_f(wpk[ci][:, 259 + 3 * C : 260 + 3 * C]) for ci in range(NCT)]

        # x in 8 pieces of [128, 1024]: both channel tiles of the query
        # half first so Q/V^T GEMMs and stats start at ~1/4 of the load.
        xA = [x_pool.tile([P, NQ], F32R, tag=f"xA{ci}", name=f"xA{ci}")
              for ci in range(NCT)]
        xB = [x_pool.tile([P, NQ], F32R, tag=f"xB{ci}", name=f"xB{ci}")
              for ci in range(NCT)]
        x_pieces = []  # (ci, tile, col_lo, width) in DMA order; the
        # final piece is small so the last GroupNorm square (which gates
        # the stats chain) finishes right after the last DMA byte.
        plan = [(0, 0, 0, 1024), (0, 1, 0, 1024),
                (0, 0, 1024, 1024), (0, 1, 1024, 1024),
                (1, 0, 0, 1024), (1, 1, 0, 1024),
                (1, 0, 1024, 1024), (1, 1, 1024, 768), (1, 1, 1792, 256)]
        for half, ci, lo, w in plan:
            xt = (xA, xB)[half][ci]
            nc.sync.dma_start(
                xt[:, lo : lo + w],
                x_d.ap()[ci * P : (ci + 1) * P,
                         half * NQ + lo : half * NQ + lo + w])
            x_pieces.append((ci, xt, lo, w))

        def xs(ci, lo, w):
            """f32r view of x columns [lo, lo+w) (must not straddle NQ)."""
            if lo < NQ:
                assert lo + w <= NQ
                return xA[ci][:, lo : lo + w]
            return xB[ci][:, lo - NQ : lo - NQ + w]

        ones_col = consts.tile([P, 1], F32, tag="ones_col", name="ones_col")
        nc.vector.memset(ones_col[:], 1.0)
        ones_fp8 = consts.tile([P, 32], FP8, tag="ones_fp8", name="ones_fp8")
        nc.vector.memset(ones_fp8[:], 1.0)
        ones_row = consts.tile([1, P], F32, tag="ones_row", name="ones_row")
        nc.vector.memset(ones_row[:], 1.0)

        q2 = qk_pool.tile([P, NCT * NQ], FP8, tag="q2", name="q2")
        k2 = qk_pool.tile([P, NCT * N], FP8, tag="k2", name="k2")
        vT = [vt_pool.tile([P, 2 * C], FP8, tag="vt", name="vt")
              for _ in range(NKT // 2)]
        xbd = [xb_pool.tile([P, NQ], F32, tag=f"xb{ci}", name=f"xb{ci}")
               for ci in range(NCT)]
        hx = qk_pool.tile([P, 2 * NKT], F32, tag="hx", name="hx")
        hx3 = hx[:].rearrange("p (c k) -> p c k", c=2)
        bias_k = qk_pool.tile([P, NKT], F32, tag="bias_k", name="bias_k")
        # stats-derived broadcast columns:
        # [inv, -mean*inv, S*inv, S*inv^2, -S*mean*inv^2]
        mi_bc = consts.tile([P, 5], F32, tag="mi_bc", name="mi_bc")
        inv_row = consts.tile([1, P], F32, tag="inv_row", name="inv_row")
        dcc = [consts.tile([P, 1], F32, tag=f"dcc{ci}", name=f"dcc{ci}")
               for ci in range(NCT)]

        p_pre = ExitStack()
        st_pool = p_pre.enter_context(tc.tile_pool(name="stats", bufs=1))
        ps_st = p_pre.enter_context(
            tc.tile_pool(name="ps_st", bufs=1, space="PSUM"))
        ps_qkp = p_pre.enter_context(
            tc.tile_pool(name="ps_qkp", bufs=2, space="PSUM"))

        # ---------- GroupNorm moments ----------
        # column sums on the PE (idle in the prologue): 16 accumulating
        # [1,512] matmuls in x-piece DMA order; sums of squares on the
        # ACT (also idle) with the free-dim accumulator. Stats for piece
        # i are interleaved with the prologue GEMMs so at most two
        # not-yet-satisfied stat matmuls sit in the PE wait queue.
        colsum = ps_st.tile([1, QCH], F32, tag="colsum", name="colsum")
        NPC = len(x_pieces)
        q4 = st_pool.tile([P, NPC], F32, tag="q4", name="q4")
        scr = st_pool.tile([P, NQ // 2], F32, tag="scr", name="scr")

        def emit_stats(i):
            ci, xt, lo, w = x_pieces[i]
            with tc.high_priority():
                off = 0
                while off < w:
                    cw = min(QCH, w - off)
                    nc.tensor.matmul(
                        colsum[0:1, 0:cw], _r(ones_col[:]),
                        xt[:, lo + off : lo + off + cw],
                        start=(i == 0 and off == 0),
                        stop=(i == NPC - 1 and off + cw == w),
                        skip_group_check=True)
                    off += cw
                nc.scalar.activation(scr[:, 0:w], _f(xt[:, lo : lo + w]),
                                     AF.Square, accum_out=q4[:, i : i + 1])

        # ---------- prologue GEMMs (emission order = arrival order) ----
        def emit_vt(kt):
            lo = kt * P
            pv = ps_qkp.tile([P, C + 2], F32, tag="ps_v", name="ps_v")
            for ci in range(NCT):
                nc.tensor.matmul(pv[:], xs(ci, lo, P), wv_ext[ci],
                                 start=(ci == 0), stop=(ci == NCT - 1))
            # stash the two bias columns so pv can be released without
            # waiting for the stats; bias_k is batch-built later.
            nc.vector.tensor_copy(hx3[:, :, kt : kt + 1], pv[:, C : C + 2])
            nc.vector.tensor_copy(
                vT[kt // 2][:, (kt % 2) * C : (kt % 2 + 1) * C], pv[:, 0:C])

        def emit_qk(which, oc, col):
            w = wqA if which == "q" else wkA
            dst, width = (q2, NQ) if which == "q" else (k2, N)
            pk = ps_qkp.tile([P, QCH], F32, tag="ps_qk", name="ps_qk")
            for ci in range(NCT):
                nc.tensor.matmul(pk[:],
                                 w[ci][:, oc * P : (oc + 1) * P],
                                 xs(ci, col, QCH),
                                 start=(ci == 0), stop=(ci == NCT - 1))
            nc.vector.tensor_copy(
                dst[:, oc * width + col : oc * width + col + QCH], pk[:])

        # stats p0,p1 | V^T 0..3 + Q(qc0) | stats p2,p3 | V^T 4..7 +
        # Q(qc1) + K0 | stats p4..7 (all GEMMs need only x pieces 0,1)
        emit_stats(0)
        emit_stats(1)
        for kt in range(4):
            emit_vt(kt)
        for oc in range(NCT):
            emit_qk("q", oc, 0)
        emit_stats(2)
        emit_stats(3)
        for kt in range(4, 8):
            emit_vt(kt)
        for oc in range(NCT):
            emit_qk("q", oc, QCH)
        for oc in range(NCT):
            emit_qk("k", oc, 0)
        for i in range(4, NPC):
            emit_stats(i)

        # ---------- stats chain ----------
        # Entirely on the ACT engine (idle in the prologue, and immune
        # to the DVE copy congestion): activation computes
        # func(in*scale + bias) with per-partition AP scale, which gives
        # scalar-scalar multiply via scale=AP. The two tensor-tensor
        # combines that ACT cannot do (dcc, bias_k) go to GPSIMD.
        with tc.high_priority():
            s_tot = st_pool.tile([1, 2], F32, tag="stot", name="stot")
            scr8 = st_pool.tile([1, NPC], F32, tag="scr8", name="scr8")
            nc.vector.tensor_reduce(s_tot[:, 0:1], colsum[:], axis=AX.X,
                                    op=OP.add)
            psq = ps_st.tile([1, NPC], F32, tag="psq", name="psq")
            nc.tensor.matmul(psq[:], ones_col[:], q4[:])
            nc.scalar.activation(scr8[:], psq[:], AF.Copy,
                                 accum_out=s_tot[:, 1:2])

            inv_cn = 1.0 / float(C * N)
            mean_sb = st_pool.tile([1, 1], F32, tag="mean", name="mean")
            nc.scalar.activation(mean_sb[:], s_tot[:, 0:1], AF.Copy,
                                 scale=inv_cn)
            msq = st_pool.tile([1, 1], F32, tag="msq", name="msq")
            nc.scalar.activation(msq[:], mean_sb[:], AF.Square)
            epsm = st_pool.tile([1, 1], F32, tag="epsm", name="epsm")
            nc.scalar.activation(epsm[:], msq[:], AF.Copy, scale=-1.0,
                                 bias=EPS)
            lnv = st_pool.tile([1, 1], F32, tag="lnv", name="lnv")
            nc.scalar.activation(lnv[:], s_tot[:, 1:2], AF.Ln, scale=inv_cn,
                                 bias=epsm[:])
            mi_sb = st_pool.tile([1, 5], F32, tag="mi", name="mi")
            inv_c = mi_sb[:, 0:1]
            nc.scalar.activation(inv_c, lnv[:], AF.Exp, scale=-0.5)
            ninv = st_pool.tile([1, 1], F32, tag="ninv", name="ninv")
            nc.scalar.activation(ninv[:], inv_c, AF.Copy, scale=-1.0)
            nc.scalar.activation(mi_sb[:, 1:2], mean_sb[:], AF.Copy,
                                 scale=ninv[:])                  # -mean*inv
            nc.scalar.activation(mi_sb[:, 2:3], inv_c, AF.Copy,
                                 scale=SCALE)                    # S*inv
            nc.scalar.activation(mi_sb[:, 3:4], inv_c, AF.Copy,
                                 scale=mi_sb[:, 2:3])            # S*inv^2
            nc.scalar.activation(mi_sb[:, 4:5], mi_sb[:, 1:2], AF.Copy,
                                 scale=mi_sb[:, 2:3])            # -S*m*inv^2
            ps_bc5 = ps_st.tile([P, 5], F32, tag="ps_bc5", name="ps_bc5")
            nc.tensor.matmul(ps_bc5[:], ones_row[:], mi_sb[:])
            nc.scalar.activation(mi_bc[:], ps_bc5[:], AF.Copy)
            nc.scalar.activation(inv_row[:], ones_row[:], AF.Copy,
                                 scale=mi_sb[:, 0:1])
            minv_neg = mi_bc[:, 1:2]
            si_bc = mi_bc[:, 2:3]
            si2_bc = mi_bc[:, 3:4]
            m2n_bc = mi_bc[:, 4:5]
            for ci in range(NCT):
                nc.gpsimd.tensor_scalar(dcc[ci][:], pc2[ci], minv_neg,
                                        dc1[ci], op0=OP.mult, op1=OP.add)
            # bias_k = S*inv*(h1.x) - S*mean*inv^2*(h2.x), batch 0..15
            # on the DVE (stt exists there; the first exps gate on it),
            # later batches on the idle GPSIMD (no stt -> 3 ops).
            bt = qk_pool.tile([P, 2 * NKT], F32, tag="bt", name="bt")

            def emit_bias(lo, hi):
                nc.gpsimd.tensor_scalar(bt[:, lo:hi], hx3[:, 0, lo:hi],
                                        si_bc, None, op0=OP.mult)
                nc.gpsimd.tensor_scalar(bt[:, 32 + lo : 32 + hi],
                                        hx3[:, 1, lo:hi], m2n_bc, None,
                                        op0=OP.mult)
                nc.gpsimd.tensor_tensor(bias_k[:, lo:hi], bt[:, lo:hi],
                                        bt[:, 32 + lo : 32 + hi], OP.add)

            nc.vector.tensor_scalar(bt[:, 0:16], hx3[:, 0, 0:16],
                                    si_bc, None, op0=OP.mult)
            nc.vector.scalar_tensor_tensor(bias_k[:, 0:16], hx3[:, 1, 0:16],
                                           m2n_bc, bt[:, 0:16],
                                           op0=OP.mult, op1=OP.add)

        p_pre.close()

        # ---------- sweep pools ----------
        p_sw = ExitStack()
        ps_s = p_sw.enter_context(
            tc.tile_pool(name="ps_s", bufs=2, space="PSUM"))
        p_qkv2 = ExitStack()
        ps_qk2 = p_qkv2.enter_context(
            tc.tile_pool(name="ps_qk2", bufs=2, space="PSUM"))

        def emit_vt2(kt):
            lo = kt * P
            pv = ps_qk2.tile([P, C + 2], F32, tag="ps_v2", name="ps_v2")
            for ci in range(NCT):
                nc.tensor.matmul(pv[:], xs(ci, lo, P), wv_ext[ci],
                                 start=(ci == 0), stop=(ci == NCT - 1))
            nc.vector.tensor_copy(hx3[:, :, kt : kt + 1], pv[:, C : C + 2])
            nc.vector.tensor_copy(
                vT[kt // 2][:, (kt % 2) * C : (kt % 2 + 1) * C], pv[:, 0:C])

        def emit_qk2(which, oc, col):
            w = wqA if which == "q" else wkA
            dst, width = (q2, NQ) if which == "q" else (k2, N)
            pk = ps_qk2.tile([P, QCH], F32, tag="ps_qk2t", name="ps_qk2t")
            for ci in range(NCT):
                nc.tensor.matmul(pk[:],
                                 w[ci][:, oc * P : (oc + 1) * P],
                                 xs(ci, col, QCH),
                                 start=(ci == 0), stop=(ci == NCT - 1))
            nc.vector.tensor_copy(
                dst[:, oc * width + col : oc * width + col + QCH], pk[:])

        k3all = k2[:].rearrange("p (j n) -> p j n", j=2)
        q3 = q2[:].rearrange("p (j n) -> p j n", j=2)

        def emit_scores_exp(kt, half):
            """scoresT + 1024-wide exp for (key tile kt, query half).
            High priority: the exp stream is the whole-kernel critical
            path, so its scores matmuls must win PE arbitration over AV
            bursts whenever both are ready."""
            if kt % 2 == 0 and half == 0:
                exp_tiles[kt // 2] = exp_pool.tile(
                    [P, 2 * NQ], FP8, tag="expt", name="expt")
            with tc.high_priority():
                ps = ps_s.tile([P, 2 * QCH], F32, tag="s", name="s")
                k3 = k3all[:, :, kt * P : (kt + 1) * P]
                for qh in range(2):
                    qcol = half * 2 * QCH + qh * QCH
                    nc.tensor.matmul(
                        ps[:, qh * QCH : (qh + 1) * QCH],
                        k3, q3[:, :, qcol : qcol + QCH],
                        skip_group_check=True,
                        perf_mode=mybir.MatmulPerfMode.DoubleRow)
                lo = (kt % 2) * NQ + half * 2 * QCH
                nc.scalar.activation(
                    exp_tiles[kt // 2][:, lo : lo + 2 * QCH],
                    ps[:], AF.Exp, scale=si2_bc, bias=bias_k[:, kt : kt + 1])

        exp_tiles = [None] * (NKT // 2)
        ones3 = ones_fp8[:].rearrange("p (j o) -> p j o", j=2)[:, :, 0:1]

        # AV chain state (one chain at a time; 3 PSUM banks)
        p_ch = ExitStack()
        ch_h = None  # opened lazily at phase-A kt16

        def av_step(qc, p, ph, pd, first, last):
            et3 = exp_tiles[p].rearrange(
                "p (j q) -> p j q", j=2)[:, :, qc * QCH : (qc + 1) * QCH]
            vt3 = vT[p].rearrange("p (j c) -> p j c", j=2)
            for ct in range(NCT):
                nc.tensor.matmul(
                    ph[ct][:], vt3[:, :, ct * P : (ct + 1) * P], et3[:],
                    start=first, stop=last, skip_group_check=True,
                    perf_mode=mybir.MatmulPerfMode.DoubleRow)
            nc.tensor.matmul(
                pd[0:1, :], ones3, et3[:],
                start=first, stop=last, skip_group_check=True,
                perf_mode=mybir.MatmulPerfMode.DoubleRow)

        p_tail = ExitStack()

        with tc.tile_pool(name="att_sb", bufs=2) as att_pool, \
             tc.tile_pool(name="out_sb", bufs=4) as out_pool:

            tail_state = {}

            def tail_stage1(qc, ph, pd):
                """recip + inv-scaled broadcast (DVE + PE). The
                broadcast lands back in the (now-free) denominator bank,
                so no extra PSUM bank is needed."""
                rec = att_pool.tile([1, QCH], F32, tag="rec", name="rec")
                with nc.allow_low_precision(reason="f32r fp32-width"):
                    nc.vector.reciprocal(_r(rec[:]), pd[0:1, :])
                nc.tensor.matmul(pd[:], _r(inv_row[:]), _r(rec[:]),
                                 skip_group_check=True)
                rec_bc = att_pool.tile([P, QCH], F32, tag="rec_bc",
                                       name="rec_bc")
                nc.vector.tensor_copy(rec_bc[:], pd[:])
                tail_state[qc] = (ph, rec_bc)

            def tail_stage2(qc):
                """h = ph * (inv/denom) into SBUF; releases the chain."""
                ph, rec_bc = tail_state[qc]
                h_sb = []
                for ct in range(NCT):
                    h = att_pool.tile([P, QCH], F32, tag=f"hsb{ct}",
                                      name=f"hsb{ct}")
                    nc.vector.tensor_tensor(_r(h[:]), ph[ct][:], rec_bc[:],
                                            OP.mult)
                    h_sb.append(h)
                tail_state[qc] = h_sb

            def tail_stage3(qc, oc, ep=False):
                """proj GEMM + residual add + store for one oc. In the
                epilogue the scores banks are idle; borrow them for po so
                consecutive proj GEMMs don't serialize on one bank."""
                h_sb = tail_state[qc]
                qsl = slice(qc * QCH, (qc + 1) * QCH)
                if ep:
                    po = ps_s.tile([P, 2 * QCH], F32, tag="s",
                                   name="po_ep")[:, 0:QCH]
                else:
                    po = ch_h.tile([P, QCH], F32, tag="po", name="po")
                for ci in range(NCT):
                    nc.tensor.matmul(
                        po[:], wpT[ci][:, oc * P : (oc + 1) * P],
                        _r(h_sb[ci][:]),
                        start=(ci == 0), stop=(ci == NCT - 1),
                        skip_group_check=True)
                ot = out_pool.tile([P, QCH], F32, tag="ot", name="ot")
                nc.vector.tensor_tensor(ot[:], po[:], xbd[oc][:, qsl],
                                        OP.add)
                nc.sync.dma_start(out_d.ap()[oc * P : (oc + 1) * P, qsl],
                                  ot[:])

            # ================= phase A (query half 0) =================
            NP2 = NKT // 2
            ph_cur = pd_cur = None
            av_done = 0  # p index consumed for current chain
            for kt in range(NKT):
                # leftover GEMM injections: V^T 8..31 at kt 0..11 (2/kt),
                # K chunks 1..7 at kt 0..6; Q half-1 at kt 2..5; bias
                # batches (GPSIMD) once their hx columns have landed.
                if kt < 12:
                    emit_vt2(8 + 2 * kt)
                    emit_vt2(9 + 2 * kt)
                if kt < 7:
                    for oc in range(NCT):
                        emit_qk2("k", oc, (kt + 1) * QCH)
                if 2 <= kt < 6:
                    j = kt - 2
                    emit_qk2("q", j % 2, 2 * QCH + (j // 2) * QCH)
                if kt == 11:
                    emit_bias(16, 24)
                if kt == 14:
                    emit_bias(24, 32)
                if kt in (18, 20):
                    # residual tiles on the (idle) GPSIMD engine, emitted
                    # here so the scheduler keeps them off the bias path
                    ci = kt // 2 - 9
                    nc.gpsimd.tensor_scalar(xbd[ci][:], _f(xA[ci][:]),
                                            dcc[ci][:], None, op0=OP.add)
                if kt == 16:
                    p_qkv2.close()

                emit_scores_exp(kt, 0)

                if kt >= 17:
                    if kt == 17:
                        ch_h = p_ch.enter_context(
                            tc.tile_pool(name="ps_ch", bufs=1, space="PSUM"))
                        ph_cur = [ch_h.tile([P, QCH], F32, tag=f"h{ct}",
                                            name=f"h{ct}")
                                  for ct in range(NCT)]
                        pd_cur = ch_h.tile([P, QCH], F32, tag="d", name="d")
                    # consume p with exp done (2p+1 <= kt), max 2/step
                    target = min((kt - 1) // 2 + 1, NP2)
                    budget = 2
                    while av_done < target and budget > 0:
                        av_step(0, av_done, ph_cur, pd_cur,
                                av_done == 0, av_done == NP2 - 1)
                        av_done += 1
                        budget -= 1
            # finish qc0 chain (p15 needs the last A exp)
            while av_done < NP2:
                av_step(0, av_done, ph_cur, pd_cur,
                        av_done == 0, av_done == NP2 - 1)
                av_done += 1

            # ================= phase B (query half 1) =================
            tail_stage1(0, ph_cur, pd_cur)
            qc_av = 1        # chain currently running
            av_done = 0
            for kt in range(NKT):
                emit_scores_exp(kt, 1)
                if kt == 0:
                    tail_stage2(0)   # frees the qc0 chain PSUM
                if kt == 1:
                    tail_stage3(0, 0)
                if kt == 2:
                    tail_stage3(0, 1)
                # AV for qc1 (burst; all H0..no, all its exps exist) then
                # qc2 (paced behind the B exp sweep)
                if kt >= 1 and qc_av <= 2:
                    if qc_av == 1:
                        target = NP2
                        budget = 3
                    else:
                        target = min((kt - 1) // 2 + 1, NP2)
                        budget = 4
                    while av_done < target and budget > 0:
                        av_step(qc_av, av_done, ph_cur, pd_cur,
                                av_done == 0, av_done == NP2 - 1)
                        av_done += 1
                        budget -= 1
                    if av_done == NP2:
                        tail_stage1(qc_av, ph_cur, pd_cur)
                        tail_stage2(qc_av)
                        if qc_av == 1:
                            qc_av = 2
                            av_done = 0
                        else:
                            qc_av = 3
                if kt == 10:
                    tail_stage3(1, 0)
                if kt == 11:
                    tail_stage3(1, 1)
            # ================= epilogue: qc2 tail + qc3 ===============
            # qc3's denominator accumulates FIRST (the d bank frees as
            # soon as qc2's rec_bc is copied out), so its reciprocal +
            # broadcast chain overlaps the qc3 ph matmuls; epilogue proj
            # matmuls borrow idle scores banks to avoid po-bank churn.
            if qc_av == 2:
                while av_done < NP2:
                    av_step(2, av_done, ph_cur, pd_cur,
                            av_done == 0, av_done == NP2 - 1)
                    av_done += 1
                tail_stage1(2, ph_cur, pd_cur)
                tail_stage2(2)

            def av3_pd(p, first, last):
                et3 = exp_tiles[p].rearrange(
                    "p (j q) -> p j q", j=2)[:, :, 3 * QCH : 4 * QCH]
                nc.tensor.matmul(
                    pd_cur[0:1, :], ones3, et3[:],
                    start=first, stop=last, skip_group_check=True,
                    perf_mode=mybir.MatmulPerfMode.DoubleRow)

            def av3_ph(p, first, last):
                et3 = exp_tiles[p].rearrange(
                    "p (j q) -> p j q", j=2)[:, :, 3 * QCH : 4 * QCH]
                vt3 = vT[p].rearrange("p (j c) -> p j c", j=2)
                for ct in range(NCT):
                    nc.tensor.matmul(
                        ph_cur[ct][:], vt3[:, :, ct * P : (ct + 1) * P],
                        et3[:], start=first, stop=last,
                        skip_group_check=True,
                        perf_mode=mybir.MatmulPerfMode.DoubleRow)

            for p in range(NP2):
                av3_pd(p, p == 0, p == NP2 - 1)
            tail_stage1(3, None, pd_cur)
            tail_stage3(2, 0, ep=True)
            for p in range(NP2):
                av3_ph(p, p == 0, p == NP2 - 1)
            tail_stage3(2, 1, ep=True)
            tail_state[3] = (ph_cur, tail_state[3][1])
            tail_stage2(3)
            tail_stage3(3, 0, ep=True)
            tail_stage3(3, 1, ep=True)
            p_tail.close()
            p_ch.close()
        p_sw.close()

    _split_excess_waits(nc)
    return nc


def make_in_maps(x, norm_gamma, norm_beta, qkv_w, qkv_b, proj_w, proj_b):
    f = np.float32
    d = np.float64
    qkv_w = np.asarray(qkv_w, dtype=d)
    qkv_b = np.asarray(qkv_b, dtype=d)
    proj_w = np.asarray(proj_w, dtype=d)
    proj_b = np.asarray(proj_b, dtype=d)
    g = np.asarray(norm_gamma, dtype=d)
    beta = np.asarray(norm_beta, dtype=d)
    Wq, Wk, Wv = qkv_w[0:C], qkv_w[C : 2 * C], qkv_w[2 * C : 3 * C]
    bq, bk, bv = qkv_b[0:C], qkv_b[C : 2 * C], qkv_b[2 * C : 3 * C]

    wqT = (Wq.T * g[:, None])          # [c_in, c_out], rows scaled by gamma
    wkT = (Wk.T * g[:, None])
    wvT = (Wv.T * g[:, None])
    u1 = bq + Wq @ beta
    u2 = Wq @ g
    h1 = wkT @ u1
    h2 = wkT @ u2
    dc1 = proj_w @ (bv + Wv @ beta) + proj_b
    pc2 = proj_w @ (Wv @ g)

    wpack = np.zeros((C, PACKW), dtype=f)
    wpack[:, 0:C] = wvT
    wpack[:, C] = h1
    wpack[:, C + 1] = h2
    wpack[:, 258 : 258 + C] = wqT
    wpack[:, 258 + C : 258 + 2 * C] = wkT
    wpack[:, 258 + 2 * C : 258 + 3 * C] = proj_w.T
    wpack[:, 258 + 3 * C] = dc1
    wpack[:, 259 + 3 * C] = pc2
    wpack = np.ascontiguousarray(wpack)

    in_maps = []
    xf = np.asarray(x, dtype=f).reshape(B, C, N)
    for core in range(8):
        b, h = divmod(core, 2)
        xs = xf[b]
        if h == 1:
            xs = np.concatenate([xs[:, NQ:], xs[:, :NQ]], axis=1)
        in_maps.append({"x": np.ascontiguousarray(xs), "wpack": wpack})
    return in_maps


def assemble_output(results):
    out = np.empty((B, C, N), dtype=np.float32)
    for core in range(8):
        b, h = divmod(core, 2)
        out[b][:, h * NQ : (h + 1) * NQ] = results[core]["out"]
    return out.reshape(B, C, HH, WW, DD)


_PROGRAM = None
_N_CALLS = 0
_RUNNER = None


def get_program():
    global _PROGRAM
    if _PROGRAM is None:
        _PROGRAM = build_program()
    return _PROGRAM


def _build_cached_runner(nc):
    """Persistent jitted executor (same execution path that
    run_bass_kernel_spmd takes under axon, via bass2jax/PJRT) so repeat
    kernel() calls skip the multi-minute neuronx-cc recompile."""
    import jax
    from jax.experimental.shard_map import shard_map
    from jax.sharding import Mesh, PartitionSpec
    from concourse import bass2jax

    bass2jax.install_neuronx_cc_hook()
    n_cores = 8
    partition_name = (nc.partition_id_tensor.name
                      if nc.partition_id_tensor else None)
    in_names, out_names, out_avals, zero_outs = [], [], [], []
    for alloc in nc.m.functions[0].allocations:
        if not isinstance(alloc, mybir.MemoryLocationSet):
            continue
        name = alloc.memorylocations[0].name
        if alloc.kind == "ExternalInput":
            if name != partition_name:
                in_names.append(name)
        elif alloc.kind == "ExternalOutput":
            out_names.append(name)
            shape = tuple(alloc.tensor_shape)
            dtype = mybir.dt.np(alloc.dtype)
            out_avals.append(jax.core.ShapedArray(shape, dtype))
            zero_outs.append(np.zeros(shape, dtype))
    n_params = len(in_names)
    all_in_names = list(in_names) + list(out_names)
    if partition_name is not None:
        all_in_names.append(partition_name)

    def _body(*args):
        operands = list(args)
        if partition_name is not None:
            operands.append(bass2jax.partition_id_tensor())
        outs = bass2jax._bass_exec_p.bind(
            *operands,
            out_avals=tuple(out_avals),
            in_names=tuple(all_in_names),
            out_names=tuple(out_names),
            lowering_input_output_aliases=(),
            sim_require_finite=True,
            sim_require_nnan=True,
            nc=nc,
        )
        return tuple(outs)

    devices = jax.devices()[:n_cores]
    mesh = Mesh(np.asarray(devices), ("core",))
    n_outs = len(out_names)
    fn = jax.jit(
        shard_map(_body, mesh=mesh,
                  in_specs=(PartitionSpec("core"),) * (n_params + n_outs),
                  out_specs=(PartitionSpec("core"),) * n_outs,
                  check_rep=False),
        keep_unused=True,
    )

    def run(in_maps):
        per_core = [[np.asarray(m[name]) for name in in_names]
                    for m in in_maps]
        concat_in = [
            np.concatenate([per_core[c][i] for c in range(n_cores)], axis=0)
            for i in range(n_params)
        ]
        concat_zeros = [
            np.zeros((n_cores * z.shape[0], *z.shape[1:]), z.dtype)
            for z in zero_outs
        ]
        out_arrs = fn(*concat_in, *concat_zeros)
        return [
            {name: np.asarray(out_arrs[i]).reshape(
                n_cores, *out_avals[i].shape)[c]
             for i, name in enumerate(out_names)}
            for c in range(n_cores)
        ]

    return run


def kernel(x, norm_gamma, norm_beta, qkv_w, qkv_b, proj_w, proj_b):
    global _N_CALLS, _RUNNER
    nc = get_program()
    in_maps = make_in_maps(x, norm_gamma, norm_beta, qkv_w, qkv_b,
                           proj_w, proj_b)
    _N_CALLS += 1
    if _N_CALLS == 1:
        res = run_bass_kernel_spmd(nc, in_maps, core_ids=list(range(8)))
        return assemble_output(res.results)
    if _RUNNER is None:
        _RUNNER = _build_cached_runner(nc)
    return assemble_output(_RUNNER(in_maps))


# revision 20
# speedup vs baseline: 1.1764x; 1.0188x over previous
"""AttnBlock3d (GroupNorm + single-head self-attention + proj + residual)
on 8 Trainium2 NeuronCores.

Sharding: 8 shards = (batch sample b in 0..3) x (query-half h in 0..1).
Every core runs the SAME program (SPMD): the host permutes each sample's
N=4096 spatial columns so that the core's 2048 query positions come
first. GroupNorm / K / V are permutation-invariant in the column order,
and attention output for a query column does not depend on the ordering
of key columns, so the math is unchanged.

Key algebra (all exact; lets every big GEMM start without waiting for
the GroupNorm statistics):
  xn = A*x + Bvec per channel, A = inv_std*gamma, Bvec = beta - mean*A.
  The gamma factor is folded into the weights on the HOST
  (W' = W diag(gamma)), so q = Wq@xn + bq = inv_std*(Wq'@x) + bq2.
  Softmax over k is invariant to anything constant along k, so only the
  [k]-indexed part of the score bias survives; it comes out of two
  extra output columns of the V^T GEMM (h1.x, h2.x) combined with the
  stats. inv^2*SCALE enters through the ACT Exp per-partition scale.
  The V-side affine (v = inv*v_raw + cvv) is folded THROUGH the proj:
  out = WpT@(ph * inv/denom) + [Wp@cvv + Wp@bv + bp + x], with the
  channel constant dcc = dc1 + (-mean*inv)*pc2 built from host vectors
  dc1 = Wp@(bv + Wv@beta) + bp and pc2 = Wp@Wv@gamma, pre-added into
  the residual tiles.

Schedule (q-major two-phase sweep):
  - prologue: 2 packed weight DMAs + 8 x-piece DMAs; GroupNorm moments
    via PE column-sum matmuls + ACT Squares (both idle then); Q(H0)/K0
    GEMMs and fp8 copies so the first exp fires as soon as the stats
    chain resolves.
  - phase A: for kt in 0..31: scoresT(kt, query-half H0) -> 1024-wide
    Exp. Leftover K chunks / V^T tiles / Q(H1) GEMMs ride the PE+DVE
    slack early in A; AV chain for qc0 runs kt>=16 (PSUM frees then);
  - phase B: same over H1; AV for qc1 (burst) + qc2 (paced) and the
    divide/proj/residual tails for qc0..2 all inside the sweep.
  - epilogue: only qc3's AV + tail.
  PSUM: scores 2x[128,1024] (4 banks) + AV chain ph0,ph1,pd (3) +
  proj po (1) = 8, with prologue pools (stats, qkv) scoped to close
  before the chain/proj pools open.
"""

import numpy as np
from contextlib import ExitStack

import bass_rust
import concourse.bass as bass
import concourse.tile as tile
from concourse import mybir
from concourse.bass_utils import run_bass_kernel_spmd

F32 = mybir.dt.float32
F32R = mybir.dt.float32r
BF16 = mybir.dt.bfloat16
FP8 = mybir.dt.float8e4
AX = mybir.AxisListType
OP = mybir.AluOpType
AF = mybir.ActivationFunctionType

B, C, HH, WW, DD = 4, 256, 16, 16, 16
N = HH * WW * DD          # 4096 spatial positions per sample
NQ = N // 2               # 2048 query positions per core
P = 128                   # partitions
NCT = C // P              # 2 channel tiles
NKT = N // P              # 32 key tiles
QCH = 512                 # q-chunk width (one PSUM bank of fp32)
NQC = NQ // QCH           # 4 q chunks
EPS = 1e-6
SCALE = float(C) ** -0.5  # 0.0625
PACKW = 258 + 3 * C + 2   # wv_ext | wq | wk | wp | dc1 | pc2


def _split_excess_waits(nc, cap=1):
    """walrus in this env rejects >1 sync wait per instruction; peel
    extras onto no-ops inserted before the offender on the same engine."""
    n = 0
    for f in nc.m.functions:
        for blk in f.blocks:
            insts = blk.instructions
            new_insts = []
            for inst in insts:
                si = inst.sync_info
                if si is not None and si.on_wait is not None and len(si.on_wait) > cap:
                    waits = list(si.on_wait)
                    extra, keep = waits[:-cap], waits[-cap:]
                    for j in range(0, len(extra), cap):
                        nop = mybir.InstNoOp(
                            name=f"{inst.name}_ws{j}", ins=[], outs=[]
                        )
                        nop.engine = inst.engine
                        nop.sync_info = bass_rust.SyncInfo(
                            on_wait=extra[j : j + cap], on_update=[]
                        )
                        new_insts.append(nop)
                    inst.sync_info = bass_rust.SyncInfo(
                        on_wait=keep, on_update=list(si.on_update)
                    )
                    n += 1
                new_insts.append(inst)
            if len(new_insts) != len(insts):
                insts[:] = new_insts
    return n


def _r(ap):
    return ap.bitcast(F32R)


def _f(ap):
    return ap.bitcast(F32)


def build_program():
    nc = bass.Bass("TRN2", target_bir_lowering=False, debug=False)

    x_d = nc.dram_tensor("x", [C, N], F32R, kind="ExternalInput")
    w_d = nc.dram_tensor("wpack", [C, PACKW], F32R, kind="ExternalInput")
    out_d = nc.dram_tensor("out", [C, NQ], F32, kind="ExternalOutput")

    with tile.TileContext(nc) as tc, ExitStack() as ctx:
        # ---------- persistent pools ----------
        consts = ctx.enter_context(tc.tile_pool(name="consts", bufs=1))
        qk_pool = ctx.enter_context(tc.tile_pool(name="qk", bufs=1))
        vt_pool = ctx.enter_context(tc.tile_pool(name="vt", bufs=NKT // 2))
        xb_pool = ctx.enter_context(tc.tile_pool(name="xb", bufs=1))
        x_pool = ctx.enter_context(tc.tile_pool(name="xio", bufs=1))
        exp_pool = ctx.enter_context(tc.tile_pool(name="expt", bufs=NKT // 2))

        # packed weights: 2 DMAs total (one per channel tile)
        wpk = [consts.tile([P, PACKW], F32R, tag=f"wpk{ci}", name=f"wpk{ci}")
               for ci in range(NCT)]
        for ci in range(NCT):
            nc.sync.dma_start(wpk[ci][:], w_d.ap()[ci * P : (ci + 1) * P, :])
        wv_ext = [wpk[ci][:, 0:258] for ci in range(NCT)]
        wqA = [wpk[ci][:, 258 : 258 + C] for ci in range(NCT)]
        wkA = [wpk[ci][:, 258 + C : 258 + 2 * C] for ci in range(NCT)]
        wpT = [wpk[ci][:, 258 + 2 * C : 258 + 3 * C] for ci in range(NCT)]
        dc1 = [_f(wpk[ci][:, 258 + 3 * C : 259 + 3 * C]) for ci in range(NCT)]
        pc2 = [_f(wpk[ci][:, 259 + 3 * C : 260 + 3 * C]) for ci in range(NCT)]

        # x in 8 pieces of [128, 1024]: both channel tiles of the query
        # half first so Q/V^T GEMMs and stats start at ~1/4 of the load.
        xA = [x_pool.tile([P, NQ], F32R, tag=f"xA{ci}", name=f"xA{ci}")
              for ci in range(NCT)]
        xB = [x_pool.tile([P, NQ], F32R, tag=f"xB{ci}", name=f"xB{ci}")
              for ci in range(NCT)]
        x_pieces = []  # (ci, tile, col_lo, width) in DMA order; the
        # final piece is small so the last GroupNorm square (which gates
        # the stats chain) finishes right after the last DMA byte.
        plan = [(0, 0, 0, 1024), (0, 1, 0, 1024),
                (0, 0, 1024, 1024), (0, 1, 1024, 1024),
                (1, 0, 0, 1024), (1, 1, 0, 1024),
                (1, 0, 1024, 1024), (1, 1, 1024, 768), (1, 1, 1792, 256)]
        for half, ci, lo, w in plan:
            xt = (xA, xB)[half][ci]
            nc.sync.dma_start(
                xt[:, lo : lo + w],
                x_d.ap()[ci * P : (ci + 1) * P,
                         half * NQ + lo : half * NQ + lo + w])
            x_pieces.append((ci, xt, lo, w))

        def xs(ci, lo, w):
            """f32r view of x columns [lo, lo+w) (must not straddle NQ)."""
            if lo < NQ:
                assert lo + w <= NQ
                return xA[ci][:, lo : lo + w]
            return xB[ci][:, lo - NQ : lo - NQ + w]

        ones_col = consts.tile([P, 1], F32, tag="ones_col", name="ones_col")
        nc.vector.memset(ones_col[:], 1.0)
        ones_fp8 = consts.tile([P, 32], FP8, tag="ones_fp8", name="ones_fp8")
        nc.vector.memset(ones_fp8[:], 1.0)
        ones_row = consts.tile([1, P], F32, tag="ones_row", name="ones_row")
        nc.vector.memset(ones_row[:], 1.0)

        q2 = qk_pool.tile([P, NCT * NQ], FP8, tag="q2", name="q2")
        k2 = qk_pool.tile([P, NCT * N], FP8, tag="k2", name="k2")
        vT = [vt_pool.tile([P, 2 * C], FP8, tag="vt", name="vt")
              for _ in range(NKT // 2)]
        xbd = [xb_pool.tile([P, NQ], F32, tag=f"xb{ci}", name=f"xb{ci}")
               for ci in range(NCT)]
        hx = qk_pool.tile([P, 2 * NKT], F32, tag="hx", name="hx")
        hx3 = hx[:].rearrange("p (c k) -> p c k", c=2)
        bias_k = qk_pool.tile([P, NKT], F32, tag="bias_k", name="bias_k")
        # stats-derived broadcast columns:
        # [inv, -mean*inv, S*inv, S*inv^2, -S*mean*inv^2]
        mi_bc = consts.tile([P, 5], F32, tag="mi_bc", name="mi_bc")
        inv_row = consts.tile([1, P], F32, tag="inv_row", name="inv_row")
        dcc = [consts.tile([P, 1], F32, tag=f"dcc{ci}", name=f"dcc{ci}")
               for ci in range(NCT)]

        p_pre = ExitStack()
        st_pool = p_pre.enter_context(tc.tile_pool(name="stats", bufs=1))
        ps_st = p_pre.enter_context(
            tc.tile_pool(name="ps_st", bufs=1, space="PSUM"))
        ps_qkp = p_pre.enter_context(
            tc.tile_pool(name="ps_qkp", bufs=2, space="PSUM"))

        # ---------- GroupNorm moments ----------
        # column sums on the PE (idle in the prologue): 16 accumulating
        # [1,512] matmuls in x-piece DMA order; sums of squares on the
        # ACT (also idle) with the free-dim accumulator. Stats for piece
        # i are interleaved with the prologue GEMMs so at most two
        # not-yet-satisfied stat matmuls sit in the PE wait queue.
        colsum = ps_st.tile([1, QCH], F32, tag="colsum", name="colsum")
        NPC = len(x_pieces)
        q4 = st_pool.tile([P, NPC], F32, tag="q4", name="q4")
        scr = st_pool.tile([P, NQ // 2], F32, tag="scr", name="scr")

        def emit_stats(i):
            ci, xt, lo, w = x_pieces[i]
            with tc.high_priority():
                off = 0
                while off < w:
                    cw = min(QCH, w - off)
                    nc.tensor.matmul(
                        colsum[0:1, 0:cw], _r(ones_col[:]),
                        xt[:, lo + off : lo + off + cw],
                        start=(i == 0 and off == 0),
                        stop=(i == NPC - 1 and off + cw == w),
                        skip_group_check=True)
                    off += cw
                nc.scalar.activation(scr[:, 0:w], _f(xt[:, lo : lo + w]),
                                     AF.Square, accum_out=q4[:, i : i + 1])

        # ---------- prologue GEMMs (emission order = arrival order) ----
        def emit_vt(kt):
            lo = kt * P
            pv = ps_qkp.tile([P, C + 2], F32, tag="ps_v", name="ps_v")
            for ci in range(NCT):
                nc.tensor.matmul(pv[:], xs(ci, lo, P), wv_ext[ci],
                                 start=(ci == 0), stop=(ci == NCT - 1))
            # stash the two bias columns so pv can be released without
            # waiting for the stats; bias_k is batch-built later.
            nc.vector.tensor_copy(hx3[:, :, kt : kt + 1], pv[:, C : C + 2])
            nc.vector.tensor_copy(
                vT[kt // 2][:, (kt % 2) * C : (kt % 2 + 1) * C], pv[:, 0:C])

        def emit_qk(which, oc, col):
            w = wqA if which == "q" else wkA
            dst, width = (q2, NQ) if which == "q" else (k2, N)
            pk = ps_qkp.tile([P, QCH], F32, tag="ps_qk", name="ps_qk")
            for ci in range(NCT):
                nc.tensor.matmul(pk[:],
                                 w[ci][:, oc * P : (oc + 1) * P],
                                 xs(ci, col, QCH),
                                 start=(ci == 0), stop=(ci == NCT - 1))
            nc.vector.tensor_copy(
                dst[:, oc * width + col : oc * width + col + QCH], pk[:])

        # stats p0,p1 | V^T 0..3 + Q(qc0) | stats p2,p3 | V^T 4..7 +
        # Q(qc1) + K0 | stats p4..7 (all GEMMs need only x pieces 0,1)
        emit_stats(0)
        emit_stats(1)
        for kt in range(4):
            emit_vt(kt)
        for oc in range(NCT):
            emit_qk("q", oc, 0)
        emit_stats(2)
        emit_stats(3)
        for kt in range(4, 8):
            emit_vt(kt)
        for oc in range(NCT):
            emit_qk("q", oc, QCH)
        for oc in range(NCT):
            emit_qk("k", oc, 0)
        for i in range(4, NPC):
            emit_stats(i)

        # ---------- stats chain ----------
        # Entirely on the ACT engine (idle in the prologue, and immune
        # to the DVE copy congestion): activation computes
        # func(in*scale + bias) with per-partition AP scale, which gives
        # scalar-scalar multiply via scale=AP. The two tensor-tensor
        # combines that ACT cannot do (dcc, bias_k) go to GPSIMD.
        with tc.high_priority():
            s_sum = st_pool.tile([1, 1], F32, tag="s_sum", name="s_sum")
            s_sq = st_pool.tile([1, 1], F32, tag="s_sq", name="s_sq")
            scr8 = st_pool.tile([1, NPC], F32, tag="scr8", name="scr8")
            scr512 = st_pool.tile([1, QCH], F32, tag="scr512", name="scr512")
            psq = ps_st.tile([1, NPC], F32, tag="psq", name="psq")
            nc.tensor.matmul(psq[:], ones_col[:], q4[:])
            nc.scalar.activation(scr8[:], psq[:], AF.Copy,
                                 accum_out=s_sq[:])
            nc.scalar.activation(scr512[:], colsum[:], AF.Copy,
                                 accum_out=s_sum[:])

            inv_cn = 1.0 / float(C * N)
            mean_sb = st_pool.tile([1, 1], F32, tag="mean", name="mean")
            nc.scalar.activation(mean_sb[:], s_sum[:], AF.Copy,
                                 scale=inv_cn)
            msq = st_pool.tile([1, 1], F32, tag="msq", name="msq")
            nc.scalar.activation(msq[:], mean_sb[:], AF.Square)
            epsm = st_pool.tile([1, 1], F32, tag="epsm", name="epsm")
            nc.scalar.activation(epsm[:], msq[:], AF.Copy, scale=-1.0,
                                 bias=EPS)
            lnv = st_pool.tile([1, 1], F32, tag="lnv", name="lnv")
            nc.scalar.activation(lnv[:], s_sq[:], AF.Ln, scale=inv_cn,
                                 bias=epsm[:])
            mi_sb = st_pool.tile([1, 5], F32, tag="mi", name="mi")
            inv_c = mi_sb[:, 0:1]
            nc.scalar.activation(inv_c, lnv[:], AF.Exp, scale=-0.5)
            ninv = st_pool.tile([1, 1], F32, tag="ninv", name="ninv")
            nc.scalar.activation(ninv[:], inv_c, AF.Copy, scale=-1.0)
            nc.scalar.activation(mi_sb[:, 1:2], mean_sb[:], AF.Copy,
                                 scale=ninv[:])                  # -mean*inv
            nc.scalar.activation(mi_sb[:, 2:3], inv_c, AF.Copy,
                                 scale=SCALE)                    # S*inv
            nc.scalar.activation(mi_sb[:, 3:4], inv_c, AF.Copy,
                                 scale=mi_sb[:, 2:3])            # S*inv^2
            nc.scalar.activation(mi_sb[:, 4:5], mi_sb[:, 1:2], AF.Copy,
                                 scale=mi_sb[:, 2:3])            # -S*m*inv^2
            ps_bc5 = ps_st.tile([P, 5], F32, tag="ps_bc5", name="ps_bc5")
            nc.tensor.matmul(ps_bc5[:], ones_row[:], mi_sb[:])
            nc.scalar.activation(mi_bc[:], ps_bc5[:], AF.Copy)
            nc.scalar.activation(inv_row[:], ones_row[:], AF.Copy,
                                 scale=mi_sb[:, 0:1])
            minv_neg = mi_bc[:, 1:2]
            si_bc = mi_bc[:, 2:3]
            si2_bc = mi_bc[:, 3:4]
            m2n_bc = mi_bc[:, 4:5]
            for ci in range(NCT):
                nc.gpsimd.tensor_scalar(dcc[ci][:], pc2[ci], minv_neg,
                                        dc1[ci], op0=OP.mult, op1=OP.add)
            # bias_k = S*inv*(h1.x) - S*mean*inv^2*(h2.x), batch 0..15
            # on the DVE (stt exists there; the first exps gate on it),
            # later batches on the idle GPSIMD (no stt -> 3 ops).
            bt = qk_pool.tile([P, 2 * NKT], F32, tag="bt", name="bt")

            def emit_bias(lo, hi):
                nc.gpsimd.tensor_scalar(bt[:, lo:hi], hx3[:, 0, lo:hi],
                                        si_bc, None, op0=OP.mult)
                nc.gpsimd.tensor_scalar(bt[:, 32 + lo : 32 + hi],
                                        hx3[:, 1, lo:hi], m2n_bc, None,
                                        op0=OP.mult)
                nc.gpsimd.tensor_tensor(bias_k[:, lo:hi], bt[:, lo:hi],
                                        bt[:, 32 + lo : 32 + hi], OP.add)

            nc.vector.tensor_scalar(bt[:, 0:16], hx3[:, 0, 0:16],
                                    si_bc, None, op0=OP.mult)
            nc.vector.scalar_tensor_tensor(bias_k[:, 0:16], hx3[:, 1, 0:16],
                                           m2n_bc, bt[:, 0:16],
                                           op0=OP.mult, op1=OP.add)

        p_pre.close()

        # ---------- sweep pools ----------
        p_sw = ExitStack()
        ps_s = p_sw.enter_context(
            tc.tile_pool(name="ps_s", bufs=2, space="PSUM"))
        p_qkv2 = ExitStack()
        ps_qk2 = p_qkv2.enter_context(
            tc.tile_pool(name="ps_qk2", bufs=2, space="PSUM"))

        def emit_vt2(kt):
            lo = kt * P
            pv = ps_qk2.tile([P, C + 2], F32, tag="ps_v2", name="ps_v2")
            for ci in range(NCT):
                nc.tensor.matmul(pv[:], xs(ci, lo, P), wv_ext[ci],
                                 start=(ci == 0), stop=(ci == NCT - 1))
            nc.vector.tensor_copy(hx3[:, :, kt : kt + 1], pv[:, C : C + 2])
            nc.vector.tensor_copy(
                vT[kt // 2][:, (kt % 2) * C : (kt % 2 + 1) * C], pv[:, 0:C])

        def emit_qk2(which, oc, col):
            w = wqA if which == "q" else wkA
            dst, width = (q2, NQ) if which == "q" else (k2, N)
            pk = ps_qk2.tile([P, QCH], F32, tag="ps_qk2t", name="ps_qk2t")
            for ci in range(NCT):
                nc.tensor.matmul(pk[:],
                                 w[ci][:, oc * P : (oc + 1) * P],
                                 xs(ci, col, QCH),
                                 start=(ci == 0), stop=(ci == NCT - 1))
            nc.vector.tensor_copy(
                dst[:, oc * width + col : oc * width + col + QCH], pk[:])

        k3all = k2[:].rearrange("p (j n) -> p j n", j=2)
        q3 = q2[:].rearrange("p (j n) -> p j n", j=2)

        def emit_scores_exp(kt, half):
            """scoresT + 1024-wide exp for (key tile kt, query half).
            High priority: the exp stream is the whole-kernel critical
            path, so its scores matmuls must win PE arbitration over AV
            bursts whenever both are ready."""
            if kt % 2 == 0 and half == 0:
                exp_tiles[kt // 2] = exp_pool.tile(
                    [P, 2 * NQ], FP8, tag="expt", name="expt")
            with tc.high_priority():
                ps = ps_s.tile([P, 2 * QCH], F32, tag="s", name="s")
                k3 = k3all[:, :, kt * P : (kt + 1) * P]
                for qh in range(2):
                    qcol = half * 2 * QCH + qh * QCH
                    nc.tensor.matmul(
                        ps[:, qh * QCH : (qh + 1) * QCH],
                        k3, q3[:, :, qcol : qcol + QCH],
                        skip_group_check=True,
                        perf_mode=mybir.MatmulPerfMode.DoubleRow)
                lo = (kt % 2) * NQ + half * 2 * QCH
                nc.scalar.activation(
                    exp_tiles[kt // 2][:, lo : lo + 2 * QCH],
                    ps[:], AF.Exp, scale=si2_bc, bias=bias_k[:, kt : kt + 1])

        exp_tiles = [None] * (NKT // 2)
        ones3 = ones_fp8[:].rearrange("p (j o) -> p j o", j=2)[:, :, 0:1]

        # AV chain state (one chain at a time; 3 PSUM banks)
        p_ch = ExitStack()
        ch_h = None  # opened lazily at phase-A kt16

        def av_step(qc, p, ph, pd, first, last):
            et3 = exp_tiles[p].rearrange(
                "p (j q) -> p j q", j=2)[:, :, qc * QCH : (qc + 1) * QCH]
            vt3 = vT[p].rearrange("p (j c) -> p j c", j=2)
            for ct in range(NCT):
                nc.tensor.matmul(
                    ph[ct][:], vt3[:, :, ct * P : (ct + 1) * P], et3[:],
                    start=first, stop=last, skip_group_check=True,
                    perf_mode=mybir.MatmulPerfMode.DoubleRow)
            nc.tensor.matmul(
                pd[0:1, :], ones3, et3[:],
                start=first, stop=last, skip_group_check=True,
                perf_mode=mybir.MatmulPerfMode.DoubleRow)

        p_tail = ExitStack()

        with tc.tile_pool(name="att_sb", bufs=2) as att_pool, \
             tc.tile_pool(name="out_sb", bufs=4) as out_pool:

            tail_state = {}

            def tail_stage1(qc, ph, pd, cp_act=False):
                """recip + inv-scaled broadcast (DVE + PE). The
                broadcast lands back in the (now-free) denominator bank,
                so no extra PSUM bank is needed. In the epilogue the
                PSUM->SBUF copy goes to the idle ACT engine instead of
                the DVE, which is the epilogue's critical engine."""
                rec = att_pool.tile([1, QCH], F32, tag="rec", name="rec")
                with nc.allow_low_precision(reason="f32r fp32-width"):
                    nc.vector.reciprocal(_r(rec[:]), pd[0:1, :])
                nc.tensor.matmul(pd[:], _r(inv_row[:]), _r(rec[:]),
                                 skip_group_check=True)
                rec_bc = att_pool.tile([P, QCH], F32, tag="rec_bc",
                                       name="rec_bc")
                if cp_act:
                    nc.scalar.activation(rec_bc[:], pd[:], AF.Copy)
                else:
                    nc.vector.tensor_copy(rec_bc[:], pd[:])
                tail_state[qc] = (ph, rec_bc)

            def tail_stage2(qc):
                """h = ph * (inv/denom) into SBUF; releases the chain."""
                ph, rec_bc = tail_state[qc]
                h_sb = []
                for ct in range(NCT):
                    h = att_pool.tile([P, QCH], F32, tag=f"hsb{ct}",
                                      name=f"hsb{ct}")
                    nc.vector.tensor_tensor(_r(h[:]), ph[ct][:], rec_bc[:],
                                            OP.mult)
                    h_sb.append(h)
                tail_state[qc] = h_sb

            def tail_stage3(qc, oc, po=None):
                """proj GEMM + residual add + store for one oc. The
                epilogue passes explicit idle-bank APs for po so the
                proj GEMMs don't serialize on the single chain po bank."""
                h_sb = tail_state[qc]
                qsl = slice(qc * QCH, (qc + 1) * QCH)
                if po is None:
                    po = ch_h.tile([P, QCH], F32, tag="po", name="po")
                for ci in range(NCT):
                    nc.tensor.matmul(
                        po[:], wpT[ci][:, oc * P : (oc + 1) * P],
                        _r(h_sb[ci][:]),
                        start=(ci == 0), stop=(ci == NCT - 1),
                        skip_group_check=True)
                ot = out_pool.tile([P, QCH], F32, tag="ot", name="ot")
                nc.vector.tensor_tensor(ot[:], po[:], xbd[oc][:, qsl],
                                        OP.add)
                nc.sync.dma_start(out_d.ap()[oc * P : (oc + 1) * P, qsl],
                                  ot[:])

            # ================= phase A (query half 0) =================
            NP2 = NKT // 2
            ph_cur = pd_cur = None
            av_done = 0  # p index consumed for current chain
            for kt in range(NKT):
                # leftover GEMM injections: V^T 8..31 at kt 0..11 (2/kt),
                # K chunks 1..7 at kt 0..6; Q half-1 at kt 2..5; bias
                # batches (GPSIMD) once their hx columns have landed.
                if kt < 12:
                    emit_vt2(8 + 2 * kt)
                    emit_vt2(9 + 2 * kt)
                if kt < 7:
                    for oc in range(NCT):
                        emit_qk2("k", oc, (kt + 1) * QCH)
                if 2 <= kt < 6:
                    j = kt - 2
                    emit_qk2("q", j % 2, 2 * QCH + (j // 2) * QCH)
                if kt == 11:
                    emit_bias(16, 24)
                if kt == 14:
                    emit_bias(24, 32)
                if kt in (18, 20):
                    # residual tiles on the (idle) GPSIMD engine, emitted
                    # here so the scheduler keeps them off the bias path
                    ci = kt // 2 - 9
                    nc.gpsimd.tensor_scalar(xbd[ci][:], _f(xA[ci][:]),
                                            dcc[ci][:], None, op0=OP.add)
                if kt == 16:
                    p_qkv2.close()

                emit_scores_exp(kt, 0)

                if kt >= 17:
                    if kt == 17:
                        ch_h = p_ch.enter_context(
                            tc.tile_pool(name="ps_ch", bufs=1, space="PSUM"))
                        ph_cur = [ch_h.tile([P, QCH], F32, tag=f"h{ct}",
                                            name=f"h{ct}")
                                  for ct in range(NCT)]
                        pd_cur = ch_h.tile([P, QCH], F32, tag="d", name="d")
                    # consume p with exp done (2p+1 <= kt), max 2/step
                    target = min((kt - 1) // 2 + 1, NP2)
                    budget = 2
                    while av_done < target and budget > 0:
                        av_step(0, av_done, ph_cur, pd_cur,
                                av_done == 0, av_done == NP2 - 1)
                        av_done += 1
                        budget -= 1
            # finish qc0 chain (p15 needs the last A exp)
            while av_done < NP2:
                av_step(0, av_done, ph_cur, pd_cur,
                        av_done == 0, av_done == NP2 - 1)
                av_done += 1

            # ================= phase B (query half 1) =================
            tail_stage1(0, ph_cur, pd_cur)
            qc_av = 1        # chain currently running
            av_done = 0
            for kt in range(NKT):
                emit_scores_exp(kt, 1)
                if kt == 0:
                    tail_stage2(0)   # frees the qc0 chain PSUM
                if kt == 1:
                    tail_stage3(0, 0)
                if kt == 2:
                    tail_stage3(0, 1)
                # AV for qc1 (burst; all H0..no, all its exps exist) then
                # qc2 (paced behind the B exp sweep)
                if kt >= 1 and qc_av <= 2:
                    if qc_av == 1:
                        target = NP2
                        budget = 3
                    else:
                        target = min((kt - 1) // 2 + 1, NP2)
                        budget = 4
                    while av_done < target and budget > 0:
                        av_step(qc_av, av_done, ph_cur, pd_cur,
                                av_done == 0, av_done == NP2 - 1)
                        av_done += 1
                        budget -= 1
                    if av_done == NP2:
                        tail_stage1(qc_av, ph_cur, pd_cur)
                        tail_stage2(qc_av)
                        if qc_av == 1:
                            qc_av = 2
                            av_done = 0
                        else:
                            qc_av = 3
                if kt == 10:
                    tail_stage3(1, 0)
                if kt == 11:
                    tail_stage3(1, 1)
            # ================= epilogue: qc2 tail + qc3 ===============
            # qc3's denominator accumulates FIRST (the d bank frees as
            # soon as qc2's rec_bc is copied out), so its reciprocal +
            # broadcast chain overlaps the qc3 ph matmuls; epilogue proj
            # matmuls borrow idle scores banks to avoid po-bank churn.
            if qc_av == 2:
                while av_done < NP2:
                    av_step(2, av_done, ph_cur, pd_cur,
                            av_done == 0, av_done == NP2 - 1)
                    av_done += 1
                tail_stage1(2, ph_cur, pd_cur, cp_act=True)
                tail_stage2(2)

            # qc3's accumulators live in the now-idle scores banks so
            # its AV does not wait for qc2's tail to release the chain;
            # its denominator accumulates first so the reciprocal +
            # broadcast chain overlaps the ph matmuls.
            ph3 = ps_s.tile([P, 2 * QCH], F32, tag="s", name="ph3")
            po2 = ps_s.tile([P, 2 * QCH], F32, tag="s", name="po2")

            def av3_pd(p, first, last):
                et3 = exp_tiles[p].rearrange(
                    "p (j q) -> p j q", j=2)[:, :, 3 * QCH : 4 * QCH]
                nc.tensor.matmul(
                    pd_cur[0:1, :], ones3, et3[:],
                    start=first, stop=last, skip_group_check=True,
                    perf_mode=mybir.MatmulPerfMode.DoubleRow)

            def av3_ph(p, first, last):
                et3 = exp_tiles[p].rearrange(
                    "p (j q) -> p j q", j=2)[:, :, 3 * QCH : 4 * QCH]
                vt3 = vT[p].rearrange("p (j c) -> p j c", j=2)
                for ct in range(NCT):
                    nc.tensor.matmul(
                        ph3[:, ct * QCH : (ct + 1) * QCH],
                        vt3[:, :, ct * P : (ct + 1) * P],
                        et3[:], start=first, stop=last,
                        skip_group_check=True,
                        perf_mode=mybir.MatmulPerfMode.DoubleRow)

            for p in range(NP2):
                av3_pd(p, p == 0, p == NP2 - 1)
            tail_stage1(3, None, pd_cur, cp_act=True)
            for p in range(NP2):
                av3_ph(p, p == 0, p == NP2 - 1)
            tail_stage3(2, 0, po=po2[:, 0:QCH])
            tail_stage3(2, 1, po=po2[:, QCH : 2 * QCH])
            tail_state[3] = ([ph3[:, 0:QCH], ph3[:, QCH : 2 * QCH]],
                             tail_state[3][1])
            tail_stage2(3)
            tail_stage3(3, 0)
            tail_stage3(3, 1, po=pd_cur[:])
            p_tail.close()
            p_ch.close()
        p_sw.close()

    _split_excess_waits(nc)
    return nc


def make_in_maps(x, norm_gamma, norm_beta, qkv_w, qkv_b, proj_w, proj_b):
    f = np.float32
    d = np.float64
    qkv_w = np.asarray(qkv_w, dtype=d)
    qkv_b = np.asarray(qkv_b, dtype=d)
    proj_w = np.asarray(proj_w, dtype=d)
    proj_b = np.asarray(proj_b, dtype=d)
    g = np.asarray(norm_gamma, dtype=d)
    beta = np.asarray(norm_beta, dtype=d)
    Wq, Wk, Wv = qkv_w[0:C], qkv_w[C : 2 * C], qkv_w[2 * C : 3 * C]
    bq, bk, bv = qkv_b[0:C], qkv_b[C : 2 * C], qkv_b[2 * C : 3 * C]

    wqT = (Wq.T * g[:, None])          # [c_in, c_out], rows scaled by gamma
    wkT = (Wk.T * g[:, None])
    wvT = (Wv.T * g[:, None])
    u1 = bq + Wq @ beta
    u2 = Wq @ g
    h1 = wkT @ u1
    h2 = wkT @ u2
    dc1 = proj_w @ (bv + Wv @ beta) + proj_b
    pc2 = proj_w @ (Wv @ g)

    wpack = np.zeros((C, PACKW), dtype=f)
    wpack[:, 0:C] = wvT
    wpack[:, C] = h1
    wpack[:, C + 1] = h2
    wpack[:, 258 : 258 + C] = wqT
    wpack[:, 258 + C : 258 + 2 * C] = wkT
    wpack[:, 258 + 2 * C : 258 + 3 * C] = proj_w.T
    wpack[:, 258 + 3 * C] = dc1
    wpack[:, 259 + 3 * C] = pc2
    wpack = np.ascontiguousarray(wpack)

    in_maps = []
    xf = np.asarray(x, dtype=f).reshape(B, C, N)
    for core in range(8):
        b, h = divmod(core, 2)
        xs = xf[b]
        if h == 1:
            xs = np.concatenate([xs[:, NQ:], xs[:, :NQ]], axis=1)
        in_maps.append({"x": np.ascontiguousarray(xs), "wpack": wpack})
    return in_maps


def assemble_output(results):
    out = np.empty((B, C, N), dtype=np.float32)
    for core in range(8):
        b, h = divmod(core, 2)
        out[b][:, h * NQ : (h + 1) * NQ] = results[core]["out"]
    return out.reshape(B, C, HH, WW, DD)


_PROGRAM = None
_N_CALLS = 0
_RUNNER = None


def get_program():
    global _PROGRAM
    if _PROGRAM is None:
        _PROGRAM = build_program()
    return _PROGRAM


def _build_cached_runner(nc):
    """Persistent jitted executor (same execution path that
    run_bass_kernel_spmd takes under axon, via bass2jax/PJRT) so repeat
    kernel() calls skip the multi-minute neuronx-cc recompile."""
    import jax
    from jax.experimental.shard_map import shard_map
    from jax.sharding import Mesh, PartitionSpec
    from concourse import bass2jax

    bass2jax.install_neuronx_cc_hook()
    n_cores = 8
    partition_name = (nc.partition_id_tensor.name
                      if nc.partition_id_tensor else None)
    in_names, out_names, out_avals, zero_outs = [], [], [], []
    for alloc in nc.m.functions[0].allocations:
        if not isinstance(alloc, mybir.MemoryLocationSet):
            continue
        name = alloc.memorylocations[0].name
        if alloc.kind == "ExternalInput":
            if name != partition_name:
                in_names.append(name)
        elif alloc.kind == "ExternalOutput":
            out_names.append(name)
            shape = tuple(alloc.tensor_shape)
            dtype = mybir.dt.np(alloc.dtype)
            out_avals.append(jax.core.ShapedArray(shape, dtype))
            zero_outs.append(np.zeros(shape, dtype))
    n_params = len(in_names)
    all_in_names = list(in_names) + list(out_names)
    if partition_name is not None:
        all_in_names.append(partition_name)

    def _body(*args):
        operands = list(args)
        if partition_name is not None:
            operands.append(bass2jax.partition_id_tensor())
        outs = bass2jax._bass_exec_p.bind(
            *operands,
            out_avals=tuple(out_avals),
            in_names=tuple(all_in_names),
            out_names=tuple(out_names),
            lowering_input_output_aliases=(),
            sim_require_finite=True,
            sim_require_nnan=True,
            nc=nc,
        )
        return tuple(outs)

    devices = jax.devices()[:n_cores]
    mesh = Mesh(np.asarray(devices), ("core",))
    n_outs = len(out_names)
    fn = jax.jit(
        shard_map(_body, mesh=mesh,
                  in_specs=(PartitionSpec("core"),) * (n_params + n_outs),
                  out_specs=(PartitionSpec("core"),) * n_outs,
                  check_rep=False),
        keep_unused=True,
    )

    def run(in_maps):
        per_core = [[np.asarray(m[name]) for name in in_names]
                    for m in in_maps]
        concat_in = [
            np.concatenate([per_core[c][i] for c in range(n_cores)], axis=0)
            for i in range(n_params)
        ]
        concat_zeros = [
            np.zeros((n_cores * z.shape[0], *z.shape[1:]), z.dtype)
            for z in zero_outs
        ]
        out_arrs = fn(*concat_in, *concat_zeros)
        return [
            {name: np.asarray(out_arrs[i]).reshape(
                n_cores, *out_avals[i].shape)[c]
             for i, name in enumerate(out_names)}
            for c in range(n_cores)
        ]

    return run


def kernel(x, norm_gamma, norm_beta, qkv_w, qkv_b, proj_w, proj_b):
    global _N_CALLS, _RUNNER
    nc = get_program()
    in_maps = make_in_maps(x, norm_gamma, norm_beta, qkv_w, qkv_b,
                           proj_w, proj_b)
    _N_CALLS += 1
    if _N_CALLS == 1:
        res = run_bass_kernel_spmd(nc, in_maps, core_ids=list(range(8)))
        return assemble_output(res.results)
    if _RUNNER is None:
        _RUNNER = _build_cached_runner(nc)
    return assemble_output(_RUNNER(in_maps))


# revision 21
# speedup vs baseline: 1.1979x; 1.0182x over previous
"""AttnBlock3d (GroupNorm + single-head self-attention + proj + residual)
on 8 Trainium2 NeuronCores.

Sharding: 8 shards = (batch sample b in 0..3) x (query-half h in 0..1).
Every core runs the SAME program (SPMD): the host permutes each sample's
N=4096 spatial columns so that the core's 2048 query positions come
first. GroupNorm / K / V are permutation-invariant in the column order,
and attention output for a query column does not depend on the ordering
of key columns, so the math is unchanged.

Key algebra (all exact; lets every big GEMM start without waiting for
the GroupNorm statistics):
  xn = A*x + Bvec per channel, A = inv_std*gamma, Bvec = beta - mean*A.
  The gamma factor is folded into the weights on the HOST
  (W' = W diag(gamma)), so q = Wq@xn + bq = inv_std*(Wq'@x) + bq2.
  Softmax over k is invariant to anything constant along k, so only the
  [k]-indexed part of the score bias survives; it comes out of two
  extra output columns of the V^T GEMM (h1.x, h2.x) combined with the
  stats. inv^2*SCALE enters through the ACT Exp per-partition scale.
  The V-side affine (v = inv*v_raw + cvv) is folded THROUGH the proj:
  out = WpT@(ph * inv/denom) + [Wp@cvv + Wp@bv + bp + x], with the
  channel constant dcc = dc1 + (-mean*inv)*pc2 built from host vectors
  dc1 = Wp@(bv + Wv@beta) + bp and pc2 = Wp@Wv@gamma, pre-added into
  the residual tiles.

Schedule (q-major two-phase sweep):
  - prologue: 2 packed weight DMAs + 8 x-piece DMAs; GroupNorm moments
    via PE column-sum matmuls + ACT Squares (both idle then); Q(H0)/K0
    GEMMs and fp8 copies so the first exp fires as soon as the stats
    chain resolves.
  - phase A: for kt in 0..31: scoresT(kt, query-half H0) -> 1024-wide
    Exp. Leftover K chunks / V^T tiles / Q(H1) GEMMs ride the PE+DVE
    slack early in A; AV chain for qc0 runs kt>=16 (PSUM frees then);
  - phase B: same over H1; AV for qc1 (burst) + qc2 (paced) and the
    divide/proj/residual tails for qc0..2 all inside the sweep.
  - epilogue: only qc3's AV + tail.
  PSUM: scores 2x[128,1024] (4 banks) + AV chain ph0,ph1,pd (3) +
  proj po (1) = 8, with prologue pools (stats, qkv) scoped to close
  before the chain/proj pools open.
"""

import numpy as np
from contextlib import ExitStack

import bass_rust
import concourse.bass as bass
import concourse.tile as tile
from concourse import mybir
from concourse.bass_utils import run_bass_kernel_spmd

F32 = mybir.dt.float32
F32R = mybir.dt.float32r
BF16 = mybir.dt.bfloat16
FP8 = mybir.dt.float8e4
AX = mybir.AxisListType
OP = mybir.AluOpType
AF = mybir.ActivationFunctionType

B, C, HH, WW, DD = 4, 256, 16, 16, 16
N = HH * WW * DD          # 4096 spatial positions per sample
NQ = N // 2               # 2048 query positions per core
P = 128                   # partitions
NCT = C // P              # 2 channel tiles
NKT = N // P              # 32 key tiles
QCH = 512                 # q-chunk width (one PSUM bank of fp32)
NQC = NQ // QCH           # 4 q chunks
EPS = 1e-6
SCALE = float(C) ** -0.5  # 0.0625
PACKW = 258 + 3 * C + 2   # wv_ext | wq | wk | wp | dc1 | pc2


def _split_excess_waits(nc, cap=1):
    """walrus in this env rejects >1 sync wait per instruction; peel
    extras onto no-ops inserted before the offender on the same engine."""
    n = 0
    for f in nc.m.functions:
        for blk in f.blocks:
            insts = blk.instructions
            new_insts = []
            for inst in insts:
                si = inst.sync_info
                if si is not None and si.on_wait is not None and len(si.on_wait) > cap:
                    waits = list(si.on_wait)
                    extra, keep = waits[:-cap], waits[-cap:]
                    for j in range(0, len(extra), cap):
                        nop = mybir.InstNoOp(
                            name=f"{inst.name}_ws{j}", ins=[], outs=[]
                        )
                        nop.engine = inst.engine
                        nop.sync_info = bass_rust.SyncInfo(
                            on_wait=extra[j : j + cap], on_update=[]
                        )
                        new_insts.append(nop)
                    inst.sync_info = bass_rust.SyncInfo(
                        on_wait=keep, on_update=list(si.on_update)
                    )
                    n += 1
                new_insts.append(inst)
            if len(new_insts) != len(insts):
                insts[:] = new_insts
    return n


def _r(ap):
    return ap.bitcast(F32R)


def _f(ap):
    return ap.bitcast(F32)


def build_program():
    nc = bass.Bass("TRN2", target_bir_lowering=False, debug=False)

    x_d = nc.dram_tensor("x", [C, N], F32R, kind="ExternalInput")
    w_d = nc.dram_tensor("wpack", [C, PACKW], F32R, kind="ExternalInput")
    out_d = nc.dram_tensor("out", [C, NQ], F32, kind="ExternalOutput")

    with tile.TileContext(nc) as tc, ExitStack() as ctx:
        # ---------- persistent pools ----------
        consts = ctx.enter_context(tc.tile_pool(name="consts", bufs=1))
        qk_pool = ctx.enter_context(tc.tile_pool(name="qk", bufs=1))
        vt_pool = ctx.enter_context(tc.tile_pool(name="vt", bufs=NKT // 2))
        xb_pool = ctx.enter_context(tc.tile_pool(name="xb", bufs=1))
        x_pool = ctx.enter_context(tc.tile_pool(name="xio", bufs=1))
        exp_pool = ctx.enter_context(tc.tile_pool(name="expt", bufs=NKT // 2))

        # packed weights: 2 DMAs total (one per channel tile)
        wpk = [consts.tile([P, PACKW], F32R, tag=f"wpk{ci}", name=f"wpk{ci}")
               for ci in range(NCT)]
        for ci in range(NCT):
            nc.sync.dma_start(wpk[ci][:], w_d.ap()[ci * P : (ci + 1) * P, :])
        wv_ext = [wpk[ci][:, 0:258] for ci in range(NCT)]
        wqA = [wpk[ci][:, 258 : 258 + C] for ci in range(NCT)]
        wkA = [wpk[ci][:, 258 + C : 258 + 2 * C] for ci in range(NCT)]
        wpT = [wpk[ci][:, 258 + 2 * C : 258 + 3 * C] for ci in range(NCT)]
        dc1 = [_f(wpk[ci][:, 258 + 3 * C : 259 + 3 * C]) for ci in range(NCT)]
        pc2 = [_f(wpk[ci][:, 259 + 3 * C : 260 + 3 * C]) for ci in range(NCT)]

        # x in 8 pieces of [128, 1024]: both channel tiles of the query
        # half first so Q/V^T GEMMs and stats start at ~1/4 of the load.
        xA = [x_pool.tile([P, NQ], F32R, tag=f"xA{ci}", name=f"xA{ci}")
              for ci in range(NCT)]
        xB = [x_pool.tile([P, NQ], F32R, tag=f"xB{ci}", name=f"xB{ci}")
              for ci in range(NCT)]
        x_pieces = []  # (ci, tile, col_lo, width) in DMA order; the
        # final piece is small so the last GroupNorm square (which gates
        # the stats chain) finishes right after the last DMA byte.
        plan = [(0, 0, 0, 1024), (0, 1, 0, 1024),
                (0, 0, 1024, 1024), (0, 1, 1024, 1024),
                (1, 0, 0, 1024), (1, 1, 0, 1024),
                (1, 0, 1024, 1024), (1, 1, 1024, 768), (1, 1, 1792, 256)]
        for half, ci, lo, w in plan:
            xt = (xA, xB)[half][ci]
            nc.sync.dma_start(
                xt[:, lo : lo + w],
                x_d.ap()[ci * P : (ci + 1) * P,
                         half * NQ + lo : half * NQ + lo + w])
            x_pieces.append((ci, xt, lo, w))

        def xs(ci, lo, w):
            """f32r view of x columns [lo, lo+w) (must not straddle NQ)."""
            if lo < NQ:
                assert lo + w <= NQ
                return xA[ci][:, lo : lo + w]
            return xB[ci][:, lo - NQ : lo - NQ + w]

        ones_col = consts.tile([P, 1], F32R, tag="ones_col",
                               name="ones_col")
        nc.vector.memset(ones_col[:], 1.0)
        ones_fp8 = consts.tile([P, 32], FP8, tag="ones_fp8", name="ones_fp8")
        nc.vector.memset(ones_fp8[:], 1.0)
        ones_row = consts.tile([1, P], F32, tag="ones_row", name="ones_row")
        nc.vector.memset(ones_row[:], 1.0)

        q2 = qk_pool.tile([P, NCT * NQ], FP8, tag="q2", name="q2")
        k2 = qk_pool.tile([P, NCT * N], FP8, tag="k2", name="k2")
        vT = [vt_pool.tile([P, 2 * C], FP8, tag="vt", name="vt")
              for _ in range(NKT // 2)]
        xbd = [xb_pool.tile([P, NQ], F32, tag=f"xb{ci}", name=f"xb{ci}")
               for ci in range(NCT)]
        hx = qk_pool.tile([P, 2 * NKT], F32, tag="hx", name="hx")
        hx3 = hx[:].rearrange("p (c k) -> p c k", c=2)
        bias_k = qk_pool.tile([P, NKT], F32, tag="bias_k", name="bias_k")
        # stats-derived broadcast columns:
        # [inv, -mean*inv, S*inv, S*inv^2, -S*mean*inv^2]
        mi_bc = consts.tile([P, 5], F32, tag="mi_bc", name="mi_bc")
        inv_row = consts.tile([1, P], F32, tag="inv_row", name="inv_row")
        dcc = [consts.tile([P, 1], F32, tag=f"dcc{ci}", name=f"dcc{ci}")
               for ci in range(NCT)]

        p_pre = ExitStack()
        st_pool = p_pre.enter_context(tc.tile_pool(name="stats", bufs=1))
        ps_st = p_pre.enter_context(
            tc.tile_pool(name="ps_st", bufs=1, space="PSUM"))
        ps_qkp = p_pre.enter_context(
            tc.tile_pool(name="ps_qkp", bufs=2, space="PSUM"))

        # ---------- GroupNorm moments ----------
        # column sums on the PE (idle in the prologue): 16 accumulating
        # [1,512] matmuls in x-piece DMA order; sums of squares on the
        # ACT (also idle) with the free-dim accumulator. Stats for piece
        # i are interleaved with the prologue GEMMs so at most two
        # not-yet-satisfied stat matmuls sit in the PE wait queue.
        colsum = ps_st.tile([1, QCH], F32, tag="colsum", name="colsum")
        NPC = len(x_pieces)
        q4 = st_pool.tile([P, NPC], F32, tag="q4", name="q4")
        scr = st_pool.tile([P, NQ // 2], F32, tag="scr", name="scr")

        def emit_stats(i):
            ci, xt, lo, w = x_pieces[i]
            with tc.high_priority():
                off = 0
                while off < w:
                    cw = min(QCH, w - off)
                    nc.tensor.matmul(
                        colsum[0:1, 0:cw], ones_col[:],
                        xt[:, lo + off : lo + off + cw],
                        start=(i == 0 and off == 0),
                        stop=(i == NPC - 1 and off + cw == w),
                        skip_group_check=True)
                    off += cw
                nc.scalar.activation(scr[:, 0:w], _f(xt[:, lo : lo + w]),
                                     AF.Square, accum_out=q4[:, i : i + 1])

        # ---------- prologue GEMMs (emission order = arrival order) ----
        def emit_vt(kt):
            lo = kt * P
            pv = ps_qkp.tile([P, C + 2], F32, tag="ps_v", name="ps_v")
            for ci in range(NCT):
                nc.tensor.matmul(pv[:], xs(ci, lo, P), wv_ext[ci],
                                 start=(ci == 0), stop=(ci == NCT - 1))
            # stash the two bias columns so pv can be released without
            # waiting for the stats; bias_k is batch-built later.
            nc.vector.tensor_copy(hx3[:, :, kt : kt + 1], pv[:, C : C + 2])
            nc.vector.tensor_copy(
                vT[kt // 2][:, (kt % 2) * C : (kt % 2 + 1) * C], pv[:, 0:C])

        def emit_qk(which, oc, col):
            w = wqA if which == "q" else wkA
            dst, width = (q2, NQ) if which == "q" else (k2, N)
            pk = ps_qkp.tile([P, QCH], F32, tag="ps_qk", name="ps_qk")
            for ci in range(NCT):
                nc.tensor.matmul(pk[:],
                                 w[ci][:, oc * P : (oc + 1) * P],
                                 xs(ci, col, QCH),
                                 start=(ci == 0), stop=(ci == NCT - 1))
            nc.vector.tensor_copy(
                dst[:, oc * width + col : oc * width + col + QCH], pk[:])

        # stats p0,p1 | V^T 0..3 + Q(qc0) | stats p2,p3 | V^T 4..7 +
        # Q(qc1) + K0 | stats p4..7 (all GEMMs need only x pieces 0,1)
        emit_stats(0)
        emit_stats(1)
        for kt in range(4):
            emit_vt(kt)
        for oc in range(NCT):
            emit_qk("q", oc, 0)
        emit_stats(2)
        emit_stats(3)
        for kt in range(4, 8):
            emit_vt(kt)
        for oc in range(NCT):
            emit_qk("q", oc, QCH)
        for oc in range(NCT):
            emit_qk("k", oc, 0)
        for i in range(4, NPC):
            emit_stats(i)

        # ---------- stats chain ----------
        # Entirely on the ACT engine (idle in the prologue, and immune
        # to the DVE copy congestion): activation computes
        # func(in*scale + bias) with per-partition AP scale, which gives
        # scalar-scalar multiply via scale=AP. The two tensor-tensor
        # combines that ACT cannot do (dcc, bias_k) go to GPSIMD.
        with tc.high_priority():
            s_sum = st_pool.tile([1, 1], F32, tag="s_sum", name="s_sum")
            s_sq = st_pool.tile([1, 1], F32, tag="s_sq", name="s_sq")
            scr8 = st_pool.tile([1, NPC], F32, tag="scr8", name="scr8")
            scr512 = st_pool.tile([1, QCH], F32, tag="scr512", name="scr512")
            psq = ps_st.tile([1, NPC], F32, tag="psq", name="psq")
            nc.tensor.matmul(psq[:], _f(ones_col[:]), q4[:])
            nc.scalar.activation(scr8[:], psq[:], AF.Copy,
                                 accum_out=s_sq[:])
            nc.scalar.activation(scr512[:], colsum[:], AF.Copy,
                                 accum_out=s_sum[:])

            inv_cn = 1.0 / float(C * N)
            mean_sb = st_pool.tile([1, 1], F32, tag="mean", name="mean")
            nc.scalar.activation(mean_sb[:], s_sum[:], AF.Copy,
                                 scale=inv_cn)
            msq = st_pool.tile([1, 1], F32, tag="msq", name="msq")
            nc.scalar.activation(msq[:], mean_sb[:], AF.Square)
            epsm = st_pool.tile([1, 1], F32, tag="epsm", name="epsm")
            nc.scalar.activation(epsm[:], msq[:], AF.Copy, scale=-1.0,
                                 bias=EPS)
            lnv = st_pool.tile([1, 1], F32, tag="lnv", name="lnv")
            nc.scalar.activation(lnv[:], s_sq[:], AF.Ln, scale=inv_cn,
                                 bias=epsm[:])
            mi_sb = st_pool.tile([1, 5], F32, tag="mi", name="mi")
            inv_c = mi_sb[:, 0:1]
            nc.scalar.activation(inv_c, lnv[:], AF.Exp, scale=-0.5)
            ninv = st_pool.tile([1, 1], F32, tag="ninv", name="ninv")
            nc.scalar.activation(ninv[:], inv_c, AF.Copy, scale=-1.0)
            nc.scalar.activation(mi_sb[:, 1:2], mean_sb[:], AF.Copy,
                                 scale=ninv[:])                  # -mean*inv
            nc.scalar.activation(mi_sb[:, 2:3], inv_c, AF.Copy,
                                 scale=SCALE)                    # S*inv
            nc.scalar.activation(mi_sb[:, 3:4], inv_c, AF.Copy,
                                 scale=mi_sb[:, 2:3])            # S*inv^2
            nc.scalar.activation(mi_sb[:, 4:5], mi_sb[:, 1:2], AF.Copy,
                                 scale=mi_sb[:, 2:3])            # -S*m*inv^2
            ps_bc5 = ps_st.tile([P, 5], F32, tag="ps_bc5", name="ps_bc5")
            nc.tensor.matmul(ps_bc5[:], ones_row[:], mi_sb[:])
            nc.scalar.activation(mi_bc[:], ps_bc5[:], AF.Copy)
            nc.vector.tensor_scalar(_r(inv_row[:]), ones_row[:],
                                    mi_sb[:, 0:1], None, op0=OP.mult)
            minv_neg = mi_bc[:, 1:2]
            si_bc = mi_bc[:, 2:3]
            si2_bc = mi_bc[:, 3:4]
            m2n_bc = mi_bc[:, 4:5]
            for ci in range(NCT):
                nc.gpsimd.tensor_scalar(dcc[ci][:], pc2[ci], minv_neg,
                                        dc1[ci], op0=OP.mult, op1=OP.add)
            # bias_k = S*inv*(h1.x) - S*mean*inv^2*(h2.x), batch 0..15
            # on the DVE (stt exists there; the first exps gate on it),
            # later batches on the idle GPSIMD (no stt -> 3 ops).
            bt = qk_pool.tile([P, 2 * NKT], F32, tag="bt", name="bt")

            def emit_bias(lo, hi):
                nc.gpsimd.tensor_scalar(bt[:, lo:hi], hx3[:, 0, lo:hi],
                                        si_bc, None, op0=OP.mult)
                nc.gpsimd.tensor_scalar(bt[:, 32 + lo : 32 + hi],
                                        hx3[:, 1, lo:hi], m2n_bc, None,
                                        op0=OP.mult)
                nc.gpsimd.tensor_tensor(bias_k[:, lo:hi], bt[:, lo:hi],
                                        bt[:, 32 + lo : 32 + hi], OP.add)

            nc.vector.tensor_scalar(bt[:, 0:16], hx3[:, 0, 0:16],
                                    si_bc, None, op0=OP.mult)
            nc.vector.scalar_tensor_tensor(bias_k[:, 0:16], hx3[:, 1, 0:16],
                                           m2n_bc, bt[:, 0:16],
                                           op0=OP.mult, op1=OP.add)

        p_pre.close()

        # ---------- sweep pools ----------
        p_sw = ExitStack()
        ps_s = p_sw.enter_context(
            tc.tile_pool(name="ps_s", bufs=2, space="PSUM"))
        p_qkv2 = ExitStack()
        ps_qk2 = p_qkv2.enter_context(
            tc.tile_pool(name="ps_qk2", bufs=2, space="PSUM"))

        def emit_vt2(kt):
            lo = kt * P
            pv = ps_qk2.tile([P, C + 2], F32, tag="ps_v2", name="ps_v2")
            for ci in range(NCT):
                nc.tensor.matmul(pv[:], xs(ci, lo, P), wv_ext[ci],
                                 start=(ci == 0), stop=(ci == NCT - 1))
            nc.vector.tensor_copy(hx3[:, :, kt : kt + 1], pv[:, C : C + 2])
            nc.vector.tensor_copy(
                vT[kt // 2][:, (kt % 2) * C : (kt % 2 + 1) * C], pv[:, 0:C])

        def emit_qk2(which, oc, col):
            w = wqA if which == "q" else wkA
            dst, width = (q2, NQ) if which == "q" else (k2, N)
            pk = ps_qk2.tile([P, QCH], F32, tag="ps_qk2t", name="ps_qk2t")
            for ci in range(NCT):
                nc.tensor.matmul(pk[:],
                                 w[ci][:, oc * P : (oc + 1) * P],
                                 xs(ci, col, QCH),
                                 start=(ci == 0), stop=(ci == NCT - 1))
            nc.vector.tensor_copy(
                dst[:, oc * width + col : oc * width + col + QCH], pk[:])

        k3all = k2[:].rearrange("p (j n) -> p j n", j=2)
        q3 = q2[:].rearrange("p (j n) -> p j n", j=2)

        def emit_scores_exp(kt, half):
            """scoresT + 1024-wide exp for (key tile kt, query half).
            High priority: the exp stream is the whole-kernel critical
            path, so its scores matmuls must win PE arbitration over AV
            bursts whenever both are ready."""
            if kt % 2 == 0 and half == 0:
                exp_tiles[kt // 2] = exp_pool.tile(
                    [P, 2 * NQ], FP8, tag="expt", name="expt")
            with tc.high_priority():
                ps = ps_s.tile([P, 2 * QCH], F32, tag="s", name="s")
                k3 = k3all[:, :, kt * P : (kt + 1) * P]
                for qh in range(2):
                    qcol = half * 2 * QCH + qh * QCH
                    nc.tensor.matmul(
                        ps[:, qh * QCH : (qh + 1) * QCH],
                        k3, q3[:, :, qcol : qcol + QCH],
                        skip_group_check=True,
                        perf_mode=mybir.MatmulPerfMode.DoubleRow)
                lo = (kt % 2) * NQ + half * 2 * QCH
                nc.scalar.activation(
                    exp_tiles[kt // 2][:, lo : lo + 2 * QCH],
                    ps[:], AF.Exp, scale=si2_bc, bias=bias_k[:, kt : kt + 1])

        exp_tiles = [None] * (NKT // 2)
        ones3 = ones_fp8[:].rearrange("p (j o) -> p j o", j=2)[:, :, 0:1]

        # AV chain state (one chain at a time; 3 PSUM banks)
        p_ch = ExitStack()
        ch_h = None  # opened lazily at phase-A kt16

        def av_step(qc, p, ph, pd, first, last):
            et3 = exp_tiles[p].rearrange(
                "p (j q) -> p j q", j=2)[:, :, qc * QCH : (qc + 1) * QCH]
            vt3 = vT[p].rearrange("p (j c) -> p j c", j=2)
            for ct in range(NCT):
                nc.tensor.matmul(
                    ph[ct][:], vt3[:, :, ct * P : (ct + 1) * P], et3[:],
                    start=first, stop=last, skip_group_check=True,
                    perf_mode=mybir.MatmulPerfMode.DoubleRow)
            nc.tensor.matmul(
                pd[0:1, :], ones3, et3[:],
                start=first, stop=last, skip_group_check=True,
                perf_mode=mybir.MatmulPerfMode.DoubleRow)

        p_tail = ExitStack()

        with tc.tile_pool(name="att_sb", bufs=2) as att_pool, \
             tc.tile_pool(name="out_sb", bufs=4) as out_pool:

            tail_state = {}

            def tail_stage1(qc, ph, pd, cp_act=False):
                """recip + inv-scaled broadcast (DVE + PE). The
                broadcast lands back in the (now-free) denominator bank,
                so no extra PSUM bank is needed. In the epilogue the
                PSUM->SBUF copy goes to the idle ACT engine instead of
                the DVE, which is the epilogue's critical engine."""
                rec = att_pool.tile([1, QCH], F32, tag="rec", name="rec")
                with nc.allow_low_precision(reason="f32r fp32-width"):
                    nc.vector.reciprocal(_r(rec[:]), pd[0:1, :])
                nc.tensor.matmul(pd[:], _r(inv_row[:]), _r(rec[:]),
                                 skip_group_check=True)
                rec_bc = att_pool.tile([P, QCH], F32, tag="rec_bc",
                                       name="rec_bc")
                if cp_act:
                    nc.scalar.activation(rec_bc[:], pd[:], AF.Copy)
                else:
                    nc.vector.tensor_copy(rec_bc[:], pd[:])
                tail_state[qc] = (ph, rec_bc)

            def tail_stage2(qc):
                """h = ph * (inv/denom) into SBUF; releases the chain."""
                ph, rec_bc = tail_state[qc]
                h_sb = []
                for ct in range(NCT):
                    h = att_pool.tile([P, QCH], F32, tag=f"hsb{ct}",
                                      name=f"hsb{ct}")
                    nc.vector.tensor_tensor(_r(h[:]), ph[ct][:], rec_bc[:],
                                            OP.mult)
                    h_sb.append(h)
                tail_state[qc] = h_sb

            def tail_stage3(qc, oc, po=None):
                """proj GEMM + residual add + store for one oc. The
                epilogue passes explicit idle-bank APs for po so the
                proj GEMMs don't serialize on the single chain po bank."""
                h_sb = tail_state[qc]
                qsl = slice(qc * QCH, (qc + 1) * QCH)
                if po is None:
                    po = ch_h.tile([P, QCH], F32, tag="po", name="po")
                for ci in range(NCT):
                    nc.tensor.matmul(
                        po[:], wpT[ci][:, oc * P : (oc + 1) * P],
                        _r(h_sb[ci][:]),
                        start=(ci == 0), stop=(ci == NCT - 1),
                        skip_group_check=True)
                ot = out_pool.tile([P, QCH], F32, tag="ot", name="ot")
                nc.vector.tensor_tensor(ot[:], po[:], xbd[oc][:, qsl],
                                        OP.add)
                nc.sync.dma_start(out_d.ap()[oc * P : (oc + 1) * P, qsl],
                                  ot[:])

            # ================= phase A (query half 0) =================
            NP2 = NKT // 2
            ph_cur = pd_cur = None
            av_done = 0  # p index consumed for current chain
            for kt in range(NKT):
                # leftover GEMM injections: V^T 8..31 at kt 0..11 (2/kt),
                # K chunks 1..7 at kt 0..6; Q half-1 at kt 2..5; bias
                # batches (GPSIMD) once their hx columns have landed.
                if kt < 12:
                    emit_vt2(8 + 2 * kt)
                    emit_vt2(9 + 2 * kt)
                if kt < 7:
                    for oc in range(NCT):
                        emit_qk2("k", oc, (kt + 1) * QCH)
                if 2 <= kt < 6:
                    j = kt - 2
                    emit_qk2("q", j % 2, 2 * QCH + (j // 2) * QCH)
                if kt == 11:
                    emit_bias(16, 24)
                if kt == 14:
                    emit_bias(24, 32)
                if kt in (18, 20):
                    # residual tiles on the (idle) GPSIMD engine, emitted
                    # here so the scheduler keeps them off the bias path
                    ci = kt // 2 - 9
                    nc.gpsimd.tensor_scalar(xbd[ci][:], _f(xA[ci][:]),
                                            dcc[ci][:], None, op0=OP.add)
                if kt == 16:
                    p_qkv2.close()

                emit_scores_exp(kt, 0)

                if kt >= 17:
                    if kt == 17:
                        ch_h = p_ch.enter_context(
                            tc.tile_pool(name="ps_ch", bufs=1, space="PSUM"))
                        ph_cur = [ch_h.tile([P, QCH], F32, tag=f"h{ct}",
                                            name=f"h{ct}")
                                  for ct in range(NCT)]
                        pd_cur = ch_h.tile([P, QCH], F32, tag="d", name="d")
                    # consume p with exp done (2p+1 <= kt), max 2/step
                    target = min((kt - 1) // 2 + 1, NP2)
                    budget = 2
                    while av_done < target and budget > 0:
                        av_step(0, av_done, ph_cur, pd_cur,
                                av_done == 0, av_done == NP2 - 1)
                        av_done += 1
                        budget -= 1
            # finish qc0 chain (p15 needs the last A exp)
            while av_done < NP2:
                av_step(0, av_done, ph_cur, pd_cur,
                        av_done == 0, av_done == NP2 - 1)
                av_done += 1

            # ================= phase B (query half 1) =================
            tail_stage1(0, ph_cur, pd_cur)
            qc_av = 1        # chain currently running
            av_done = 0
            for kt in range(NKT):
                emit_scores_exp(kt, 1)
                if kt == 0:
                    tail_stage2(0)   # frees the qc0 chain PSUM
                if kt == 1:
                    tail_stage3(0, 0)
                if kt == 2:
                    tail_stage3(0, 1)
                # AV for qc1 (burst; all H0..no, all its exps exist) then
                # qc2 (paced behind the B exp sweep)
                if kt >= 1 and qc_av <= 2:
                    if qc_av == 1:
                        target = NP2
                        budget = 3
                    else:
                        target = min((kt - 1) // 2 + 1, NP2)
                        budget = 4
                    while av_done < target and budget > 0:
                        av_step(qc_av, av_done, ph_cur, pd_cur,
                                av_done == 0, av_done == NP2 - 1)
                        av_done += 1
                        budget -= 1
                    if av_done == NP2:
                        tail_stage1(qc_av, ph_cur, pd_cur,
                                    cp_act=(qc_av == 2))
                        tail_stage2(qc_av)
                        if qc_av == 1:
                            qc_av = 2
                            av_done = 0
                        else:
                            qc_av = 3
                if kt == 10:
                    tail_stage3(1, 0)
                if kt == 11:
                    tail_stage3(1, 1)
            # ================= epilogue: qc2 tail + qc3 ===============
            # qc3's denominator accumulates FIRST (the d bank frees as
            # soon as qc2's rec_bc is copied out), so its reciprocal +
            # broadcast chain overlaps the qc3 ph matmuls; epilogue proj
            # matmuls borrow idle scores banks to avoid po-bank churn.
            if qc_av == 2:
                while av_done < NP2:
                    av_step(2, av_done, ph_cur, pd_cur,
                            av_done == 0, av_done == NP2 - 1)
                    av_done += 1
                tail_stage1(2, ph_cur, pd_cur, cp_act=True)
                tail_stage2(2)

            # qc3's accumulators live in the now-idle scores banks so
            # its AV does not wait for qc2's tail to release the chain;
            # its denominator accumulates first so the reciprocal +
            # broadcast chain overlaps the ph matmuls.
            ph3 = ps_s.tile([P, 2 * QCH], F32, tag="s", name="ph3")
            po2 = ps_s.tile([P, 2 * QCH], F32, tag="s", name="po2")
            pd3 = ch_h.tile([P, QCH], F32, tag="po", name="pd3")

            def av3_pd(p, first, last):
                et3 = exp_tiles[p].rearrange(
                    "p (j q) -> p j q", j=2)[:, :, 3 * QCH : 4 * QCH]
                nc.tensor.matmul(
                    pd3[0:1, :], ones3, et3[:],
                    start=first, stop=last, skip_group_check=True,
                    perf_mode=mybir.MatmulPerfMode.DoubleRow)

            def av3_ph(p, first, last):
                et3 = exp_tiles[p].rearrange(
                    "p (j q) -> p j q", j=2)[:, :, 3 * QCH : 4 * QCH]
                vt3 = vT[p].rearrange("p (j c) -> p j c", j=2)
                for ct in range(NCT):
                    nc.tensor.matmul(
                        ph3[:, ct * QCH : (ct + 1) * QCH],
                        vt3[:, :, ct * P : (ct + 1) * P],
                        et3[:], start=first, stop=last,
                        skip_group_check=True,
                        perf_mode=mybir.MatmulPerfMode.DoubleRow)

            for p in range(NP2):
                av3_pd(p, p == 0, p == NP2 - 1)
            tail_stage1(3, None, pd3, cp_act=True)
            for p in range(NP2):
                av3_ph(p, p == 0, p == NP2 - 1)
            tail_stage3(2, 0, po=po2[:, 0:QCH])
            tail_stage3(2, 1, po=po2[:, QCH : 2 * QCH])
            tail_state[3] = ([ph3[:, 0:QCH], ph3[:, QCH : 2 * QCH]],
                             tail_state[3][1])
            tail_stage2(3)
            tail_stage3(3, 0, po=pd_cur[:])
            tail_stage3(3, 1, po=pd3[:])
            p_tail.close()
            p_ch.close()
        p_sw.close()

    _split_excess_waits(nc)
    return nc


def make_in_maps(x, norm_gamma, norm_beta, qkv_w, qkv_b, proj_w, proj_b):
    f = np.float32
    d = np.float64
    qkv_w = np.asarray(qkv_w, dtype=d)
    qkv_b = np.asarray(qkv_b, dtype=d)
    proj_w = np.asarray(proj_w, dtype=d)
    proj_b = np.asarray(proj_b, dtype=d)
    g = np.asarray(norm_gamma, dtype=d)
    beta = np.asarray(norm_beta, dtype=d)
    Wq, Wk, Wv = qkv_w[0:C], qkv_w[C : 2 * C], qkv_w[2 * C : 3 * C]
    bq, bk, bv = qkv_b[0:C], qkv_b[C : 2 * C], qkv_b[2 * C : 3 * C]

    wqT = (Wq.T * g[:, None])          # [c_in, c_out], rows scaled by gamma
    wkT = (Wk.T * g[:, None])
    wvT = (Wv.T * g[:, None])
    u1 = bq + Wq @ beta
    u2 = Wq @ g
    h1 = wkT @ u1
    h2 = wkT @ u2
    dc1 = proj_w @ (bv + Wv @ beta) + proj_b
    pc2 = proj_w @ (Wv @ g)

    wpack = np.zeros((C, PACKW), dtype=f)
    wpack[:, 0:C] = wvT
    wpack[:, C] = h1
    wpack[:, C + 1] = h2
    wpack[:, 258 : 258 + C] = wqT
    wpack[:, 258 + C : 258 + 2 * C] = wkT
    wpack[:, 258 + 2 * C : 258 + 3 * C] = proj_w.T
    wpack[:, 258 + 3 * C] = dc1
    wpack[:, 259 + 3 * C] = pc2
    wpack = np.ascontiguousarray(wpack)

    in_maps = []
    xf = np.asarray(x, dtype=f).reshape(B, C, N)
    for core in range(8):
        b, h = divmod(core, 2)
        xs = xf[b]
        if h == 1:
            xs = np.concatenate([xs[:, NQ:], xs[:, :NQ]], axis=1)
        in_maps.append({"x": np.ascontiguousarray(xs), "wpack": wpack})
    return in_maps


def assemble_output(results):
    out = np.empty((B, C, N), dtype=np.float32)
    for core in range(8):
        b, h = divmod(core, 2)
        out[b][:, h * NQ : (h + 1) * NQ] = results[core]["out"]
    return out.reshape(B, C, HH, WW, DD)


_PROGRAM = None
_N_CALLS = 0
_RUNNER = None


def get_program():
    global _PROGRAM
    if _PROGRAM is None:
        _PROGRAM = build_program()
    return _PROGRAM


def _build_cached_runner(nc):
    """Persistent jitted executor (same execution path that
    run_bass_kernel_spmd takes under axon, via bass2jax/PJRT) so repeat
    kernel() calls skip the multi-minute neuronx-cc recompile."""
    import jax
    from jax.experimental.shard_map import shard_map
    from jax.sharding import Mesh, PartitionSpec
    from concourse import bass2jax

    bass2jax.install_neuronx_cc_hook()
    n_cores = 8
    partition_name = (nc.partition_id_tensor.name
                      if nc.partition_id_tensor else None)
    in_names, out_names, out_avals, zero_outs = [], [], [], []
    for alloc in nc.m.functions[0].allocations:
        if not isinstance(alloc, mybir.MemoryLocationSet):
            continue
        name = alloc.memorylocations[0].name
        if alloc.kind == "ExternalInput":
            if name != partition_name:
                in_names.append(name)
        elif alloc.kind == "ExternalOutput":
            out_names.append(name)
            shape = tuple(alloc.tensor_shape)
            dtype = mybir.dt.np(alloc.dtype)
            out_avals.append(jax.core.ShapedArray(shape, dtype))
            zero_outs.append(np.zeros(shape, dtype))
    n_params = len(in_names)
    all_in_names = list(in_names) + list(out_names)
    if partition_name is not None:
        all_in_names.append(partition_name)

    def _body(*args):
        operands = list(args)
        if partition_name is not None:
            operands.append(bass2jax.partition_id_tensor())
        outs = bass2jax._bass_exec_p.bind(
            *operands,
            out_avals=tuple(out_avals),
            in_names=tuple(all_in_names),
            out_names=tuple(out_names),
            lowering_input_output_aliases=(),
            sim_require_finite=True,
            sim_require_nnan=True,
            nc=nc,
        )
        return tuple(outs)

    devices = jax.devices()[:n_cores]
    mesh = Mesh(np.asarray(devices), ("core",))
    n_outs = len(out_names)
    fn = jax.jit(
        shard_map(_body, mesh=mesh,
                  in_specs=(PartitionSpec("core"),) * (n_params + n_outs),
                  out_specs=(PartitionSpec("core"),) * n_outs,
                  check_rep=False),
        keep_unused=True,
    )

    def run(in_maps):
        per_core = [[np.asarray(m[name]) for name in in_names]
                    for m in in_maps]
        concat_in = [
            np.concatenate([per_core[c][i] for c in range(n_cores)], axis=0)
            for i in range(n_params)
        ]
        concat_zeros = [
            np.zeros((n_cores * z.shape[0], *z.shape[1:]), z.dtype)
            for z in zero_outs
        ]
        out_arrs = fn(*concat_in, *concat_zeros)
        return [
            {name: np.asarray(out_arrs[i]).reshape(
                n_cores, *out_avals[i].shape)[c]
             for i, name in enumerate(out_names)}
            for c in range(n_cores)
        ]

    return run


def kernel(x, norm_gamma, norm_beta, qkv_w, qkv_b, proj_w, proj_b):
    global _N_CALLS, _RUNNER
    nc = get_program()
    in_maps = make_in_maps(x, norm_gamma, norm_beta, qkv_w, qkv_b,
                           proj_w, proj_b)
    _N_CALLS += 1
    if _N_CALLS == 1:
        res = run_bass_kernel_spmd(nc, in_maps, core_ids=list(range(8)))
        return assemble_output(res.results)
    if _RUNNER is None:
        _RUNNER = _build_cached_runner(nc)
    return assemble_output(_RUNNER(in_maps))
